# revision 2
# baseline (speedup 1.0000x reference)
"""Fused attention+MoE block on 8 trn2 NeuronCores, v2.

Sharding: tensor-parallel attention (4 q-heads + 1 KV-head per core) as
before, but the attention output partials are ReduceScattered (2 per-batch
collectives) so each core ends up owning 256 tokens of the full hidden
state in feature-major layout. The MoE is then data-parallel: every core
runs all 8 experts densely on its own 256 tokens with fp8e4 DoubleRow
matmuls (2 k-chunks per instruction) and fp8 weights streamed from HBM.
Routing (top-2 over E=8) is computed on an [E, 256] tile with
partition-dim max reductions. Output y = hidden + moe for the owned
tokens; the host just concatenates core slices.
"""

import numpy as np
import ml_dtypes

import concourse.bass as bass
from concourse.bass import _add_dep_helper
import concourse.mybir as mybir
import concourse.tile as tile
from concourse.bass_utils import run_bass_kernel_spmd
from concourse.masks import make_identity
from concourse.vector_clock import ScopedClock

F32 = mybir.dt.float32
F32R = mybir.dt.float32r
F16 = mybir.dt.float16
BF16 = mybir.dt.bfloat16
F8 = mybir.dt.float8e4
U32 = mybir.dt.uint32
AF = mybir.ActivationFunctionType
OP = mybir.AluOpType
PM = mybir.MatmulPerfMode

B, T, C = 2, 1024, 2048
H, KV, D = 32, 8, 128
E, F, TOPK = 8, 768, 2
N = B * T
CK = C // 128          # 16
NBLK = 4               # token blocks of 512
TB = 512
HQ = H // 8            # 4 q heads per core
FK = F // 128          # 6
FGU = 2 * F // 128     # 12
NOWN = 256             # tokens owned per core (128 per batch)
WS = 64.0              # fp8 weight scale
PSC = 16.0             # fp8 prod scale
EPS = 1e-6
N_CORES = 8
BIG = 1e9

# ---------------------------------------------------------------------------
# walrus here rejects >1 sync-wait per instruction; split extras onto NoOps.


class _SplitDrainTileContext(tile.TileContext):
    def _drain_and_barrier(self, tick_clock, wait_clock):
        drain_inst = self.nc.sync.drain()
        wait_clock.add_sem_waits(
            drain_inst.ins, ScopedClock({None: tick_clock.global_clock})
        )
        si = drain_inst.ins.sync_info
        if si is not None and len(si.on_wait) > 1:
            ow = list(si.on_wait)
            drain_inst.ins.sync_info = mybir.SyncInfo(
                on_wait=ow[:1], on_update=list(si.on_update)
            )
            rest = ow[1:]
            while rest:
                extra = self.nc.sync.drain()
                extra.ins.sync_info = mybir.SyncInfo(on_wait=rest[:1], on_update=[])
                rest = rest[1:]
        self.nc.all_engine_barrier()
        assert self.sems is not None
        popped = self.nc._tile_sem_poison_stack.pop()
        assert popped is self._sem_poison
        self.nc.clear_and_free_semaphores(list(self.sems.allocated().values()))
        self.nc.all_engine_barrier()


def _split_multi_waits(nc):
    for bb in nc.main_func.blocks:
        insts = list(bb.instructions)
        out = []
        changed = False
        for ins in insts:
            si = ins.sync_info
            if si is not None and len(si.on_wait) > 1:
                ow = list(si.on_wait)
                for w in ow[:-1]:
                    nop = mybir.InstNoOp(name=f"waitnop-{nc.next_id()}", ins=[], outs=[])
                    nop.engine = ins.engine
                    nop.sync_info = mybir.SyncInfo(on_wait=[w], on_update=[])
                    out.append(nop)
                ins.sync_info = mybir.SyncInfo(
                    on_wait=[ow[-1]], on_update=list(si.on_update)
                )
                changed = True
            out.append(ins)
        if changed:
            bb.instructions = out


# ---------------------------------------------------------------------------


def build_nc(phases='ABC'):
    nc = bass.Bass("TRN2", target_bir_lowering=False, debug=False, num_devices=N_CORES)

    xT = nc.dram_tensor("xT", [128, CK, N], BF16, kind="ExternalInput")
    xown = nc.dram_tensor("xown", [128, CK, NOWN], F16, kind="ExternalInput")
    qw = nc.dram_tensor("qw", [128, CK, HQ * 128], BF16, kind="ExternalInput")
    kw = nc.dram_tensor("kw", [128, CK, 128], BF16, kind="ExternalInput")
    vw = nc.dram_tensor("vw", [128, CK, 128], BF16, kind="ExternalInput")
    ow = nc.dram_tensor("ow", [128, CK, HQ, 128], BF16, kind="ExternalInput")
    gatew = nc.dram_tensor("gatew", [128, CK, E], F16, kind="ExternalInput")
    guw = nc.dram_tensor("guw", [128, E, FGU * 8 * 2 * 128], F8, kind="ExternalInput")
    dww = nc.dram_tensor("dww", [128, CK, E * 3 * 2 * 128], F8, kind="ExternalInput")
    cosb = nc.dram_tensor("cosb", [128, T], BF16, kind="ExternalInput")
    sinb = nc.dram_tensor("sinb", [128, T], BF16, kind="ExternalInput")
    masks = nc.dram_tensor("masks", [128, 4, TB], BF16, kind="ExternalInput")
    rstd1 = nc.dram_tensor("rstd1", [1, N], F32, kind="ExternalInput")
    qnw = nc.dram_tensor("qnw", [128, 1], F32, kind="ExternalInput")
    knw = nc.dram_tensor("knw", [128, 1], F32, kind="ExternalInput")
    protb = nc.dram_tensor("protb", [128, 128], BF16, kind="ExternalInput")
    sel8 = nc.dram_tensor("sel8", [E, E * 128], BF16, kind="ExternalInput")

    y = nc.dram_tensor("y", [128, CK, NOWN], F16, kind="ExternalOutput")

    with _SplitDrainTileContext(nc) as tc:
        with (
            tc.tile_pool(name="const", bufs=1) as cpool,
            tc.tile_pool(name="dram", bufs=1, space="DRAM") as dram,
            tc.tile_pool(name="ps", bufs=1, space="PSUM") as ps,
            tc.tile_pool(name="big", bufs=1) as big,
            tc.tile_pool(name="work", bufs=3) as wk,
        ):
            ab = tc.alloc_tile_pool(name="ab", bufs=1)
            MM = dict(tag="mm", bufs=5)       # f32 [128,TB] psum
            ROW = dict(tag="row", bufs=2)     # f32 [<=8,TB] psum
            TRP = dict(tag="trp", bufs=1)     # bf16 [128,128] psum
            TBF = dict(tag="t512b", bufs=3)   # bf16 [128,TB] transients
            RWF = dict(tag="rowf", bufs=2)    # f32 [1,TB]
            BCS = dict(tag="bcs", bufs=3)     # f32 [128,TB] bcast results
            SM8 = dict(tag="sm8", bufs=2)     # f32 [8,NOWN] small routing tiles
            SM1 = dict(tag="sm1", bufs=3)     # f32 [1,NOWN]
            MC = dict(tag="mc", bufs=2)       # bf16 [128,NOWN] moe transients

            # ---- constants ----
            ident = cpool.tile([128, 128], BF16)
            make_identity(nc, ident)
            ones_bf = cpool.tile([128, 1], BF16)
            nc.vector.memset(ones_bf, 1.0)
            onesrow_f = cpool.tile([1, 128], F32)
            nc.vector.memset(onesrow_f, 1.0)
            onesrow_r = cpool.tile([1, 128], F32R)
            nc.vector.tensor_copy(out=onesrow_r, in_=onesrow_f)
            ones8_f = cpool.tile([1, E], F32)
            nc.vector.memset(ones8_f, 1.0)
            ones8_r = cpool.tile([1, E], F32R)
            nc.vector.tensor_copy(out=ones8_r, in_=ones8_f)
            bias_q = cpool.tile([1, 1], F32)
            nc.vector.memset(bias_q, float(D) * EPS)
            bias_eps = cpool.tile([1, 1], F32)
            nc.vector.memset(bias_eps, EPS)

            qw_sb = ab.tile([128, CK, HQ * 128], BF16)
            nc.sync.dma_start(out=qw_sb, in_=qw[:])
            kw_sb = ab.tile([128, CK, 128], BF16)
            nc.sync.dma_start(out=kw_sb, in_=kw[:])
            vw_sb = ab.tile([128, CK, 128], BF16)
            nc.sync.dma_start(out=vw_sb, in_=vw[:])
            xtb0 = ab.tile([128, CK, TB], BF16, tag="xtb", bufs=1, name="xtb0")
            nc.sync.dma_start(out=xtb0[:, 0:CK // 2, :], in_=xT[:, 0:CK // 2, 0:TB])
            x0dma = nc.sync.dma_start(out=xtb0[:, CK // 2:CK, :],
                                      in_=xT[:, CK // 2:CK, 0:TB])
            prot_sb = cpool.tile([128, 128], BF16)
            nc.sync.dma_start(out=prot_sb, in_=protb[:])
            cos_sb = cpool.tile([128, T], BF16)
            nc.sync.dma_start(out=cos_sb, in_=cosb[:])
            sin_sb = cpool.tile([128, T], BF16)
            nc.sync.dma_start(out=sin_sb, in_=sinb[:])
            masks_sb = cpool.tile([128, 4, TB], BF16)
            qnw_sb = cpool.tile([128, 1], F32)
            nc.sync.dma_start(out=qnw_sb, in_=qnw[:])
            knw_sb = cpool.tile([128, 1], F32)
            nc.sync.dma_start(out=knw_sb, in_=knw[:])
            gatew_sb = cpool.tile([128, CK, E], F16)
            sel8_sb = cpool.tile([E, E * 128], BF16)

            ow_sb = ab.tile([128, CK, HQ, 128], BF16)

            qT_sb = ab.tile([128, HQ, T], BF16)            # per-batch
            kT_sb = ab.tile([128, T], BF16)
            vnat_sb = ab.tile([128, T // 128, 128], BF16)
            xh_sb = big.tile([128, CK, NOWN], F16)          # own hidden (x+attn)
            xn8_sb = big.tile([128, CK, NOWN], F8)
            prod8_sb = big.tile([128, E, FK, NOWN], F8)
            comb_row = big.tile([E, NOWN], F32)             # routing weights

            def deferred_const_loads():
                nc.sync.dma_start(out=ow_sb, in_=ow[:])
                nc.sync.dma_start(out=masks_sb, in_=masks[:])
                nc.sync.dma_start(out=gatew_sb, in_=gatew[:])
                nc.sync.dma_start(out=sel8_sb, in_=sel8[:])

            rsin = [dram.tile([8, 128, CK, 128], F16, name=f"rsin{b}") for b in range(B)]
            rsout = [dram.tile([128, CK, 128], F16, name=f"rsout{b}") for b in range(B)]

            def bcast(row_f32_ap, width=TB, **pool_kw):
                """[1,width] f32 -> SBUF [128,width] f32 via K=1 f32r matmul."""
                rr = wk.tile([1, TB], F32R, tag="rwr", bufs=2)
                nc.vector.tensor_copy(out=rr[:, 0:width], in_=row_f32_ap)
                bc_ps = ps.tile([128, TB], F32, **MM)
                nc.tensor.matmul(bc_ps[:, 0:width], onesrow_r,
                                 rr[:, 0:width], start=True, stop=True)
                kw_ = pool_kw or BCS
                bc = wk.tile([128, TB], F32, **kw_)
                nc.vector.tensor_copy(out=bc[:, 0:width], in_=bc_ps[:, 0:width])
                return bc[:, 0:width]

            def bcast8(row_f32_ap):
                """[1,NOWN] f32 -> SBUF [E,NOWN] f32, exact (fp32 matmul so
                is_equal against the source values still holds bitwise)."""
                p8 = ps.tile([8, TB], F32, **ROW)
                nc.tensor.matmul(p8[0:E, 0:NOWN], ones8_f, row_f32_ap,
                                 start=True, stop=True)
                t8 = wk.tile([E, NOWN], F32, **SM8)
                nc.vector.tensor_copy(out=t8, in_=p8[0:E, 0:NOWN])
                return t8

            def colsum_rstd(feat_ps, scale, bias_ap):
                """rsqrt(scale*colsum(feat^2)+bias) -> [1,TB] f32 row."""
                sq = wk.tile([128, TB], BF16, **TBF)
                nc.scalar.activation(out=sq, in_=feat_ps, func=AF.Square)
                ssum = ps.tile([8, TB], F32, **ROW)
                nc.tensor.matmul(ssum[0:1, :], ones_bf, sq, start=True, stop=True)
                srow = wk.tile([1, TB], F32, **RWF)
                nc.scalar.activation(out=srow, in_=ssum[0:1, :], func=AF.Sqrt,
                                     scale=scale, bias=bias_ap)
                rrow = wk.tile([1, TB], F32, **RWF)
                nc.vector.reciprocal(out=rrow, in_=srow)
                return rrow

            def rope_norm(feat_ps, j, w_sb, rstd_bc, out_ap):
                """out = rope(w * feat * rstd); out_ap bf16 [128,TB].
                rotate_half is a signed partition permutation -> PE matmul."""
                tcol = j * TB
                qhat = wk.tile([128, TB], BF16, **TBF)
                nc.vector.scalar_tensor_tensor(
                    out=qhat, in0=feat_ps, scalar=w_sb, in1=rstd_bc,
                    op0=OP.mult, op1=OP.mult,
                )
                rot_ps = ps.tile([128, TB], F32, **MM)
                nc.tensor.matmul(rot_ps, prot_sb, qhat, start=True, stop=True)
                qc = wk.tile([128, TB], BF16, **TBF)
                nc.vector.tensor_tensor(
                    out=qc, in0=qhat, in1=cos_sb[:, tcol:tcol + TB], op=OP.mult,
                )
                rsm = wk.tile([128, TB], BF16, **TBF)
                nc.vector.tensor_tensor(
                    out=rsm, in0=rot_ps, in1=sin_sb[:, tcol:tcol + TB], op=OP.mult,
                )
                nc.vector.tensor_tensor(out=out_ap, in0=qc, in1=rsm, op=OP.add)

            # =========================== Phase A: QKV ======================
            def phaseA(g):
                b, j = divmod(g, 2)
                n0 = g * TB
                if g == 0:
                    xtb = xtb0
                else:
                    xtb = ab.tile([128, CK, TB], BF16, tag="xtb", bufs=1)
                    nc.sync.dma_start(out=xtb[:, 0:CK // 2, :],
                                      in_=xT[:, 0:CK // 2, n0:n0 + TB])
                    nc.sync.dma_start(out=xtb[:, CK // 2:CK, :],
                                      in_=xT[:, CK // 2:CK, n0:n0 + TB])
                r1row = wk.tile([1, TB], F32, **RWF)
                nc.sync.dma_start(out=r1row, in_=rstd1[:, n0:n0 + TB])
                r1bc = bcast(r1row[:])

                def accum(kind, hd):
                    p = ps.tile([128, TB], F32, **MM)
                    for kc in range(CK):
                        if kind == 'q':
                            lhs = qw_sb[:, kc, hd * 128:(hd + 1) * 128]
                        elif kind == 'k':
                            lhs = kw_sb[:, kc, :]
                        else:
                            lhs = vw_sb[:, kc, :]
                        nc.tensor.matmul(p, lhs, xtb[:, kc, :],
                                         start=(kc == 0), stop=(kc == CK - 1))
                    return p

                def stage2(kind, p):
                    if kind == 'q':
                        return colsum_rstd(p, 1.0, bias_q)
                    if kind == 'k':
                        return colsum_rstd(p, 1.0 / D, bias_eps)
                    return None

                def stage3(kind, hd, p, row):
                    if kind == 'q':
                        qbc = bcast(row[:])
                        rope_norm(p, j, qnw_sb, qbc,
                                  qT_sb[:, hd, j * TB:j * TB + TB])
                    elif kind == 'k':
                        kbc = bcast(row[:])
                        rope_norm(p, j, knw_sb, kbc,
                                  kT_sb[:, j * TB:j * TB + TB])
                    else:
                        vhat = wk.tile([128, TB], BF16, **TBF)
                        nc.vector.tensor_tensor(out=vhat, in0=p, in1=r1bc,
                                                op=OP.mult)
                        for cc in range(TB // 128):
                            vtr = ps.tile([128, 128], BF16, **TRP)
                            nc.tensor.transpose(
                                vtr, vhat[:, cc * 128:(cc + 1) * 128], ident)
                            nc.vector.tensor_copy(
                                out=vnat_sb[:, j * 4 + cc, :], in_=vtr)

                seq = [('q', 0), ('q', 1), ('q', 2), ('q', 3),
                       ('k', None), ('v', None)]
                st = []
                for idx, (kind, hd) in enumerate(seq):
                    p = accum(kind, hd)
                    st.append([kind, hd, p, None])
                    if idx >= 1:
                        st[idx - 1][3] = stage2(st[idx - 1][0], st[idx - 1][2])
                    if idx >= 2:
                        k3, h3, p3, r3 = st[idx - 2]
                        stage3(k3, h3, p3, r3)
                st[-1][3] = stage2(st[-1][0], st[-1][2])
                stage3(st[-2][0], st[-2][1], st[-2][2], st[-2][3])
                stage3(st[-1][0], st[-1][1], st[-1][2], st[-1][3])

            # ====================== Phase B: attention =====================
            def phaseB(g):
                b, j = divmod(g, 2)
                q0 = j * TB
                ntk = 4 * j + 4
                avT = ab.tile([128, HQ, TB], BF16, tag="avT", bufs=1)

                def fin(hd, av_ps, den_ps):
                    dsb = wk.tile([1, TB], F32, **RWF)
                    nc.vector.tensor_copy(out=dsb, in_=den_ps[0:1, :])
                    rec = wk.tile([1, TB], F32, **RWF)
                    nc.vector.reciprocal(out=rec, in_=dsb)
                    rbc = bcast(rec[:])
                    nc.vector.tensor_tensor(out=avT[:, hd, :], in0=av_ps,
                                            in1=rbc, op=OP.mult)

                pend = None
                for hd in range(HQ):
                    av_ps = ps.tile([128, TB], F32, **MM)
                    den_ps = ps.tile([8, TB], F32, **ROW)
                    for i in range(ntk):
                        tk0 = i * 128
                        sc_ps = ps.tile([128, TB], F32, **MM)
                        nc.tensor.matmul(sc_ps, kT_sb[:, tk0:tk0 + 128],
                                         qT_sb[:, hd, q0:q0 + TB],
                                         start=True, stop=True)
                        ex = wk.tile([128, TB], BF16, **TBF)
                        s = i - 4 * j
                        if s < 0:
                            nc.scalar.activation(out=ex, in_=sc_ps, func=AF.Exp)
                        else:
                            ext = wk.tile([128, TB], BF16, **TBF)
                            nc.scalar.activation(out=ext, in_=sc_ps, func=AF.Exp)
                            nc.vector.tensor_tensor(out=ex, in0=ext,
                                                    in1=masks_sb[:, s, :],
                                                    op=OP.mult)
                        nc.tensor.matmul(den_ps[0:1, :], ones_bf, ex,
                                         start=(i == 0), stop=(i == ntk - 1))
                        nc.tensor.matmul(av_ps, vnat_sb[:, i, :], ex,
                                         start=(i == 0), stop=(i == ntk - 1))
                    if pend is not None:
                        fin(*pend)
                    pend = (hd, av_ps, den_ps)
                fin(*pend)
                for mq in range(4):
                    attq = ab.tile([128, 4, TB], F16, tag="attb", bufs=2)
                    for mi in range(4):
                        m = 4 * mq + mi
                        att_ps = ps.tile([128, TB], F32, **MM)
                        for hk in range(HQ):
                            nc.tensor.matmul(att_ps, ow_sb[:, m, hk, :],
                                             avT[:, hk, :], start=(hk == 0),
                                             stop=(hk == HQ - 1))
                        nc.scalar.copy(out=attq[:, mi, :], in_=att_ps)
                    for tg in range(4):
                        nc.sync.dma_start(
                            out=rsin[b][4 * j + tg, :, 4 * mq:4 * mq + 4, :],
                            in_=attq[:, :, tg * 128:(tg + 1) * 128])
                if j == 1:
                    nc.gpsimd.collective_compute(
                        "ReduceScatter", OP.add,
                        replica_groups=[list(range(N_CORES))],
                        ins=[rsin[b].opt()], outs=[rsout[b].opt()],
                    )

            # ========================= Phase C: MoE ========================
            HGU = FK * 8 * 2 * 128    # flat size of one gate (or up) half

            def load_guw_half(e, half, eng=None, after=None):
                t = wk.tile([128, FK * 8, 2, 128], F8, tag="wgu", bufs=4)
                d = (eng or nc.gpsimd).dma_start(
                    out=t,
                    in_=guw[:, e, half * HGU:(half + 1) * HGU].rearrange(
                        "p (a b c) -> p a b c", b=2, c=128))
                if after is not None:
                    _add_dep_helper(d.ins, after.ins, sync=True,
                                    reason="prefetch after startup loads")
                return t

            def load_dww(m, eng=None, after=None):
                t = wk.tile([128, E * 3, 2, 128], F8, tag="wdw", bufs=2)
                d = (eng or nc.sync).dma_start(out=t, in_=dww[:, m, :].rearrange(
                    "p (a b c) -> p a b c", b=2, c=128))
                if after is not None:
                    _add_dep_helper(d.ins, after.ins, sync=True,
                                    reason="prefetch after startup loads")
                return t

            def phaseC(pre_gu, pre_dw, moe):
                # assemble own hidden = attn partial sums (+x) for owned tokens
                for b in range(B):
                    nc.sync.dma_start(out=xh_sb[:, :, 128 * b:128 * b + 128],
                                      in_=rsout[b][:])
                for fq in range(4):
                    xow = wk.tile([128, 4, NOWN], F16, tag="xow", bufs=1)
                    nc.gpsimd.dma_start(out=xow, in_=xown[:, 4 * fq:4 * fq + 4, :])
                    for fi in range(4):
                        fc = 4 * fq + fi
                        nc.vector.tensor_tensor(out=xh_sb[:, fc, :],
                                                in0=xh_sb[:, fc, :],
                                                in1=xow[:, fi, :], op=OP.add)
                lg_ps = ps.tile([8, TB], F32, **ROW)
                den2_ps = ps.tile([8, TB], F32, **ROW)
                for fc in range(CK):
                    nc.tensor.matmul(lg_ps[0:E, 0:NOWN], gatew_sb[:, fc, :],
                                     xh_sb[:, fc, :],
                                     start=(fc == 0), stop=(fc == CK - 1))
                    sq = wk.tile([128, NOWN], BF16, **MC)
                    nc.scalar.activation(out=sq, in_=xh_sb[:, fc, :], func=AF.Square)
                    nc.tensor.matmul(den2_ps[0:1, 0:NOWN], ones_bf, sq,
                                     start=(fc == 0), stop=(fc == CK - 1))
                s2 = wk.tile([1, NOWN], F32, **SM1)
                nc.scalar.activation(out=s2, in_=den2_ps[0:1, 0:NOWN], func=AF.Sqrt,
                                     scale=1.0 / C, bias=bias_eps)
                rstd2 = wk.tile([1, NOWN], F32, tag="rstd2", bufs=1)
                nc.vector.reciprocal(out=rstd2, in_=s2)
                r2bc = bcast(rstd2[:], width=NOWN)
                for fc in range(CK):
                    nc.vector.tensor_tensor(out=xn8_sb[:, fc, :],
                                            in0=xh_sb[:, fc, :],
                                            in1=r2bc, op=OP.mult)

                # ---- top-2 routing on [E, NOWN] ----
                lg = wk.tile([E, NOWN], F32, tag="lg", bufs=1)
                nc.vector.tensor_copy(out=lg, in_=lg_ps[0:E, 0:NOWN])
                m1 = wk.tile([1, NOWN], F32, **SM1)
                nc.gpsimd.tensor_reduce(out=m1, in_=lg, axis=mybir.AxisListType.C,
                                        op=OP.max)
                m1bc = bcast8(m1[:])
                eq1 = wk.tile([E, NOWN], F32, tag="eq1", bufs=1)
                nc.vector.tensor_tensor(out=eq1, in0=lg, in1=m1bc, op=OP.is_equal)
                lg2 = wk.tile([E, NOWN], F32, **SM8)
                nc.vector.scalar_tensor_tensor(out=lg2, in0=eq1, scalar=-BIG,
                                               in1=lg, op0=OP.mult, op1=OP.add)
                m2 = wk.tile([1, NOWN], F32, **SM1)
                nc.gpsimd.tensor_reduce(out=m2, in_=lg2, axis=mybir.AxisListType.C,
                                        op=OP.max)
                m2bc = bcast8(m2[:])
                eq2 = wk.tile([E, NOWN], F32, **SM8)
                nc.vector.tensor_tensor(out=eq2, in0=lg, in1=m2bc, op=OP.is_equal)
                # dlt = (m1-m2)*rstd2 ; w1 = sigmoid(dlt); w2 = 1-w1
                dlt = wk.tile([1, NOWN], F32, **SM1)
                nc.vector.tensor_tensor(out=dlt, in0=m1, in1=m2, op=OP.subtract)
                dlts = wk.tile([1, NOWN], F32, **SM1)
                nc.vector.tensor_tensor(out=dlts, in0=dlt, in1=rstd2, op=OP.mult)
                w1 = wk.tile([1, NOWN], F32, **SM1)
                nc.scalar.activation(out=w1, in_=dlts, func=AF.Sigmoid)
                w1bc = bcast8(w1[:])
                # comb = eq1*w1 + eq2*(1-w1) = (eq1-eq2)*w1 + eq2, in place
                nc.vector.tensor_tensor(out=eq1, in0=eq1, in1=eq2, op=OP.subtract)
                nc.vector.tensor_tensor(out=eq1, in0=eq1, in1=w1bc, op=OP.mult)
                nc.vector.tensor_tensor(out=eq1, in0=eq1, in1=eq2, op=OP.add)
                # scale by PSC/WS (prod fp8 scale / up-weight descale)
                nc.vector.tensor_scalar(out=comb_row, in0=eq1, scalar1=PSC / WS,
                                        scalar2=None, op0=OP.mult)

                # ---- pass 1: gate/up + silu -> prod8 per expert ----
                def load_guw_moe(e, half, eng):
                    t = moe.tile([128, FK * 8, 2, 128], F8, tag="wgu2", bufs=5)
                    eng.dma_start(
                        out=t,
                        in_=guw[:, e, half * HGU:(half + 1) * HGU].rearrange(
                            "p (a b c) -> p a b c", b=2, c=128))
                    return t

                def load_dww_moe(m, eng):
                    t = moe.tile([128, E * 3, 2, 128], F8, tag="wdw2", bufs=2)
                    eng.dma_start(out=t, in_=dww[:, m, :].rearrange(
                        "p (a b c) -> p a b c", b=2, c=128))
                    return t

                comb_bf = wk.tile([E, NOWN], BF16, tag="combbf", bufs=1)
                nc.vector.tensor_copy(out=comb_bf, in_=comb_row)
                for e in range(E):
                    wgg = pre_gu.pop((e, 0), None) or load_guw_moe(e, 0, nc.sync)
                    wgu = pre_gu.pop((e, 1), None) or load_guw_moe(e, 1, nc.scalar)
                    cb_ps = ps.tile([128, TB], F32, **MM)
                    nc.tensor.matmul(cb_ps[:, 0:NOWN],
                                     sel8_sb[:, e * 128:(e + 1) * 128],
                                     comb_bf, start=True, stop=True)
                    cbc = wk.tile([128, NOWN], F32, tag="cbc", bufs=1)
                    nc.vector.tensor_copy(out=cbc, in_=cb_ps[:, 0:NOWN])
                    for f in range(FK):
                        g_ps = ps.tile([128, TB], F32, **MM)
                        for kp in range(8):
                            nc.tensor.matmul(
                                g_ps[:, 0:NOWN], wgg[:, f * 8 + kp, :, :],
                                xn8_sb[:, 2 * kp:2 * kp + 2, :],
                                start=(kp == 0), stop=(kp == 7),
                                perf_mode=PM.DoubleRow,
                            )
                        u_ps = ps.tile([128, TB], F32, **MM)
                        for kp in range(8):
                            nc.tensor.matmul(
                                u_ps[:, 0:NOWN], wgu[:, f * 8 + kp, :, :],
                                xn8_sb[:, 2 * kp:2 * kp + 2, :],
                                start=(kp == 0), stop=(kp == 7),
                                perf_mode=PM.DoubleRow,
                            )
                        sil = wk.tile([128, NOWN], BF16, **MC)
                        nc.scalar.activation(out=sil, in_=g_ps[:, 0:NOWN],
                                             func=AF.Silu, scale=1.0 / WS)
                        ucm = wk.tile([128, NOWN], BF16, **MC)
                        nc.vector.tensor_tensor(out=ucm, in0=u_ps[:, 0:NOWN],
                                                in1=cbc, op=OP.mult)
                        nc.vector.tensor_tensor(out=prod8_sb[:, e, f, :],
                                                in0=sil, in1=ucm, op=OP.mult)

                # ---- pass 2: down proj, accumulate experts in psum ----
                for m in range(CK):
                    wd = pre_dw.pop(m, None) or load_dww_moe(m, nc.sync)
                    eo_ps = ps.tile([128, TB], F32, **MM)
                    for e in range(E):
                        for kp in range(3):
                            nc.tensor.matmul(
                                eo_ps[:, 0:NOWN], wd[:, e * 3 + kp, :, :],
                                prod8_sb[:, e, 2 * kp:2 * kp + 2, :],
                                start=(e == 0 and kp == 0),
                                stop=(e == E - 1 and kp == 2),
                                perf_mode=PM.DoubleRow,
                            )
                    ym = wk.tile([128, NOWN], F16, tag="ymc", bufs=2)
                    nc.vector.scalar_tensor_tensor(
                        out=ym, in0=eo_ps[:, 0:NOWN],
                        scalar=1.0 / (WS * PSC), in1=xh_sb[:, m, :],
                        op0=OP.mult, op1=OP.add,
                    )
                    nc.sync.dma_start(out=y[:, m, :], in_=ym)

            pre_gu, pre_dw = {}, {}
            for g in range(NBLK):
                if 'A' in phases:
                    phaseA(g)
                if g == 0:
                    deferred_const_loads()
                    if 'C' in phases:
                        # act-queue prefetches: the Act sequencer reaches these
                        # only after A0's first Square, keeping the DMA engines
                        # free for the critical startup loads
                        for e in range(2):
                            for half in range(2):
                                pre_gu[(e, half)] = load_guw_half(
                                    e, half, nc.scalar, after=x0dma)
                        pre_dw[0] = load_dww(0, nc.scalar, after=x0dma)
                        pre_dw[1] = load_dww(1, nc.scalar, after=x0dma)
                if 'B' in phases:
                    phaseB(g)
            ab.release()
            if 'C' in phases:
                with tc.tile_pool(name="moe", bufs=1) as moe:
                    phaseC(pre_gu, pre_dw, moe)

    _split_multi_waits(nc)
    return nc


# ---------------------------------------------------------------------------

_NC_CACHE = {}


def _get_nc():
    if "nc" not in _NC_CACHE:
        _NC_CACHE["nc"] = build_nc()
    return _NC_CACHE["nc"]


def _chunk_pm(a, nchunk):
    """[nchunk*128, free...] -> [128, nchunk, free...]"""
    return np.ascontiguousarray(
        a.reshape(nchunk, 128, *a.shape[1:]).transpose(1, 0, *range(2, a.ndim + 1))
    )


def prepare_in_maps(x, cos, sin, ln1_w, q_w, k_w, v_w, o_w, qn_w, kn_w, ln2_w,
                    gate_w, gate_up_w, down_w):
    bf = ml_dtypes.bfloat16
    f8 = ml_dtypes.float8_e4m3
    x = np.asarray(x, dtype=np.float32)
    x_flat = x.reshape(N, C)

    xT = _chunk_pm(np.ascontiguousarray(x_flat.T).astype(bf), CK)
    rstd1 = (1.0 / np.sqrt((x_flat.astype(np.float64) ** 2).mean(axis=1) + EPS)
             ).astype(np.float32)[None, :]

    ln1 = np.asarray(ln1_w, dtype=np.float32)[:, None]
    ln2 = np.asarray(ln2_w, dtype=np.float32)[:, None]
    qwf = np.asarray(q_w, dtype=np.float32) * ln1
    kwf = np.asarray(k_w, dtype=np.float32) * ln1
    vwf = np.asarray(v_w, dtype=np.float32) * ln1
    gatewf = np.asarray(gate_w, dtype=np.float32) * ln2
    guwf = np.asarray(gate_up_w, dtype=np.float32) * ln2[None]    # [E, C, 2F]
    dwf = np.asarray(down_w, dtype=np.float32)                    # [E, F, C]
    owf = np.asarray(o_w, dtype=np.float32)

    cos0 = np.asarray(cos, dtype=np.float32)[0]
    sin0 = np.asarray(sin, dtype=np.float32)[0]
    cosT = np.ascontiguousarray(cos0.T).astype(bf)
    sinT = np.ascontiguousarray(sin0.T).astype(bf)
    protm = np.zeros((128, 128), dtype=np.float32)
    for m in range(64):
        protm[m + 64, m] = -1.0
    for m in range(64, 128):
        protm[m - 64, m] = 1.0

    r = np.arange(128)[:, None]
    col = np.arange(TB)[None, :]
    masks = np.stack(
        [(col >= r + 128 * s).astype(bf) for s in range(4)], axis=1
    )

    # fp8 MoE weights, shared across cores
    # guw host layout: [128, E, FGU*8*2*128]; lhsT slice [128, 2, 128] is
    # (grp, kp) with pair index i selecting k-chunk 2kp+i.
    gu6 = (guwf * WS).astype(f8)                       # [E, C, 2F]
    gu_r = gu6.reshape(E, 8, 2, 128, FGU, 128)         # e, kp, i, p, grp, d
    guw_h = np.ascontiguousarray(
        gu_r.transpose(3, 0, 4, 1, 2, 5).reshape(128, E, FGU * 8 * 2 * 128))
    # dww host layout: [128, CK(m), E*3*2*128]; lhsT slice (e, kp) pair i
    # selects f-chunk 2kp+i; partition p = f % 128; d = c within group m.
    dw6 = (dwf * WS).astype(f8)                        # [E, F, C]
    dw_r = dw6.reshape(E, 3, 2, 128, CK, 128)          # e, kp, i, p, m, d
    dww_h = np.ascontiguousarray(
        dw_r.transpose(3, 4, 0, 1, 2, 5).reshape(128, CK, E * 3 * 2 * 128))

    gatew_h = _chunk_pm(gatewf.astype(np.float16), CK)

    in_maps = []
    for c in range(N_CORES):
        oslice = owf[512 * c:512 * (c + 1), :].astype(bf)  # [512, C]
        o4 = oslice.reshape(HQ, 128, CK, 128)              # hk, p, m, d
        ow_h = np.ascontiguousarray(o4.transpose(1, 2, 0, 3))
        # owned tokens: batch b local [128c, 128c+128)
        own_idx = np.concatenate([
            np.arange(b * T + 128 * c, b * T + 128 * (c + 1)) for b in range(B)
        ])
        xo = x_flat[own_idx, :].T                          # [C, 256]
        xown_h = _chunk_pm(np.ascontiguousarray(xo).astype(np.float16), CK)
        in_maps.append({
            "xT": xT,
            "xown": xown_h,
            "qw": _chunk_pm(qwf[:, 512 * c:512 * (c + 1)].astype(bf), CK),
            "kw": _chunk_pm(kwf[:, 128 * c:128 * (c + 1)].astype(bf), CK),
            "vw": _chunk_pm(vwf[:, 128 * c:128 * (c + 1)].astype(bf), CK),
            "ow": ow_h,
            "gatew": gatew_h,
            "guw": guw_h,
            "dww": dww_h,
            "cosb": cosT,
            "sinb": sinT,
            "masks": masks,
            "rstd1": rstd1,
            "qnw": np.asarray(qn_w, dtype=np.float32)[:, None],
            "knw": np.asarray(kn_w, dtype=np.float32)[:, None],
            "protb": protm.astype(bf),
            "sel8": np.kron(np.eye(E, dtype=np.float32),
                            np.ones((1, 128), dtype=np.float32)).astype(bf),
        })

    return in_maps


def combine(ys):
    out = np.zeros((N, C), dtype=np.float32)
    for c in range(N_CORES):
        yc = np.asarray(ys[c], dtype=np.float32)     # [128, CK, 256]
        # yc[p, fc, 128*b + i] -> token b*T + 128*c + i, feature fc*128+p
        feat_major = yc.transpose(1, 0, 2).reshape(C, NOWN)
        for b in range(B):
            toks = slice(b * T + 128 * c, b * T + 128 * (c + 1))
            out[toks, :] = feat_major[:, 128 * b:128 * (b + 1)].T
    return out.reshape(B, T, C)


def kernel(**inputs):
    in_maps = prepare_in_maps(**inputs)
    nc = _get_nc()
    res = run_bass_kernel_spmd(nc, in_maps, core_ids=list(range(N_CORES)))
    return combine([res.results[c]["y"] for c in range(N_CORES)])


# revision 3
# speedup vs baseline: 1.0181x; 1.0181x over previous
"""Fused attention+MoE block on 8 trn2 NeuronCores, v2.

Sharding: tensor-parallel attention (4 q-heads + 1 KV-head per core) as
before, but the attention output partials are ReduceScattered (2 per-batch
collectives) so each core ends up owning 256 tokens of the full hidden
state in feature-major layout. The MoE is then data-parallel: every core
runs all 8 experts densely on its own 256 tokens with fp8e4 DoubleRow
matmuls (2 k-chunks per instruction) and fp8 weights streamed from HBM.
Routing (top-2 over E=8) is computed on an [E, 256] tile with
partition-dim max reductions. Output y = hidden + moe for the owned
tokens; the host just concatenates core slices.
"""

import numpy as np
import ml_dtypes

import concourse.bass as bass
from concourse.bass import _add_dep_helper
import concourse.mybir as mybir
import concourse.tile as tile
from concourse.bass_utils import run_bass_kernel_spmd
from concourse.masks import make_identity
from concourse.vector_clock import ScopedClock

F32 = mybir.dt.float32
F32R = mybir.dt.float32r
F16 = mybir.dt.float16
BF16 = mybir.dt.bfloat16
F8 = mybir.dt.float8e4
U32 = mybir.dt.uint32
AF = mybir.ActivationFunctionType
OP = mybir.AluOpType
PM = mybir.MatmulPerfMode

B, T, C = 2, 1024, 2048
H, KV, D = 32, 8, 128
E, F, TOPK = 8, 768, 2
N = B * T
CK = C // 128          # 16
NBLK = 4               # token blocks of 512
TB = 512
HQ = H // 8            # 4 q heads per core
FK = F // 128          # 6
FGU = 2 * F // 128     # 12
NOWN = 256             # tokens owned per core (128 per batch)
WS = 64.0              # fp8 weight scale
PSC = 16.0             # fp8 prod scale
EPS = 1e-6
N_CORES = 8
BIG = 1e9

# ---------------------------------------------------------------------------
# walrus here rejects >1 sync-wait per instruction; split extras onto NoOps.


class _SplitDrainTileContext(tile.TileContext):
    def _drain_and_barrier(self, tick_clock, wait_clock):
        drain_inst = self.nc.sync.drain()
        wait_clock.add_sem_waits(
            drain_inst.ins, ScopedClock({None: tick_clock.global_clock})
        )
        si = drain_inst.ins.sync_info
        if si is not None and len(si.on_wait) > 1:
            ow = list(si.on_wait)
            drain_inst.ins.sync_info = mybir.SyncInfo(
                on_wait=ow[:1], on_update=list(si.on_update)
            )
            rest = ow[1:]
            while rest:
                extra = self.nc.sync.drain()
                extra.ins.sync_info = mybir.SyncInfo(on_wait=rest[:1], on_update=[])
                rest = rest[1:]
        self.nc.all_engine_barrier()
        assert self.sems is not None
        popped = self.nc._tile_sem_poison_stack.pop()
        assert popped is self._sem_poison
        self.nc.clear_and_free_semaphores(list(self.sems.allocated().values()))
        self.nc.all_engine_barrier()


def _split_multi_waits(nc):
    for bb in nc.main_func.blocks:
        insts = list(bb.instructions)
        out = []
        changed = False
        for ins in insts:
            si = ins.sync_info
            if si is not None and len(si.on_wait) > 1:
                ow = list(si.on_wait)
                for w in ow[:-1]:
                    nop = mybir.InstNoOp(name=f"waitnop-{nc.next_id()}", ins=[], outs=[])
                    nop.engine = ins.engine
                    nop.sync_info = mybir.SyncInfo(on_wait=[w], on_update=[])
                    out.append(nop)
                ins.sync_info = mybir.SyncInfo(
                    on_wait=[ow[-1]], on_update=list(si.on_update)
                )
                changed = True
            out.append(ins)
        if changed:
            bb.instructions = out


# ---------------------------------------------------------------------------


def build_nc(phases='ABC'):
    nc = bass.Bass("TRN2", target_bir_lowering=False, debug=False, num_devices=N_CORES)

    xT = nc.dram_tensor("xT", [128, CK, N], BF16, kind="ExternalInput")
    xown = nc.dram_tensor("xown", [128, CK, NOWN], F16, kind="ExternalInput")
    qw = nc.dram_tensor("qw", [128, CK, HQ * 128], BF16, kind="ExternalInput")
    kw = nc.dram_tensor("kw", [128, CK, 128], BF16, kind="ExternalInput")
    vw = nc.dram_tensor("vw", [128, CK, 128], BF16, kind="ExternalInput")
    ow = nc.dram_tensor("ow", [128, CK, HQ, 128], BF16, kind="ExternalInput")
    gatew = nc.dram_tensor("gatew", [128, CK, E], F16, kind="ExternalInput")
    guw = nc.dram_tensor("guw", [128, E, FGU * 8 * 2 * 128], F8, kind="ExternalInput")
    dww = nc.dram_tensor("dww", [128, CK, E * 3 * 2 * 128], F8, kind="ExternalInput")
    cosb = nc.dram_tensor("cosb", [128, T], BF16, kind="ExternalInput")
    sinb = nc.dram_tensor("sinb", [128, T], BF16, kind="ExternalInput")
    masks = nc.dram_tensor("masks", [128, 4, TB], BF16, kind="ExternalInput")
    rstd1 = nc.dram_tensor("rstd1", [1, N], F32, kind="ExternalInput")
    qnw = nc.dram_tensor("qnw", [128, 1], F32, kind="ExternalInput")
    knw = nc.dram_tensor("knw", [128, 1], F32, kind="ExternalInput")
    protb = nc.dram_tensor("protb", [128, 128], BF16, kind="ExternalInput")
    sel8 = nc.dram_tensor("sel8", [E, E * 128], BF16, kind="ExternalInput")

    y = nc.dram_tensor("y", [128, CK, NOWN], F16, kind="ExternalOutput")

    with _SplitDrainTileContext(nc) as tc:
        with (
            tc.tile_pool(name="const", bufs=1) as cpool,
            tc.tile_pool(name="dram", bufs=1, space="DRAM") as dram,
            tc.tile_pool(name="ps", bufs=1, space="PSUM") as ps,
            tc.tile_pool(name="big", bufs=1) as big,
            tc.tile_pool(name="work", bufs=3) as wk,
        ):
            ab = tc.alloc_tile_pool(name="ab", bufs=1)
            MM = dict(tag="mm", bufs=5)       # f32 [128,TB] psum
            ROW = dict(tag="row", bufs=2)     # f32 [<=8,TB] psum
            TRP = dict(tag="trp", bufs=1)     # bf16 [128,128] psum
            TBF = dict(tag="t512b", bufs=3)   # bf16 [128,TB] transients
            RWF = dict(tag="rowf", bufs=2)    # f32 [1,TB]
            BCS = dict(tag="bcs", bufs=3)     # f32 [128,TB] bcast results
            SM8 = dict(tag="sm8", bufs=2)     # f32 [8,NOWN] small routing tiles
            SM1 = dict(tag="sm1", bufs=3)     # f32 [1,NOWN]
            MC = dict(tag="mc", bufs=2)       # bf16 [128,NOWN] moe transients

            # ---- constants ----
            ident = cpool.tile([128, 128], BF16)
            make_identity(nc, ident)
            ones_bf = cpool.tile([128, 1], BF16)
            nc.vector.memset(ones_bf, 1.0)
            onesrow_f = cpool.tile([1, 128], F32)
            nc.vector.memset(onesrow_f, 1.0)
            onesrow_r = cpool.tile([1, 128], F32R)
            nc.vector.tensor_copy(out=onesrow_r, in_=onesrow_f)
            ones8_f = cpool.tile([1, E], F32)
            nc.vector.memset(ones8_f, 1.0)
            ones8_r = cpool.tile([1, E], F32R)
            nc.vector.tensor_copy(out=ones8_r, in_=ones8_f)
            bias_q = cpool.tile([1, 1], F32)
            nc.vector.memset(bias_q, float(D) * EPS)
            bias_eps = cpool.tile([1, 1], F32)
            nc.vector.memset(bias_eps, EPS)

            qw_sb = ab.tile([128, CK, HQ * 128], BF16)
            nc.sync.dma_start(out=qw_sb, in_=qw[:])
            kw_sb = ab.tile([128, CK, 128], BF16)
            nc.sync.dma_start(out=kw_sb, in_=kw[:])
            vw_sb = ab.tile([128, CK, 128], BF16)
            nc.sync.dma_start(out=vw_sb, in_=vw[:])
            xtb0 = ab.tile([128, CK, TB], BF16, tag="xtb", bufs=1, name="xtb0")
            nc.sync.dma_start(out=xtb0[:, 0:CK // 2, :], in_=xT[:, 0:CK // 2, 0:TB])
            x0dma = nc.sync.dma_start(out=xtb0[:, CK // 2:CK, :],
                                      in_=xT[:, CK // 2:CK, 0:TB])
            prot_sb = cpool.tile([128, 128], BF16)
            nc.sync.dma_start(out=prot_sb, in_=protb[:])
            cos_sb = cpool.tile([128, T], BF16)
            nc.sync.dma_start(out=cos_sb, in_=cosb[:])
            sin_sb = cpool.tile([128, T], BF16)
            nc.sync.dma_start(out=sin_sb, in_=sinb[:])
            masks_sb = cpool.tile([128, 4, TB], BF16)
            qnw_sb = cpool.tile([128, 1], F32)
            nc.sync.dma_start(out=qnw_sb, in_=qnw[:])
            knw_sb = cpool.tile([128, 1], F32)
            nc.sync.dma_start(out=knw_sb, in_=knw[:])
            gatew_sb = cpool.tile([128, CK, E], F16)
            sel8_sb = cpool.tile([E, E * 128], BF16)

            ow_sb = ab.tile([128, CK, HQ, 128], BF16)

            qT_sb = ab.tile([128, HQ, T], BF16)            # per-batch
            kT_sb = ab.tile([128, T], BF16)
            vnat_sb = ab.tile([128, T // 128, 128], BF16)
            xh_sb = big.tile([128, CK, NOWN], F16)          # own hidden (x+attn)
            xn8_sb = big.tile([128, CK, NOWN], F8)
            prod8_sb = big.tile([128, E, FK, NOWN], F8)
            comb_row = big.tile([E, NOWN], F32)             # routing weights

            def deferred_const_loads():
                nc.sync.dma_start(out=ow_sb, in_=ow[:])
                nc.sync.dma_start(out=masks_sb, in_=masks[:])
                nc.sync.dma_start(out=gatew_sb, in_=gatew[:])
                nc.sync.dma_start(out=sel8_sb, in_=sel8[:])

            rsin = [dram.tile([8, 128, CK, 128], F16, name=f"rsin{b}") for b in range(B)]
            rsout = [dram.tile([128, CK, 128], F16, name=f"rsout{b}") for b in range(B)]

            def bcast(row_f32_ap, width=TB, act_copy=False, **pool_kw):
                """[1,width] f32 -> SBUF [128,width] f32 via K=1 f32r matmul."""
                rr = wk.tile([1, TB], F32R, tag="rwr", bufs=1)
                nc.vector.tensor_copy(out=rr[:, 0:width], in_=row_f32_ap)
                bc_ps = ps.tile([128, TB], F32, **MM)
                nc.tensor.matmul(bc_ps[:, 0:width], onesrow_r,
                                 rr[:, 0:width], start=True, stop=True)
                kw_ = pool_kw or BCS
                bc = wk.tile([128, TB], F32, **kw_)
                if act_copy:
                    nc.scalar.copy(out=bc[:, 0:width], in_=bc_ps[:, 0:width])
                else:
                    nc.vector.tensor_copy(out=bc[:, 0:width], in_=bc_ps[:, 0:width])
                return bc[:, 0:width]

            def bcast8(row_f32_ap):
                """[1,NOWN] f32 -> SBUF [E,NOWN] f32, exact (fp32 matmul so
                is_equal against the source values still holds bitwise)."""
                p8 = ps.tile([8, TB], F32, **ROW)
                nc.tensor.matmul(p8[0:E, 0:NOWN], ones8_f, row_f32_ap,
                                 start=True, stop=True)
                t8 = wk.tile([E, NOWN], F32, **SM8)
                nc.vector.tensor_copy(out=t8, in_=p8[0:E, 0:NOWN])
                return t8

            def colsum_rstd(feat_ps, scale, bias_ap):
                """rsqrt(scale*colsum(feat^2)+bias) -> [1,TB] f32 row."""
                sq = wk.tile([128, TB], BF16, **TBF)
                nc.scalar.activation(out=sq, in_=feat_ps, func=AF.Square)
                ssum = ps.tile([8, TB], F32, **ROW)
                nc.tensor.matmul(ssum[0:1, :], ones_bf, sq, start=True, stop=True)
                srow = wk.tile([1, TB], F32, **RWF)
                nc.scalar.activation(out=srow, in_=ssum[0:1, :], func=AF.Sqrt,
                                     scale=scale, bias=bias_ap)
                rrow = wk.tile([1, TB], F32, **RWF)
                nc.vector.reciprocal(out=rrow, in_=srow)
                return rrow

            def rope_norm(feat_sb, j, w_sb, rstd_bc, out_ap):
                """out = rope(w * feat * rstd); out_ap bf16 [128,TB].
                rotate_half is a signed partition permutation -> PE matmul."""
                tcol = j * TB
                qhat = wk.tile([128, TB], BF16, **TBF)
                nc.vector.scalar_tensor_tensor(
                    out=qhat, in0=feat_sb, scalar=w_sb, in1=rstd_bc,
                    op0=OP.mult, op1=OP.mult,
                )
                rot_ps = ps.tile([128, TB], F32, **MM)
                nc.tensor.matmul(rot_ps, prot_sb, qhat, start=True, stop=True)
                qc = wk.tile([128, TB], BF16, **TBF)
                nc.vector.tensor_tensor(
                    out=qc, in0=qhat, in1=cos_sb[:, tcol:tcol + TB], op=OP.mult,
                )
                rsm = wk.tile([128, TB], BF16, **TBF)
                nc.vector.tensor_tensor(
                    out=rsm, in0=rot_ps, in1=sin_sb[:, tcol:tcol + TB], op=OP.mult,
                )
                nc.vector.tensor_tensor(out=out_ap, in0=qc, in1=rsm, op=OP.add)

            # =========================== Phase A: QKV ======================
            def phaseA(g):
                b, j = divmod(g, 2)
                n0 = g * TB
                if g == 0:
                    xtb = xtb0
                else:
                    xtb = ab.tile([128, CK, TB], BF16, tag="xtb", bufs=1)
                    nc.sync.dma_start(out=xtb[:, 0:CK // 2, :],
                                      in_=xT[:, 0:CK // 2, n0:n0 + TB])
                    nc.sync.dma_start(out=xtb[:, CK // 2:CK, :],
                                      in_=xT[:, CK // 2:CK, n0:n0 + TB])
                r1row = wk.tile([1, TB], F32, **RWF)
                nc.sync.dma_start(out=r1row, in_=rstd1[:, n0:n0 + TB])
                r1bc = bcast(r1row[:], act_copy=True)

                def accum(kind, hd):
                    p = ps.tile([128, TB], F32, **MM)
                    for kc in range(CK):
                        if kind == 'q':
                            lhs = qw_sb[:, kc, hd * 128:(hd + 1) * 128]
                        elif kind == 'k':
                            lhs = kw_sb[:, kc, :]
                        else:
                            lhs = vw_sb[:, kc, :]
                        nc.tensor.matmul(p, lhs, xtb[:, kc, :],
                                         start=(kc == 0), stop=(kc == CK - 1))
                    return p

                def stage2(kind, p):
                    # free the psum early: rope reads the bf16 SBUF copy
                    if kind == 'q':
                        row = colsum_rstd(p, 1.0, bias_q)
                    elif kind == 'k':
                        row = colsum_rstd(p, 1.0 / D, bias_eps)
                    else:
                        row = None
                    fsb = wk.tile([128, TB], BF16, tag="fsb", bufs=2)
                    nc.scalar.copy(out=fsb, in_=p)
                    return (row, fsb)

                def stage3(kind, hd, st2):
                    row, fsb = st2
                    if kind == 'q':
                        qbc = bcast(row[:], act_copy=True)
                        rope_norm(fsb, j, qnw_sb, qbc,
                                  qT_sb[:, hd, j * TB:j * TB + TB])
                    elif kind == 'k':
                        kbc = bcast(row[:], act_copy=True)
                        rope_norm(fsb, j, knw_sb, kbc,
                                  kT_sb[:, j * TB:j * TB + TB])
                    else:
                        vhat = wk.tile([128, TB], BF16, **TBF)
                        nc.vector.tensor_tensor(out=vhat, in0=fsb, in1=r1bc,
                                                op=OP.mult)
                        for cc in range(TB // 128):
                            vtr = ps.tile([128, 128], BF16, **TRP)
                            nc.tensor.transpose(
                                vtr, vhat[:, cc * 128:(cc + 1) * 128], ident)
                            nc.vector.tensor_copy(
                                out=vnat_sb[:, j * 4 + cc, :], in_=vtr)

                seq = [('q', 0), ('q', 1), ('q', 2), ('q', 3),
                       ('k', None), ('v', None)]
                st = []
                for idx, (kind, hd) in enumerate(seq):
                    p = accum(kind, hd)
                    st.append([kind, hd, p, None])
                    if idx >= 1:
                        st[idx - 1][3] = stage2(st[idx - 1][0], st[idx - 1][2])
                    if idx >= 2:
                        stage3(st[idx - 2][0], st[idx - 2][1], st[idx - 2][3])
                st[-1][3] = stage2(st[-1][0], st[-1][2])
                stage3(st[-2][0], st[-2][1], st[-2][3])
                stage3(st[-1][0], st[-1][1], st[-1][3])

            # ====================== Phase B: attention =====================
            def phaseB(g):
                b, j = divmod(g, 2)
                q0 = j * TB
                ntk = 4 * j + 4
                avT = ab.tile([128, HQ, TB], BF16, tag="avT", bufs=1)

                def fin(hd, av_sb, den_ps):
                    dsb = wk.tile([1, TB], F32, **RWF)
                    nc.vector.tensor_copy(out=dsb, in_=den_ps[0:1, :])
                    rec = wk.tile([1, TB], F32, **RWF)
                    nc.vector.reciprocal(out=rec, in_=dsb)
                    rbc = bcast(rec[:])
                    nc.vector.tensor_tensor(out=avT[:, hd, :], in0=av_sb,
                                            in1=rbc, op=OP.mult)

                pend = None
                for hd in range(HQ):
                    av_ps = ps.tile([128, TB], F32, **MM)
                    den_ps = ps.tile([8, TB], F32, **ROW)
                    for i in range(ntk):
                        tk0 = i * 128
                        sc_ps = ps.tile([128, TB], F32, **MM)
                        nc.tensor.matmul(sc_ps, kT_sb[:, tk0:tk0 + 128],
                                         qT_sb[:, hd, q0:q0 + TB],
                                         start=True, stop=True)
                        ex = wk.tile([128, TB], BF16, **TBF)
                        s = i - 4 * j
                        if s < 0:
                            nc.scalar.activation(out=ex, in_=sc_ps, func=AF.Exp)
                        else:
                            ext = wk.tile([128, TB], BF16, **TBF)
                            nc.scalar.activation(out=ext, in_=sc_ps, func=AF.Exp)
                            nc.vector.tensor_tensor(out=ex, in0=ext,
                                                    in1=masks_sb[:, s, :],
                                                    op=OP.mult)
                        nc.tensor.matmul(den_ps[0:1, :], ones_bf, ex,
                                         start=(i == 0), stop=(i == ntk - 1))
                        nc.tensor.matmul(av_ps, vnat_sb[:, i, :], ex,
                                         start=(i == 0), stop=(i == ntk - 1))
                    # free the av psum early via an Act copy; fin reads SBUF
                    av_sb = wk.tile([128, TB], F32, **BCS)
                    nc.scalar.copy(out=av_sb, in_=av_ps)
                    if pend is not None:
                        fin(*pend)
                    pend = (hd, av_sb, den_ps)
                fin(*pend)
                for mq in range(4):
                    attq = ab.tile([128, 4, TB], F16, tag="attb", bufs=2)
                    for mi in range(4):
                        m = 4 * mq + mi
                        att_ps = ps.tile([128, TB], F32, **MM)
                        for hk in range(HQ):
                            nc.tensor.matmul(att_ps, ow_sb[:, m, hk, :],
                                             avT[:, hk, :], start=(hk == 0),
                                             stop=(hk == HQ - 1))
                        nc.scalar.copy(out=attq[:, mi, :], in_=att_ps)
                    for tg in range(4):
                        nc.sync.dma_start(
                            out=rsin[b][4 * j + tg, :, 4 * mq:4 * mq + 4, :],
                            in_=attq[:, :, tg * 128:(tg + 1) * 128])
                if j == 1:
                    nc.gpsimd.collective_compute(
                        "ReduceScatter", OP.add,
                        replica_groups=[list(range(N_CORES))],
                        ins=[rsin[b].opt()], outs=[rsout[b].opt()],
                    )

            # ========================= Phase C: MoE ========================
            HGU = FK * 8 * 2 * 128    # flat size of one gate (or up) half

            def load_guw_half(e, half, eng=None, after=None):
                t = wk.tile([128, FK * 8, 2, 128], F8, tag="wgu", bufs=4)
                d = (eng or nc.gpsimd).dma_start(
                    out=t,
                    in_=guw[:, e, half * HGU:(half + 1) * HGU].rearrange(
                        "p (a b c) -> p a b c", b=2, c=128))
                if after is not None:
                    _add_dep_helper(d.ins, after.ins, sync=True,
                                    reason="prefetch after startup loads")
                return t

            def load_dww(m, eng=None, after=None):
                t = wk.tile([128, E * 3, 2, 128], F8, tag="wdw", bufs=2)
                d = (eng or nc.sync).dma_start(out=t, in_=dww[:, m, :].rearrange(
                    "p (a b c) -> p a b c", b=2, c=128))
                if after is not None:
                    _add_dep_helper(d.ins, after.ins, sync=True,
                                    reason="prefetch after startup loads")
                return t

            def phaseC(pre_gu, pre_dw, moe):
                # assemble own hidden = attn partial sums (+x) for owned tokens
                for b in range(B):
                    nc.sync.dma_start(out=xh_sb[:, :, 128 * b:128 * b + 128],
                                      in_=rsout[b][:])
                for fq in range(4):
                    xow = wk.tile([128, 4, NOWN], F16, tag="xow", bufs=1)
                    nc.gpsimd.dma_start(out=xow, in_=xown[:, 4 * fq:4 * fq + 4, :])
                    for fi in range(4):
                        fc = 4 * fq + fi
                        nc.vector.tensor_tensor(out=xh_sb[:, fc, :],
                                                in0=xh_sb[:, fc, :],
                                                in1=xow[:, fi, :], op=OP.add)
                lg_ps = ps.tile([8, TB], F32, **ROW)
                den2_ps = ps.tile([8, TB], F32, **ROW)
                for fc in range(CK):
                    nc.tensor.matmul(lg_ps[0:E, 0:NOWN], gatew_sb[:, fc, :],
                                     xh_sb[:, fc, :],
                                     start=(fc == 0), stop=(fc == CK - 1))
                    sq = wk.tile([128, NOWN], BF16, **MC)
                    nc.scalar.activation(out=sq, in_=xh_sb[:, fc, :], func=AF.Square)
                    nc.tensor.matmul(den2_ps[0:1, 0:NOWN], ones_bf, sq,
                                     start=(fc == 0), stop=(fc == CK - 1))
                s2 = wk.tile([1, NOWN], F32, **SM1)
                nc.scalar.activation(out=s2, in_=den2_ps[0:1, 0:NOWN], func=AF.Sqrt,
                                     scale=1.0 / C, bias=bias_eps)
                rstd2 = wk.tile([1, NOWN], F32, tag="rstd2", bufs=1)
                nc.vector.reciprocal(out=rstd2, in_=s2)
                r2bc = bcast(rstd2[:], width=NOWN)
                for fc in range(CK):
                    nc.vector.tensor_tensor(out=xn8_sb[:, fc, :],
                                            in0=xh_sb[:, fc, :],
                                            in1=r2bc, op=OP.mult)

                # ---- top-2 routing on [E, NOWN] ----
                lg = wk.tile([E, NOWN], F32, tag="lg", bufs=1)
                nc.vector.tensor_copy(out=lg, in_=lg_ps[0:E, 0:NOWN])
                m1 = wk.tile([1, NOWN], F32, **SM1)
                nc.gpsimd.tensor_reduce(out=m1, in_=lg, axis=mybir.AxisListType.C,
                                        op=OP.max)
                m1bc = bcast8(m1[:])
                eq1 = wk.tile([E, NOWN], F32, tag="eq1", bufs=1)
                nc.vector.tensor_tensor(out=eq1, in0=lg, in1=m1bc, op=OP.is_equal)
                lg2 = wk.tile([E, NOWN], F32, **SM8)
                nc.vector.scalar_tensor_tensor(out=lg2, in0=eq1, scalar=-BIG,
                                               in1=lg, op0=OP.mult, op1=OP.add)
                m2 = wk.tile([1, NOWN], F32, **SM1)
                nc.gpsimd.tensor_reduce(out=m2, in_=lg2, axis=mybir.AxisListType.C,
                                        op=OP.max)
                m2bc = bcast8(m2[:])
                eq2 = wk.tile([E, NOWN], F32, **SM8)
                nc.vector.tensor_tensor(out=eq2, in0=lg, in1=m2bc, op=OP.is_equal)
                # dlt = (m1-m2)*rstd2 ; w1 = sigmoid(dlt); w2 = 1-w1
                dlt = wk.tile([1, NOWN], F32, **SM1)
                nc.vector.tensor_tensor(out=dlt, in0=m1, in1=m2, op=OP.subtract)
                dlts = wk.tile([1, NOWN], F32, **SM1)
                nc.vector.tensor_tensor(out=dlts, in0=dlt, in1=rstd2, op=OP.mult)
                w1 = wk.tile([1, NOWN], F32, **SM1)
                nc.scalar.activation(out=w1, in_=dlts, func=AF.Sigmoid)
                w1bc = bcast8(w1[:])
                # comb = eq1*w1 + eq2*(1-w1) = (eq1-eq2)*w1 + eq2, in place
                nc.vector.tensor_tensor(out=eq1, in0=eq1, in1=eq2, op=OP.subtract)
                nc.vector.tensor_tensor(out=eq1, in0=eq1, in1=w1bc, op=OP.mult)
                nc.vector.tensor_tensor(out=eq1, in0=eq1, in1=eq2, op=OP.add)
                # scale by PSC/WS (prod fp8 scale / up-weight descale)
                nc.vector.tensor_scalar(out=comb_row, in0=eq1, scalar1=PSC / WS,
                                        scalar2=None, op0=OP.mult)

                # ---- pass 1: gate/up + silu -> prod8 per expert ----
                def load_guw_moe(e, half, eng):
                    t = moe.tile([128, FK * 8, 2, 128], F8, tag="wgu2", bufs=5)
                    eng.dma_start(
                        out=t,
                        in_=guw[:, e, half * HGU:(half + 1) * HGU].rearrange(
                            "p (a b c) -> p a b c", b=2, c=128))
                    return t

                def load_dww_moe(m, eng):
                    t = moe.tile([128, E * 3, 2, 128], F8, tag="wdw2", bufs=3)
                    eng.dma_start(out=t, in_=dww[:, m, :].rearrange(
                        "p (a b c) -> p a b c", b=2, c=128))
                    return t

                comb_bf = wk.tile([E, NOWN], BF16, tag="combbf", bufs=1)
                nc.vector.tensor_copy(out=comb_bf, in_=comb_row)
                for e in range(E):
                    wgg = pre_gu.pop((e, 0), None) or load_guw_moe(e, 0, nc.sync)
                    wgu = pre_gu.pop((e, 1), None) or load_guw_moe(e, 1, nc.scalar)
                    cb_ps = ps.tile([128, TB], F32, **MM)
                    nc.tensor.matmul(cb_ps[:, 0:NOWN],
                                     sel8_sb[:, e * 128:(e + 1) * 128],
                                     comb_bf, start=True, stop=True)
                    cbc = wk.tile([128, NOWN], F32, tag="cbc", bufs=1)
                    nc.vector.tensor_copy(out=cbc, in_=cb_ps[:, 0:NOWN])
                    for f in range(FK):
                        g_ps = ps.tile([128, TB], F32, **MM)
                        for kp in range(8):
                            nc.tensor.matmul(
                                g_ps[:, 0:NOWN], wgg[:, f * 8 + kp, :, :],
                                xn8_sb[:, 2 * kp:2 * kp + 2, :],
                                start=(kp == 0), stop=(kp == 7),
                                perf_mode=PM.DoubleRow,
                            )
                        u_ps = ps.tile([128, TB], F32, **MM)
                        for kp in range(8):
                            nc.tensor.matmul(
                                u_ps[:, 0:NOWN], wgu[:, f * 8 + kp, :, :],
                                xn8_sb[:, 2 * kp:2 * kp + 2, :],
                                start=(kp == 0), stop=(kp == 7),
                                perf_mode=PM.DoubleRow,
                            )
                        sil = wk.tile([128, NOWN], BF16, **MC)
                        nc.scalar.activation(out=sil, in_=g_ps[:, 0:NOWN],
                                             func=AF.Silu, scale=1.0 / WS)
                        ucm = wk.tile([128, NOWN], BF16, **MC)
                        nc.vector.tensor_tensor(out=ucm, in0=u_ps[:, 0:NOWN],
                                                in1=cbc, op=OP.mult)
                        nc.vector.tensor_tensor(out=prod8_sb[:, e, f, :],
                                                in0=sil, in1=ucm, op=OP.mult)

                # ---- pass 2: down proj, accumulate experts in psum ----
                for m in range(CK):
                    wd = pre_dw.pop(m, None) or load_dww_moe(m, nc.sync)
                    eo_ps = ps.tile([128, TB], F32, **MM)
                    for e in range(E):
                        for kp in range(3):
                            nc.tensor.matmul(
                                eo_ps[:, 0:NOWN], wd[:, e * 3 + kp, :, :],
                                prod8_sb[:, e, 2 * kp:2 * kp + 2, :],
                                start=(e == 0 and kp == 0),
                                stop=(e == E - 1 and kp == 2),
                                perf_mode=PM.DoubleRow,
                            )
                    ym = wk.tile([128, NOWN], F16, tag="ymc", bufs=2)
                    nc.vector.scalar_tensor_tensor(
                        out=ym, in0=eo_ps[:, 0:NOWN],
                        scalar=1.0 / (WS * PSC), in1=xh_sb[:, m, :],
                        op0=OP.mult, op1=OP.add,
                    )
                    nc.sync.dma_start(out=y[:, m, :], in_=ym)

            pre_gu, pre_dw = {}, {}
            for g in range(NBLK):
                if 'A' in phases:
                    phaseA(g)
                if g == 0:
                    deferred_const_loads()
                    if 'C' in phases:
                        # act-queue prefetches: the Act sequencer reaches these
                        # only after A0's first Square, keeping the DMA engines
                        # free for the critical startup loads
                        for e in range(2):
                            for half in range(2):
                                pre_gu[(e, half)] = load_guw_half(
                                    e, half, nc.scalar, after=x0dma)
                        pre_dw[0] = load_dww(0, nc.scalar, after=x0dma)
                        pre_dw[1] = load_dww(1, nc.scalar, after=x0dma)
                if 'B' in phases:
                    phaseB(g)
            ab.release()
            if 'C' in phases:
                with tc.tile_pool(name="moe", bufs=1) as moe:
                    phaseC(pre_gu, pre_dw, moe)

    _split_multi_waits(nc)
    return nc


# ---------------------------------------------------------------------------

_NC_CACHE = {}


def _get_nc():
    if "nc" not in _NC_CACHE:
        _NC_CACHE["nc"] = build_nc()
    return _NC_CACHE["nc"]


def _chunk_pm(a, nchunk):
    """[nchunk*128, free...] -> [128, nchunk, free...]"""
    return np.ascontiguousarray(
        a.reshape(nchunk, 128, *a.shape[1:]).transpose(1, 0, *range(2, a.ndim + 1))
    )


def prepare_in_maps(x, cos, sin, ln1_w, q_w, k_w, v_w, o_w, qn_w, kn_w, ln2_w,
                    gate_w, gate_up_w, down_w):
    bf = ml_dtypes.bfloat16
    f8 = ml_dtypes.float8_e4m3
    x = np.asarray(x, dtype=np.float32)
    x_flat = x.reshape(N, C)

    xT = _chunk_pm(np.ascontiguousarray(x_flat.T).astype(bf), CK)
    rstd1 = (1.0 / np.sqrt((x_flat.astype(np.float64) ** 2).mean(axis=1) + EPS)
             ).astype(np.float32)[None, :]

    ln1 = np.asarray(ln1_w, dtype=np.float32)[:, None]
    ln2 = np.asarray(ln2_w, dtype=np.float32)[:, None]
    qwf = np.asarray(q_w, dtype=np.float32) * ln1
    kwf = np.asarray(k_w, dtype=np.float32) * ln1
    vwf = np.asarray(v_w, dtype=np.float32) * ln1
    gatewf = np.asarray(gate_w, dtype=np.float32) * ln2
    guwf = np.asarray(gate_up_w, dtype=np.float32) * ln2[None]    # [E, C, 2F]
    dwf = np.asarray(down_w, dtype=np.float32)                    # [E, F, C]
    owf = np.asarray(o_w, dtype=np.float32)

    cos0 = np.asarray(cos, dtype=np.float32)[0]
    sin0 = np.asarray(sin, dtype=np.float32)[0]
    cosT = np.ascontiguousarray(cos0.T).astype(bf)
    sinT = np.ascontiguousarray(sin0.T).astype(bf)
    protm = np.zeros((128, 128), dtype=np.float32)
    for m in range(64):
        protm[m + 64, m] = -1.0
    for m in range(64, 128):
        protm[m - 64, m] = 1.0

    r = np.arange(128)[:, None]
    col = np.arange(TB)[None, :]
    masks = np.stack(
        [(col >= r + 128 * s).astype(bf) for s in range(4)], axis=1
    )

    # fp8 MoE weights, shared across cores
    # guw host layout: [128, E, FGU*8*2*128]; lhsT slice [128, 2, 128] is
    # (grp, kp) with pair index i selecting k-chunk 2kp+i.
    gu6 = (guwf * WS).astype(f8)                       # [E, C, 2F]
    gu_r = gu6.reshape(E, 8, 2, 128, FGU, 128)         # e, kp, i, p, grp, d
    guw_h = np.ascontiguousarray(
        gu_r.transpose(3, 0, 4, 1, 2, 5).reshape(128, E, FGU * 8 * 2 * 128))
    # dww host layout: [128, CK(m), E*3*2*128]; lhsT slice (e, kp) pair i
    # selects f-chunk 2kp+i; partition p = f % 128; d = c within group m.
    dw6 = (dwf * WS).astype(f8)                        # [E, F, C]
    dw_r = dw6.reshape(E, 3, 2, 128, CK, 128)          # e, kp, i, p, m, d
    dww_h = np.ascontiguousarray(
        dw_r.transpose(3, 4, 0, 1, 2, 5).reshape(128, CK, E * 3 * 2 * 128))

    gatew_h = _chunk_pm(gatewf.astype(np.float16), CK)

    in_maps = []
    for c in range(N_CORES):
        oslice = owf[512 * c:512 * (c + 1), :].astype(bf)  # [512, C]
        o4 = oslice.reshape(HQ, 128, CK, 128)              # hk, p, m, d
        ow_h = np.ascontiguousarray(o4.transpose(1, 2, 0, 3))
        # owned tokens: batch b local [128c, 128c+128)
        own_idx = np.concatenate([
            np.arange(b * T + 128 * c, b * T + 128 * (c + 1)) for b in range(B)
        ])
        xo = x_flat[own_idx, :].T                          # [C, 256]
        xown_h = _chunk_pm(np.ascontiguousarray(xo).astype(np.float16), CK)
        in_maps.append({
            "xT": xT,
            "xown": xown_h,
            "qw": _chunk_pm(qwf[:, 512 * c:512 * (c + 1)].astype(bf), CK),
            "kw": _chunk_pm(kwf[:, 128 * c:128 * (c + 1)].astype(bf), CK),
            "vw": _chunk_pm(vwf[:, 128 * c:128 * (c + 1)].astype(bf), CK),
            "ow": ow_h,
            "gatew": gatew_h,
            "guw": guw_h,
            "dww": dww_h,
            "cosb": cosT,
            "sinb": sinT,
            "masks": masks,
            "rstd1": rstd1,
            "qnw": np.asarray(qn_w, dtype=np.float32)[:, None],
            "knw": np.asarray(kn_w, dtype=np.float32)[:, None],
            "protb": protm.astype(bf),
            "sel8": np.kron(np.eye(E, dtype=np.float32),
                            np.ones((1, 128), dtype=np.float32)).astype(bf),
        })

    return in_maps


def combine(ys):
    out = np.zeros((N, C), dtype=np.float32)
    for c in range(N_CORES):
        yc = np.asarray(ys[c], dtype=np.float32)     # [128, CK, 256]
        # yc[p, fc, 128*b + i] -> token b*T + 128*c + i, feature fc*128+p
        feat_major = yc.transpose(1, 0, 2).reshape(C, NOWN)
        for b in range(B):
            toks = slice(b * T + 128 * c, b * T + 128 * (c + 1))
            out[toks, :] = feat_major[:, 128 * b:128 * (b + 1)].T
    return out.reshape(B, T, C)


def kernel(**inputs):
    in_maps = prepare_in_maps(**inputs)
    nc = _get_nc()
    res = run_bass_kernel_spmd(nc, in_maps, core_ids=list(range(N_CORES)))
    return combine([res.results[c]["y"] for c in range(N_CORES)])


# revision 4
# speedup vs baseline: 1.0806x; 1.0613x over previous
"""Fused attention+MoE block on 8 trn2 NeuronCores, v2.

Sharding: tensor-parallel attention (4 q-heads + 1 KV-head per core) as
before, but the attention output partials are ReduceScattered (2 per-batch
collectives) so each core ends up owning 256 tokens of the full hidden
state in feature-major layout. The MoE is then data-parallel: every core
runs all 8 experts densely on its own 256 tokens with fp8e4 DoubleRow
matmuls (2 k-chunks per instruction) and fp8 weights streamed from HBM.
Routing (top-2 over E=8) is computed on an [E, 256] tile with
partition-dim max reductions. Output y = hidden + moe for the owned
tokens; the host just concatenates core slices.
"""

import numpy as np
import ml_dtypes

import concourse.bass as bass
from concourse.bass import _add_dep_helper
import concourse.mybir as mybir
import concourse.tile as tile
from concourse.bass_utils import run_bass_kernel_spmd
from concourse.masks import make_identity
from concourse.vector_clock import ScopedClock

F32 = mybir.dt.float32
F32R = mybir.dt.float32r
F16 = mybir.dt.float16
BF16 = mybir.dt.bfloat16
F8 = mybir.dt.float8e4
U32 = mybir.dt.uint32
AF = mybir.ActivationFunctionType
OP = mybir.AluOpType
PM = mybir.MatmulPerfMode

B, T, C = 2, 1024, 2048
H, KV, D = 32, 8, 128
E, F, TOPK = 8, 768, 2
N = B * T
CK = C // 128          # 16
NBLK = 4               # token blocks of 512
TB = 512
HQ = H // 8            # 4 q heads per core
FK = F // 128          # 6
FGU = 2 * F // 128     # 12
NOWN = 256             # tokens owned per core (128 per batch)
WS = 64.0              # fp8 weight scale
PSC = 16.0             # fp8 prod scale
EPS = 1e-6
N_CORES = 8
BIG = 1e9

# ---------------------------------------------------------------------------
# walrus here rejects >1 sync-wait per instruction; split extras onto NoOps.


class _SplitDrainTileContext(tile.TileContext):
    def _drain_and_barrier(self, tick_clock, wait_clock):
        drain_inst = self.nc.sync.drain()
        wait_clock.add_sem_waits(
            drain_inst.ins, ScopedClock({None: tick_clock.global_clock})
        )
        si = drain_inst.ins.sync_info
        if si is not None and len(si.on_wait) > 1:
            ow = list(si.on_wait)
            drain_inst.ins.sync_info = mybir.SyncInfo(
                on_wait=ow[:1], on_update=list(si.on_update)
            )
            rest = ow[1:]
            while rest:
                extra = self.nc.sync.drain()
                extra.ins.sync_info = mybir.SyncInfo(on_wait=rest[:1], on_update=[])
                rest = rest[1:]
        self.nc.all_engine_barrier()
        assert self.sems is not None
        popped = self.nc._tile_sem_poison_stack.pop()
        assert popped is self._sem_poison
        self.nc.clear_and_free_semaphores(list(self.sems.allocated().values()))
        self.nc.all_engine_barrier()


def _split_multi_waits(nc):
    for bb in nc.main_func.blocks:
        insts = list(bb.instructions)
        out = []
        changed = False
        for ins in insts:
            si = ins.sync_info
            if si is not None and len(si.on_wait) > 1:
                ow = list(si.on_wait)
                for w in ow[:-1]:
                    nop = mybir.InstNoOp(name=f"waitnop-{nc.next_id()}", ins=[], outs=[])
                    nop.engine = ins.engine
                    nop.sync_info = mybir.SyncInfo(on_wait=[w], on_update=[])
                    out.append(nop)
                ins.sync_info = mybir.SyncInfo(
                    on_wait=[ow[-1]], on_update=list(si.on_update)
                )
                changed = True
            out.append(ins)
        if changed:
            bb.instructions = out


# ---------------------------------------------------------------------------


def build_nc(phases='ABC'):
    nc = bass.Bass("TRN2", target_bir_lowering=False, debug=False, num_devices=N_CORES)

    xT = nc.dram_tensor("xT", [128, CK, N], BF16, kind="ExternalInput")
    xown = nc.dram_tensor("xown", [128, CK, NOWN], F16, kind="ExternalInput")
    qw = nc.dram_tensor("qw", [128, HQ, CK, 128], BF16, kind="ExternalInput")
    kw = nc.dram_tensor("kw", [128, CK, 128], BF16, kind="ExternalInput")
    vw = nc.dram_tensor("vw", [128, CK, 128], BF16, kind="ExternalInput")
    ow = nc.dram_tensor("ow", [128, CK, HQ, 128], BF16, kind="ExternalInput")
    gatew = nc.dram_tensor("gatew", [128, CK, E], F16, kind="ExternalInput")
    guw = nc.dram_tensor("guw", [128, E, FGU * 8 * 2 * 128], F8, kind="ExternalInput")
    dww = nc.dram_tensor("dww", [128, CK, E * 3 * 2 * 128], F8, kind="ExternalInput")
    cosb = nc.dram_tensor("cosb", [128, T], BF16, kind="ExternalInput")
    sinb = nc.dram_tensor("sinb", [128, T], BF16, kind="ExternalInput")
    masks = nc.dram_tensor("masks", [128, 4, TB], BF16, kind="ExternalInput")
    rstd1 = nc.dram_tensor("rstd1", [1, N], F32, kind="ExternalInput")
    qnw = nc.dram_tensor("qnw", [128, 1], F32, kind="ExternalInput")
    knw = nc.dram_tensor("knw", [128, 1], F32, kind="ExternalInput")
    protb = nc.dram_tensor("protb", [128, 128], BF16, kind="ExternalInput")
    sel8 = nc.dram_tensor("sel8", [E, E * 128], BF16, kind="ExternalInput")

    y = nc.dram_tensor("y", [128, CK, NOWN], F16, kind="ExternalOutput")

    with _SplitDrainTileContext(nc) as tc:
        with (
            tc.tile_pool(name="const", bufs=1) as cpool,
            tc.tile_pool(name="dram", bufs=1, space="DRAM") as dram,
            tc.tile_pool(name="ps", bufs=1, space="PSUM") as ps,
            tc.tile_pool(name="big", bufs=1) as big,
            tc.tile_pool(name="work", bufs=3) as wk,
        ):
            ab = tc.alloc_tile_pool(name="ab", bufs=1)
            MM = dict(tag="mm", bufs=5)       # f32 [128,TB] psum
            ROW = dict(tag="row", bufs=2)     # f32 [<=8,TB] psum
            TRP = dict(tag="trp", bufs=1)     # bf16 [128,128] psum
            TBF = dict(tag="t512b", bufs=3)   # bf16 [128,TB] transients
            RWF = dict(tag="rowf", bufs=2)    # f32 [1,TB]
            BCS = dict(tag="bcs", bufs=3)     # f32 [128,TB] bcast results
            SM8 = dict(tag="sm8", bufs=2)     # f32 [8,NOWN] small routing tiles
            SM1 = dict(tag="sm1", bufs=3)     # f32 [1,NOWN]
            MC = dict(tag="mc", bufs=2)       # bf16 [128,NOWN] moe transients

            # ---- constants ----
            ident = cpool.tile([128, 128], BF16)
            make_identity(nc, ident)
            ones_bf = cpool.tile([128, 1], BF16)
            nc.vector.memset(ones_bf, 1.0)
            onesrow_f = cpool.tile([1, 128], F32)
            nc.vector.memset(onesrow_f, 1.0)
            onesrow_r = cpool.tile([1, 128], F32R)
            nc.vector.tensor_copy(out=onesrow_r, in_=onesrow_f)
            ones8_f = cpool.tile([1, E], F32)
            nc.vector.memset(ones8_f, 1.0)
            ones8_r = cpool.tile([1, E], F32R)
            nc.vector.tensor_copy(out=ones8_r, in_=ones8_f)
            bias_q = cpool.tile([1, 1], F32)
            nc.vector.memset(bias_q, float(D) * EPS)
            bias_eps = cpool.tile([1, 1], F32)
            nc.vector.memset(bias_eps, EPS)

            xtb0 = ab.tile([128, CK, TB], BF16, tag="xtb", bufs=1, name="xtb0")
            nc.sync.dma_start(out=xtb0[:, 0:CK // 2, :], in_=xT[:, 0:CK // 2, 0:TB])
            x0dma = nc.sync.dma_start(out=xtb0[:, CK // 2:CK, :],
                                      in_=xT[:, CK // 2:CK, 0:TB])
            qw_sb = ab.tile([128, HQ, CK, 128], BF16)
            for _hd in range(HQ):
                nc.sync.dma_start(out=qw_sb[:, _hd, :, :], in_=qw[:, _hd, :, :])
            kw_sb = ab.tile([128, CK, 128], BF16)
            nc.sync.dma_start(out=kw_sb, in_=kw[:])
            vw_sb = ab.tile([128, CK, 128], BF16)
            nc.sync.dma_start(out=vw_sb, in_=vw[:])
            prot_sb = cpool.tile([128, 128], BF16)
            nc.sync.dma_start(out=prot_sb, in_=protb[:])
            cos_sb = cpool.tile([128, T], BF16)
            nc.sync.dma_start(out=cos_sb, in_=cosb[:])
            sin_sb = cpool.tile([128, T], BF16)
            nc.sync.dma_start(out=sin_sb, in_=sinb[:])
            masks_sb = cpool.tile([128, 4, TB], BF16)
            qnw_sb = cpool.tile([128, 1], F32)
            nc.sync.dma_start(out=qnw_sb, in_=qnw[:])
            knw_sb = cpool.tile([128, 1], F32)
            lastconst_dma = nc.sync.dma_start(out=knw_sb, in_=knw[:])
            gatew_sb = cpool.tile([128, CK, E], F16)
            sel8_sb = cpool.tile([E, E * 128], BF16)

            ow_sb = ab.tile([128, CK, HQ, 128], BF16)

            qT_sb = ab.tile([128, HQ, T], BF16)            # per-batch
            kT_sb = ab.tile([128, T], BF16)
            vnat_sb = ab.tile([128, T // 128, 128], BF16)
            xh_sb = big.tile([128, CK, NOWN], F16)          # own hidden (x+attn)
            xn8_sb = big.tile([128, CK, NOWN], F8)
            prod8_sb = big.tile([128, E, FK, NOWN], F8)
            comb_row = big.tile([E, NOWN], F32)             # routing weights

            def deferred_const_loads():
                nc.sync.dma_start(out=ow_sb, in_=ow[:])
                nc.sync.dma_start(out=masks_sb, in_=masks[:])
                nc.sync.dma_start(out=gatew_sb, in_=gatew[:])
                nc.sync.dma_start(out=sel8_sb, in_=sel8[:])

            rsin = [dram.tile([8, 128, CK, 128], F16, name=f"rsin{b}") for b in range(B)]
            rsout = [dram.tile([128, CK, 128], F16, name=f"rsout{b}") for b in range(B)]

            def bcast(row_f32_ap, width=TB, act_copy=False, **pool_kw):
                """[1,width] f32 -> SBUF [128,width] f32 via K=1 f32r matmul."""
                rr = wk.tile([1, TB], F32R, tag="rwr", bufs=1)
                nc.vector.tensor_copy(out=rr[:, 0:width], in_=row_f32_ap)
                bc_ps = ps.tile([128, TB], F32, **MM)
                nc.tensor.matmul(bc_ps[:, 0:width], onesrow_r,
                                 rr[:, 0:width], start=True, stop=True)
                kw_ = pool_kw or BCS
                bc = wk.tile([128, TB], F32, **kw_)
                if act_copy:
                    nc.scalar.copy(out=bc[:, 0:width], in_=bc_ps[:, 0:width])
                else:
                    nc.vector.tensor_copy(out=bc[:, 0:width], in_=bc_ps[:, 0:width])
                return bc[:, 0:width]

            def bcast8(row_f32_ap):
                """[1,NOWN] f32 -> SBUF [E,NOWN] f32, exact (fp32 matmul so
                is_equal against the source values still holds bitwise)."""
                p8 = ps.tile([8, TB], F32, **ROW)
                nc.tensor.matmul(p8[0:E, 0:NOWN], ones8_f, row_f32_ap,
                                 start=True, stop=True)
                t8 = wk.tile([E, NOWN], F32, **SM8)
                nc.vector.tensor_copy(out=t8, in_=p8[0:E, 0:NOWN])
                return t8

            def colsum_rstd(feat_ps, scale, bias_ap):
                """rsqrt(scale*colsum(feat^2)+bias) -> [1,TB] f32 row."""
                sq = wk.tile([128, TB], BF16, **TBF)
                nc.scalar.activation(out=sq, in_=feat_ps, func=AF.Square)
                ssum = ps.tile([8, TB], F32, **ROW)
                nc.tensor.matmul(ssum[0:1, :], ones_bf, sq, start=True, stop=True)
                srow = wk.tile([1, TB], F32, **RWF)
                nc.scalar.activation(out=srow, in_=ssum[0:1, :], func=AF.Sqrt,
                                     scale=scale, bias=bias_ap)
                rrow = wk.tile([1, TB], F32, **RWF)
                nc.vector.reciprocal(out=rrow, in_=srow)
                return rrow

            def rope_norm(feat_sb, j, w_sb, rstd_bc, out_ap):
                """out = rope(w * feat * rstd); out_ap bf16 [128,TB].
                rotate_half is a signed partition permutation -> PE matmul."""
                tcol = j * TB
                qhat = wk.tile([128, TB], BF16, **TBF)
                nc.vector.scalar_tensor_tensor(
                    out=qhat, in0=feat_sb, scalar=w_sb, in1=rstd_bc,
                    op0=OP.mult, op1=OP.mult,
                )
                rot_ps = ps.tile([128, TB], F32, **MM)
                nc.tensor.matmul(rot_ps, prot_sb, qhat, start=True, stop=True)
                qc = wk.tile([128, TB], BF16, **TBF)
                nc.vector.tensor_tensor(
                    out=qc, in0=qhat, in1=cos_sb[:, tcol:tcol + TB], op=OP.mult,
                )
                rsm = wk.tile([128, TB], BF16, **TBF)
                nc.vector.tensor_tensor(
                    out=rsm, in0=rot_ps, in1=sin_sb[:, tcol:tcol + TB], op=OP.mult,
                )
                nc.vector.tensor_tensor(out=out_ap, in0=qc, in1=rsm, op=OP.add)

            # =========================== Phase A: QKV ======================
            def phaseA(g):
                b, j = divmod(g, 2)
                n0 = g * TB
                if g == 0:
                    xtb = xtb0
                else:
                    xtb = ab.tile([128, CK, TB], BF16, tag="xtb", bufs=1)
                    nc.sync.dma_start(out=xtb[:, 0:CK // 2, :],
                                      in_=xT[:, 0:CK // 2, n0:n0 + TB])
                    nc.sync.dma_start(out=xtb[:, CK // 2:CK, :],
                                      in_=xT[:, CK // 2:CK, n0:n0 + TB])
                r1row = wk.tile([1, TB], F32, **RWF)
                nc.sync.dma_start(out=r1row, in_=rstd1[:, n0:n0 + TB])
                r1bc = bcast(r1row[:], act_copy=True)

                def accum(kind, hd):
                    p = ps.tile([128, TB], F32, **MM)
                    for kc in range(CK):
                        if kind == 'q':
                            lhs = qw_sb[:, hd, kc, :]
                        elif kind == 'k':
                            lhs = kw_sb[:, kc, :]
                        else:
                            lhs = vw_sb[:, kc, :]
                        nc.tensor.matmul(p, lhs, xtb[:, kc, :],
                                         start=(kc == 0), stop=(kc == CK - 1))
                    return p

                def stage2(kind, p):
                    # free the psum early: rope reads the bf16 SBUF copy
                    if kind == 'q':
                        row = colsum_rstd(p, 1.0, bias_q)
                    elif kind == 'k':
                        row = colsum_rstd(p, 1.0 / D, bias_eps)
                    else:
                        row = None
                    fsb = wk.tile([128, TB], BF16, tag="fsb", bufs=2)
                    nc.scalar.copy(out=fsb, in_=p)
                    return (row, fsb)

                def stage3(kind, hd, st2):
                    row, fsb = st2
                    if kind == 'q':
                        qbc = bcast(row[:], act_copy=True)
                        rope_norm(fsb, j, qnw_sb, qbc,
                                  qT_sb[:, hd, j * TB:j * TB + TB])
                    elif kind == 'k':
                        kbc = bcast(row[:], act_copy=True)
                        rope_norm(fsb, j, knw_sb, kbc,
                                  kT_sb[:, j * TB:j * TB + TB])
                    else:
                        vhat = wk.tile([128, TB], BF16, **TBF)
                        nc.vector.tensor_tensor(out=vhat, in0=fsb, in1=r1bc,
                                                op=OP.mult)
                        for cc in range(TB // 128):
                            vtr = ps.tile([128, 128], BF16, **TRP)
                            nc.tensor.transpose(
                                vtr, vhat[:, cc * 128:(cc + 1) * 128], ident)
                            nc.vector.tensor_copy(
                                out=vnat_sb[:, j * 4 + cc, :], in_=vtr)

                seq = [('q', 0), ('q', 1), ('q', 2), ('q', 3),
                       ('k', None), ('v', None)]
                st = []
                for idx, (kind, hd) in enumerate(seq):
                    p = accum(kind, hd)
                    st.append([kind, hd, p, None])
                    if idx >= 1:
                        st[idx - 1][3] = stage2(st[idx - 1][0], st[idx - 1][2])
                    if idx >= 2:
                        stage3(st[idx - 2][0], st[idx - 2][1], st[idx - 2][3])
                st[-1][3] = stage2(st[-1][0], st[-1][2])
                stage3(st[-2][0], st[-2][1], st[-2][3])
                stage3(st[-1][0], st[-1][1], st[-1][3])

            # ====================== Phase B: attention =====================
            def phaseB(g):
                b, j = divmod(g, 2)
                q0 = j * TB
                ntk = 4 * j + 4
                avT = ab.tile([128, HQ, TB], BF16, tag="avT", bufs=1)

                def fin(hd, av_sb, den_ps):
                    dsb = wk.tile([1, TB], F32, **RWF)
                    nc.vector.tensor_copy(out=dsb, in_=den_ps[0:1, :])
                    rec = wk.tile([1, TB], F32, **RWF)
                    nc.vector.reciprocal(out=rec, in_=dsb)
                    rbc = bcast(rec[:])
                    nc.vector.tensor_tensor(out=avT[:, hd, :], in0=av_sb,
                                            in1=rbc, op=OP.mult)

                pend = None
                for hd in range(HQ):
                    av_ps = ps.tile([128, TB], F32, **MM)
                    den_ps = ps.tile([8, TB], F32, **ROW)
                    for i in range(ntk):
                        tk0 = i * 128
                        s = i - 4 * j
                        # diagonal tiles: columns < 128*s are fully masked, so
                        # restrict score/exp/den/av to the live column range
                        c0 = max(s, 0) * 128
                        w = TB - c0
                        sc_ps = ps.tile([128, TB], F32, **MM)
                        nc.tensor.matmul(sc_ps[:, c0:TB], kT_sb[:, tk0:tk0 + 128],
                                         qT_sb[:, hd, q0 + c0:q0 + TB],
                                         start=True, stop=True,
                                         skip_group_check=True)
                        ex = wk.tile([128, TB], BF16, **TBF)
                        if s < 0:
                            nc.scalar.activation(out=ex, in_=sc_ps, func=AF.Exp)
                        else:
                            ext = wk.tile([128, TB], BF16, **TBF)
                            nc.scalar.activation(out=ext[:, c0:TB],
                                                 in_=sc_ps[:, c0:TB], func=AF.Exp)
                            nc.vector.tensor_tensor(out=ex[:, c0:TB],
                                                    in0=ext[:, c0:TB],
                                                    in1=masks_sb[:, s, c0:TB],
                                                    op=OP.mult)
                        nc.tensor.matmul(den_ps[0:1, c0:TB], ones_bf,
                                         ex[:, c0:TB],
                                         start=(i == 0), stop=(i == ntk - 1),
                                         skip_group_check=True)
                        nc.tensor.matmul(av_ps[:, c0:TB], vnat_sb[:, i, :],
                                         ex[:, c0:TB],
                                         start=(i == 0), stop=(i == ntk - 1),
                                         skip_group_check=True)
                    # free the av psum early via an Act copy; fin reads SBUF
                    av_sb = wk.tile([128, TB], F32, **BCS)
                    nc.scalar.copy(out=av_sb, in_=av_ps)
                    if pend is not None:
                        fin(*pend)
                    pend = (hd, av_sb, den_ps)
                fin(*pend)
                for mq in range(4):
                    attq = ab.tile([128, 4, TB], F16, tag="attb", bufs=2)
                    for mi in range(4):
                        m = 4 * mq + mi
                        att_ps = ps.tile([128, TB], F32, **MM)
                        for hk in range(HQ):
                            nc.tensor.matmul(att_ps, ow_sb[:, m, hk, :],
                                             avT[:, hk, :], start=(hk == 0),
                                             stop=(hk == HQ - 1))
                        nc.scalar.copy(out=attq[:, mi, :], in_=att_ps)
                    for tg in range(4):
                        nc.sync.dma_start(
                            out=rsin[b][4 * j + tg, :, 4 * mq:4 * mq + 4, :],
                            in_=attq[:, :, tg * 128:(tg + 1) * 128])
                if j == 1:
                    nc.gpsimd.collective_compute(
                        "ReduceScatter", OP.add,
                        replica_groups=[list(range(N_CORES))],
                        ins=[rsin[b].opt()], outs=[rsout[b].opt()],
                    )

            # ========================= Phase C: MoE ========================
            HGU = FK * 8 * 2 * 128    # flat size of one gate (or up) half

            def load_guw_half(e, half, eng=None, after=None):
                t = wk.tile([128, FK * 8, 2, 128], F8, tag="wgu", bufs=4)
                d = (eng or nc.gpsimd).dma_start(
                    out=t,
                    in_=guw[:, e, half * HGU:(half + 1) * HGU].rearrange(
                        "p (a b c) -> p a b c", b=2, c=128))
                if after is not None:
                    _add_dep_helper(d.ins, after.ins, sync=True,
                                    reason="prefetch after startup loads")
                return t

            def load_dww(m, eng=None, after=None):
                t = wk.tile([128, E * 3, 2, 128], F8, tag="wdw", bufs=2)
                d = (eng or nc.sync).dma_start(out=t, in_=dww[:, m, :].rearrange(
                    "p (a b c) -> p a b c", b=2, c=128))
                if after is not None:
                    _add_dep_helper(d.ins, after.ins, sync=True,
                                    reason="prefetch after startup loads")
                return t

            def phaseC(pre_gu, pre_dw, moe):
                # assemble own hidden = attn partial sums (+x) for owned tokens
                for b in range(B):
                    nc.sync.dma_start(out=xh_sb[:, :, 128 * b:128 * b + 128],
                                      in_=rsout[b][:])
                for fq in range(4):
                    xow = wk.tile([128, 4, NOWN], F16, tag="xow", bufs=1)
                    nc.gpsimd.dma_start(out=xow, in_=xown[:, 4 * fq:4 * fq + 4, :])
                    for fi in range(4):
                        fc = 4 * fq + fi
                        nc.vector.tensor_tensor(out=xh_sb[:, fc, :],
                                                in0=xh_sb[:, fc, :],
                                                in1=xow[:, fi, :], op=OP.add)
                lg_ps = ps.tile([8, TB], F32, **ROW)
                den2_ps = ps.tile([8, TB], F32, **ROW)
                for fc in range(CK):
                    nc.tensor.matmul(lg_ps[0:E, 0:NOWN], gatew_sb[:, fc, :],
                                     xh_sb[:, fc, :],
                                     start=(fc == 0), stop=(fc == CK - 1))
                    sq = wk.tile([128, NOWN], BF16, **MC)
                    nc.scalar.activation(out=sq, in_=xh_sb[:, fc, :], func=AF.Square)
                    nc.tensor.matmul(den2_ps[0:1, 0:NOWN], ones_bf, sq,
                                     start=(fc == 0), stop=(fc == CK - 1))
                s2 = wk.tile([1, NOWN], F32, **SM1)
                nc.scalar.activation(out=s2, in_=den2_ps[0:1, 0:NOWN], func=AF.Sqrt,
                                     scale=1.0 / C, bias=bias_eps)
                rstd2 = wk.tile([1, NOWN], F32, tag="rstd2", bufs=1)
                nc.vector.reciprocal(out=rstd2, in_=s2)
                r2bc = bcast(rstd2[:], width=NOWN)
                for fc in range(CK):
                    nc.vector.tensor_tensor(out=xn8_sb[:, fc, :],
                                            in0=xh_sb[:, fc, :],
                                            in1=r2bc, op=OP.mult)

                # ---- top-2 routing on [E, NOWN] ----
                lg = wk.tile([E, NOWN], F32, tag="lg", bufs=1)
                nc.vector.tensor_copy(out=lg, in_=lg_ps[0:E, 0:NOWN])
                m1 = wk.tile([1, NOWN], F32, **SM1)
                nc.gpsimd.tensor_reduce(out=m1, in_=lg, axis=mybir.AxisListType.C,
                                        op=OP.max)
                m1bc = bcast8(m1[:])
                eq1 = wk.tile([E, NOWN], F32, tag="eq1", bufs=1)
                nc.vector.tensor_tensor(out=eq1, in0=lg, in1=m1bc, op=OP.is_equal)
                lg2 = wk.tile([E, NOWN], F32, **SM8)
                nc.vector.scalar_tensor_tensor(out=lg2, in0=eq1, scalar=-BIG,
                                               in1=lg, op0=OP.mult, op1=OP.add)
                m2 = wk.tile([1, NOWN], F32, **SM1)
                nc.gpsimd.tensor_reduce(out=m2, in_=lg2, axis=mybir.AxisListType.C,
                                        op=OP.max)
                m2bc = bcast8(m2[:])
                eq2 = wk.tile([E, NOWN], F32, **SM8)
                nc.vector.tensor_tensor(out=eq2, in0=lg, in1=m2bc, op=OP.is_equal)
                # dlt = (m1-m2)*rstd2 ; w1 = sigmoid(dlt); w2 = 1-w1
                dlt = wk.tile([1, NOWN], F32, **SM1)
                nc.vector.tensor_tensor(out=dlt, in0=m1, in1=m2, op=OP.subtract)
                dlts = wk.tile([1, NOWN], F32, **SM1)
                nc.vector.tensor_tensor(out=dlts, in0=dlt, in1=rstd2, op=OP.mult)
                w1 = wk.tile([1, NOWN], F32, **SM1)
                nc.scalar.activation(out=w1, in_=dlts, func=AF.Sigmoid)
                w1bc = bcast8(w1[:])
                # comb = eq1*w1 + eq2*(1-w1) = (eq1-eq2)*w1 + eq2, in place
                nc.vector.tensor_tensor(out=eq1, in0=eq1, in1=eq2, op=OP.subtract)
                nc.vector.tensor_tensor(out=eq1, in0=eq1, in1=w1bc, op=OP.mult)
                nc.vector.tensor_tensor(out=eq1, in0=eq1, in1=eq2, op=OP.add)
                # scale by PSC/WS (prod fp8 scale / up-weight descale)
                nc.vector.tensor_scalar(out=comb_row, in0=eq1, scalar1=PSC / WS,
                                        scalar2=None, op0=OP.mult)

                # ---- pass 1: gate/up + silu -> prod8 per expert ----
                def load_guw_moe(e, half, eng):
                    t = moe.tile([128, FK * 8, 2, 128], F8, tag="wgu2", bufs=5)
                    eng.dma_start(
                        out=t,
                        in_=guw[:, e, half * HGU:(half + 1) * HGU].rearrange(
                            "p (a b c) -> p a b c", b=2, c=128))
                    return t

                def load_dww_moe(m, eng):
                    t = moe.tile([128, E * 3, 2, 128], F8, tag="wdw2", bufs=3)
                    eng.dma_start(out=t, in_=dww[:, m, :].rearrange(
                        "p (a b c) -> p a b c", b=2, c=128))
                    return t

                comb_bf = wk.tile([E, NOWN], BF16, tag="combbf", bufs=1)
                nc.vector.tensor_copy(out=comb_bf, in_=comb_row)
                for e in range(E):
                    wgg = pre_gu.pop((e, 0), None) or load_guw_moe(e, 0, nc.sync)
                    wgu = pre_gu.pop((e, 1), None) or load_guw_moe(e, 1, nc.scalar)
                    cb_ps = ps.tile([128, TB], F32, **MM)
                    nc.tensor.matmul(cb_ps[:, 0:NOWN],
                                     sel8_sb[:, e * 128:(e + 1) * 128],
                                     comb_bf, start=True, stop=True)
                    cbc = wk.tile([128, NOWN], F32, tag="cbc", bufs=1)
                    nc.vector.tensor_copy(out=cbc, in_=cb_ps[:, 0:NOWN])
                    for f in range(FK):
                        g_ps = ps.tile([128, TB], F32, **MM)
                        for kp in range(8):
                            nc.tensor.matmul(
                                g_ps[:, 0:NOWN], wgg[:, f * 8 + kp, :, :],
                                xn8_sb[:, 2 * kp:2 * kp + 2, :],
                                start=(kp == 0), stop=(kp == 7),
                                perf_mode=PM.DoubleRow,
                            )
                        u_ps = ps.tile([128, TB], F32, **MM)
                        for kp in range(8):
                            nc.tensor.matmul(
                                u_ps[:, 0:NOWN], wgu[:, f * 8 + kp, :, :],
                                xn8_sb[:, 2 * kp:2 * kp + 2, :],
                                start=(kp == 0), stop=(kp == 7),
                                perf_mode=PM.DoubleRow,
                            )
                        sil = wk.tile([128, NOWN], BF16, **MC)
                        nc.scalar.activation(out=sil, in_=g_ps[:, 0:NOWN],
                                             func=AF.Silu, scale=1.0 / WS)
                        ucm = wk.tile([128, NOWN], BF16, **MC)
                        nc.vector.tensor_tensor(out=ucm, in0=u_ps[:, 0:NOWN],
                                                in1=cbc, op=OP.mult)
                        nc.vector.tensor_tensor(out=prod8_sb[:, e, f, :],
                                                in0=sil, in1=ucm, op=OP.mult)

                # ---- pass 2: down proj, accumulate experts in psum ----
                for m in range(CK):
                    wd = pre_dw.pop(m, None) or load_dww_moe(m, nc.sync)
                    eo_ps = ps.tile([128, TB], F32, **MM)
                    for e in range(E):
                        for kp in range(3):
                            nc.tensor.matmul(
                                eo_ps[:, 0:NOWN], wd[:, e * 3 + kp, :, :],
                                prod8_sb[:, e, 2 * kp:2 * kp + 2, :],
                                start=(e == 0 and kp == 0),
                                stop=(e == E - 1 and kp == 2),
                                perf_mode=PM.DoubleRow,
                            )
                    ym = wk.tile([128, NOWN], F16, tag="ymc", bufs=2)
                    nc.vector.scalar_tensor_tensor(
                        out=ym, in0=eo_ps[:, 0:NOWN],
                        scalar=1.0 / (WS * PSC), in1=xh_sb[:, m, :],
                        op0=OP.mult, op1=OP.add,
                    )
                    nc.sync.dma_start(out=y[:, m, :], in_=ym)

            pre_gu, pre_dw = {}, {}
            for g in range(NBLK):
                if 'A' in phases:
                    phaseA(g)
                if g == 0:
                    deferred_const_loads()
                    if 'C' in phases:
                        # act-queue prefetches: the Act sequencer reaches these
                        # only after A0's first Square, keeping the DMA engines
                        # free for the critical startup loads
                        for e in range(2):
                            for half in range(2):
                                pre_gu[(e, half)] = load_guw_half(
                                    e, half, nc.scalar, after=lastconst_dma)
                        pre_dw[0] = load_dww(0, nc.scalar, after=lastconst_dma)
                        pre_dw[1] = load_dww(1, nc.scalar, after=lastconst_dma)
                if 'B' in phases:
                    phaseB(g)
            ab.release()
            if 'C' in phases:
                with tc.tile_pool(name="moe", bufs=1) as moe:
                    phaseC(pre_gu, pre_dw, moe)

    _split_multi_waits(nc)
    return nc


# ---------------------------------------------------------------------------

_NC_CACHE = {}


def _get_nc():
    if "nc" not in _NC_CACHE:
        _NC_CACHE["nc"] = build_nc()
    return _NC_CACHE["nc"]


def _chunk_pm(a, nchunk):
    """[nchunk*128, free...] -> [128, nchunk, free...]"""
    return np.ascontiguousarray(
        a.reshape(nchunk, 128, *a.shape[1:]).transpose(1, 0, *range(2, a.ndim + 1))
    )


def prepare_in_maps(x, cos, sin, ln1_w, q_w, k_w, v_w, o_w, qn_w, kn_w, ln2_w,
                    gate_w, gate_up_w, down_w):
    bf = ml_dtypes.bfloat16
    f8 = ml_dtypes.float8_e4m3
    x = np.asarray(x, dtype=np.float32)
    x_flat = x.reshape(N, C)

    xT = _chunk_pm(np.ascontiguousarray(x_flat.T).astype(bf), CK)
    rstd1 = (1.0 / np.sqrt((x_flat.astype(np.float64) ** 2).mean(axis=1) + EPS)
             ).astype(np.float32)[None, :]

    ln1 = np.asarray(ln1_w, dtype=np.float32)[:, None]
    ln2 = np.asarray(ln2_w, dtype=np.float32)[:, None]
    qwf = np.asarray(q_w, dtype=np.float32) * ln1
    kwf = np.asarray(k_w, dtype=np.float32) * ln1
    vwf = np.asarray(v_w, dtype=np.float32) * ln1
    gatewf = np.asarray(gate_w, dtype=np.float32) * ln2
    guwf = np.asarray(gate_up_w, dtype=np.float32) * ln2[None]    # [E, C, 2F]
    dwf = np.asarray(down_w, dtype=np.float32)                    # [E, F, C]
    owf = np.asarray(o_w, dtype=np.float32)

    cos0 = np.asarray(cos, dtype=np.float32)[0]
    sin0 = np.asarray(sin, dtype=np.float32)[0]
    cosT = np.ascontiguousarray(cos0.T).astype(bf)
    sinT = np.ascontiguousarray(sin0.T).astype(bf)
    protm = np.zeros((128, 128), dtype=np.float32)
    for m in range(64):
        protm[m + 64, m] = -1.0
    for m in range(64, 128):
        protm[m - 64, m] = 1.0

    r = np.arange(128)[:, None]
    col = np.arange(TB)[None, :]
    masks = np.stack(
        [(col >= r + 128 * s).astype(bf) for s in range(4)], axis=1
    )

    # fp8 MoE weights, shared across cores
    # guw host layout: [128, E, FGU*8*2*128]; lhsT slice [128, 2, 128] is
    # (grp, kp) with pair index i selecting k-chunk 2kp+i.
    gu6 = (guwf * WS).astype(f8)                       # [E, C, 2F]
    gu_r = gu6.reshape(E, 8, 2, 128, FGU, 128)         # e, kp, i, p, grp, d
    guw_h = np.ascontiguousarray(
        gu_r.transpose(3, 0, 4, 1, 2, 5).reshape(128, E, FGU * 8 * 2 * 128))
    # dww host layout: [128, CK(m), E*3*2*128]; lhsT slice (e, kp) pair i
    # selects f-chunk 2kp+i; partition p = f % 128; d = c within group m.
    dw6 = (dwf * WS).astype(f8)                        # [E, F, C]
    dw_r = dw6.reshape(E, 3, 2, 128, CK, 128)          # e, kp, i, p, m, d
    dww_h = np.ascontiguousarray(
        dw_r.transpose(3, 4, 0, 1, 2, 5).reshape(128, CK, E * 3 * 2 * 128))

    gatew_h = _chunk_pm(gatewf.astype(np.float16), CK)

    in_maps = []
    for c in range(N_CORES):
        oslice = owf[512 * c:512 * (c + 1), :].astype(bf)  # [512, C]
        o4 = oslice.reshape(HQ, 128, CK, 128)              # hk, p, m, d
        ow_h = np.ascontiguousarray(o4.transpose(1, 2, 0, 3))
        # owned tokens: batch b local [128c, 128c+128)
        own_idx = np.concatenate([
            np.arange(b * T + 128 * c, b * T + 128 * (c + 1)) for b in range(B)
        ])
        xo = x_flat[own_idx, :].T                          # [C, 256]
        xown_h = _chunk_pm(np.ascontiguousarray(xo).astype(np.float16), CK)
        in_maps.append({
            "xT": xT,
            "xown": xown_h,
            "qw": np.ascontiguousarray(
                qwf[:, 512 * c:512 * (c + 1)].astype(bf)
                .reshape(CK, 128, HQ, 128).transpose(1, 2, 0, 3)),
            "kw": _chunk_pm(kwf[:, 128 * c:128 * (c + 1)].astype(bf), CK),
            "vw": _chunk_pm(vwf[:, 128 * c:128 * (c + 1)].astype(bf), CK),
            "ow": ow_h,
            "gatew": gatew_h,
            "guw": guw_h,
            "dww": dww_h,
            "cosb": cosT,
            "sinb": sinT,
            "masks": masks,
            "rstd1": rstd1,
            "qnw": np.asarray(qn_w, dtype=np.float32)[:, None],
            "knw": np.asarray(kn_w, dtype=np.float32)[:, None],
            "protb": protm.astype(bf),
            "sel8": np.kron(np.eye(E, dtype=np.float32),
                            np.ones((1, 128), dtype=np.float32)).astype(bf),
        })

    return in_maps


def combine(ys):
    out = np.zeros((N, C), dtype=np.float32)
    for c in range(N_CORES):
        yc = np.asarray(ys[c], dtype=np.float32)     # [128, CK, 256]
        # yc[p, fc, 128*b + i] -> token b*T + 128*c + i, feature fc*128+p
        feat_major = yc.transpose(1, 0, 2).reshape(C, NOWN)
        for b in range(B):
            toks = slice(b * T + 128 * c, b * T + 128 * (c + 1))
            out[toks, :] = feat_major[:, 128 * b:128 * (b + 1)].T
    return out.reshape(B, T, C)


def kernel(**inputs):
    in_maps = prepare_in_maps(**inputs)
    nc = _get_nc()
    res = run_bass_kernel_spmd(nc, in_maps, core_ids=list(range(N_CORES)))
    return combine([res.results[c]["y"] for c in range(N_CORES)])


# revision 5
# speedup vs baseline: 1.0850x; 1.0041x over previous
"""Fused attention+MoE block on 8 trn2 NeuronCores, v2.

Sharding: tensor-parallel attention (4 q-heads + 1 KV-head per core) as
before, but the attention output partials are ReduceScattered (2 per-batch
collectives) so each core ends up owning 256 tokens of the full hidden
state in feature-major layout. The MoE is then data-parallel: every core
runs all 8 experts densely on its own 256 tokens with fp8e4 DoubleRow
matmuls (2 k-chunks per instruction) and fp8 weights streamed from HBM.
Routing (top-2 over E=8) is computed on an [E, 256] tile with
partition-dim max reductions. Output y = hidden + moe for the owned
tokens; the host just concatenates core slices.
"""

import numpy as np
import ml_dtypes

import concourse.bass as bass
from concourse.bass import _add_dep_helper
import concourse.mybir as mybir
import concourse.tile as tile
from concourse.bass_utils import run_bass_kernel_spmd
from concourse.masks import make_identity
from concourse.vector_clock import ScopedClock

F32 = mybir.dt.float32
F32R = mybir.dt.float32r
F16 = mybir.dt.float16
BF16 = mybir.dt.bfloat16
F8 = mybir.dt.float8e4
U32 = mybir.dt.uint32
AF = mybir.ActivationFunctionType
OP = mybir.AluOpType
PM = mybir.MatmulPerfMode

B, T, C = 2, 1024, 2048
H, KV, D = 32, 8, 128
E, F, TOPK = 8, 768, 2
N = B * T
CK = C // 128          # 16
NBLK = 4               # token blocks of 512
TB = 512
HQ = H // 8            # 4 q heads per core
FK = F // 128          # 6
FGU = 2 * F // 128     # 12
NOWN = 256             # tokens owned per core (128 per batch)
WS = 64.0              # fp8 weight scale
PSC = 16.0             # fp8 prod scale
EPS = 1e-6
N_CORES = 8
BIG = 1e9

# ---------------------------------------------------------------------------
# walrus here rejects >1 sync-wait per instruction; split extras onto NoOps.


class _SplitDrainTileContext(tile.TileContext):
    def _drain_and_barrier(self, tick_clock, wait_clock):
        drain_inst = self.nc.sync.drain()
        wait_clock.add_sem_waits(
            drain_inst.ins, ScopedClock({None: tick_clock.global_clock})
        )
        si = drain_inst.ins.sync_info
        if si is not None and len(si.on_wait) > 1:
            ow = list(si.on_wait)
            drain_inst.ins.sync_info = mybir.SyncInfo(
                on_wait=ow[:1], on_update=list(si.on_update)
            )
            rest = ow[1:]
            while rest:
                extra = self.nc.sync.drain()
                extra.ins.sync_info = mybir.SyncInfo(on_wait=rest[:1], on_update=[])
                rest = rest[1:]
        self.nc.all_engine_barrier()
        assert self.sems is not None
        popped = self.nc._tile_sem_poison_stack.pop()
        assert popped is self._sem_poison
        self.nc.clear_and_free_semaphores(list(self.sems.allocated().values()))
        self.nc.all_engine_barrier()


def _split_multi_waits(nc):
    for bb in nc.main_func.blocks:
        insts = list(bb.instructions)
        out = []
        changed = False
        for ins in insts:
            si = ins.sync_info
            if si is not None and len(si.on_wait) > 1:
                ow = list(si.on_wait)
                for w in ow[:-1]:
                    nop = mybir.InstNoOp(name=f"waitnop-{nc.next_id()}", ins=[], outs=[])
                    nop.engine = ins.engine
                    nop.sync_info = mybir.SyncInfo(on_wait=[w], on_update=[])
                    out.append(nop)
                ins.sync_info = mybir.SyncInfo(
                    on_wait=[ow[-1]], on_update=list(si.on_update)
                )
                changed = True
            out.append(ins)
        if changed:
            bb.instructions = out


# ---------------------------------------------------------------------------


def build_nc(phases='ABC'):
    nc = bass.Bass("TRN2", target_bir_lowering=False, debug=False, num_devices=N_CORES)

    xT = nc.dram_tensor("xT", [128, CK, N], BF16, kind="ExternalInput")
    xown = nc.dram_tensor("xown", [128, CK, NOWN], F16, kind="ExternalInput")
    qw = nc.dram_tensor("qw", [128, HQ, CK, 128], BF16, kind="ExternalInput")
    kw = nc.dram_tensor("kw", [128, CK, 128], BF16, kind="ExternalInput")
    vw = nc.dram_tensor("vw", [128, CK, 128], BF16, kind="ExternalInput")
    ow = nc.dram_tensor("ow", [128, CK, HQ, 128], BF16, kind="ExternalInput")
    gatew = nc.dram_tensor("gatew", [128, CK, E], F16, kind="ExternalInput")
    guw = nc.dram_tensor("guw", [128, E, FGU * 8 * 2 * 128], F8, kind="ExternalInput")
    dww = nc.dram_tensor("dww", [128, CK, E * 3 * 2 * 128], F8, kind="ExternalInput")
    cosb = nc.dram_tensor("cosb", [128, T], BF16, kind="ExternalInput")
    sinb = nc.dram_tensor("sinb", [128, T], BF16, kind="ExternalInput")
    masks = nc.dram_tensor("masks", [128, 4, TB], BF16, kind="ExternalInput")
    rstd1 = nc.dram_tensor("rstd1", [1, N], F32, kind="ExternalInput")
    qnw = nc.dram_tensor("qnw", [128, 1], F32, kind="ExternalInput")
    knw = nc.dram_tensor("knw", [128, 1], F32, kind="ExternalInput")
    protb = nc.dram_tensor("protb", [128, 128], BF16, kind="ExternalInput")
    sel8 = nc.dram_tensor("sel8", [E, E * 128], BF16, kind="ExternalInput")

    y = nc.dram_tensor("y", [128, CK, NOWN], F16, kind="ExternalOutput")

    with _SplitDrainTileContext(nc) as tc:
        with (
            tc.tile_pool(name="const", bufs=1) as cpool,
            tc.tile_pool(name="dram", bufs=1, space="DRAM") as dram,
            tc.tile_pool(name="ps", bufs=1, space="PSUM") as ps,
            tc.tile_pool(name="big", bufs=1) as big,
            tc.tile_pool(name="work", bufs=3) as wk,
        ):
            ab = tc.alloc_tile_pool(name="ab", bufs=1)
            MM = dict(tag="mm", bufs=5)       # f32 [128,TB] psum
            ROW = dict(tag="row", bufs=2)     # f32 [<=8,TB] psum
            TRP = dict(tag="trp", bufs=1)     # bf16 [128,128] psum
            TBF = dict(tag="t512b", bufs=3)   # bf16 [128,TB] transients
            RWF = dict(tag="rowf", bufs=2)    # f32 [1,TB]
            BCS = dict(tag="bcs", bufs=3)     # f32 [128,TB] bcast results
            SM8 = dict(tag="sm8", bufs=2)     # f32 [8,NOWN] small routing tiles
            SM1 = dict(tag="sm1", bufs=3)     # f32 [1,NOWN]
            MC = dict(tag="mc", bufs=2)       # bf16 [128,NOWN] moe transients

            # ---- constants ----
            ident = cpool.tile([128, 128], BF16)
            make_identity(nc, ident)
            ones_bf = cpool.tile([128, 1], BF16)
            nc.vector.memset(ones_bf, 1.0)
            onesrow_f = cpool.tile([1, 128], F32)
            nc.vector.memset(onesrow_f, 1.0)
            onesrow_r = cpool.tile([1, 128], F32R)
            nc.vector.tensor_copy(out=onesrow_r, in_=onesrow_f)
            ones8_f = cpool.tile([1, E], F32)
            nc.vector.memset(ones8_f, 1.0)
            ones8_r = cpool.tile([1, E], F32R)
            nc.vector.tensor_copy(out=ones8_r, in_=ones8_f)
            bias_q = cpool.tile([1, 1], F32)
            nc.vector.memset(bias_q, float(D) * EPS)
            bias_eps = cpool.tile([1, 1], F32)
            nc.vector.memset(bias_eps, EPS)
            idf1 = cpool.tile([1, 1], F32)
            nc.vector.memset(idf1, 1.0)

            xtb0 = ab.tile([128, CK, TB], BF16, tag="xtb", bufs=1, name="xtb0")
            nc.sync.dma_start(out=xtb0[:, 0:CK // 2, :], in_=xT[:, 0:CK // 2, 0:TB])
            x0dma = nc.sync.dma_start(out=xtb0[:, CK // 2:CK, :],
                                      in_=xT[:, CK // 2:CK, 0:TB])
            qw_sb = ab.tile([128, HQ, CK, 128], BF16)
            for _hd in range(HQ):
                nc.sync.dma_start(out=qw_sb[:, _hd, :, :], in_=qw[:, _hd, :, :])
            kw_sb = ab.tile([128, CK, 128], BF16)
            nc.sync.dma_start(out=kw_sb, in_=kw[:])
            vw_sb = ab.tile([128, CK, 128], BF16)
            nc.sync.dma_start(out=vw_sb, in_=vw[:])
            prot_sb = cpool.tile([128, 128], BF16)
            nc.sync.dma_start(out=prot_sb, in_=protb[:])
            cos_sb = cpool.tile([128, T], BF16)
            nc.sync.dma_start(out=cos_sb, in_=cosb[:])
            sin_sb = cpool.tile([128, T], BF16)
            nc.sync.dma_start(out=sin_sb, in_=sinb[:])
            masks_sb = cpool.tile([128, 4, TB], BF16)
            qnw_sb = cpool.tile([128, 1], F32)
            nc.sync.dma_start(out=qnw_sb, in_=qnw[:])
            knw_sb = cpool.tile([128, 1], F32)
            lastconst_dma = nc.sync.dma_start(out=knw_sb, in_=knw[:])
            gatew_sb = cpool.tile([128, CK, E], F16)
            sel8_sb = cpool.tile([E, E * 128], BF16)

            ow_sb = ab.tile([128, CK, HQ, 128], BF16)

            qT_sb = ab.tile([128, HQ, T], BF16)            # per-batch
            kT_sb = ab.tile([128, T], BF16)
            vnat_sb = ab.tile([128, T // 128, 128], BF16)
            rk_sb = ab.tile([128, T // 128], F32)     # per-key-tile rstd_k
            xh_sb = big.tile([128, CK, NOWN], F16)          # own hidden (x+attn)
            xn8_sb = big.tile([128, CK, NOWN], F8)
            prod8_sb = big.tile([128, E, FK, NOWN], F8)
            comb_row = big.tile([E, NOWN], F32)             # routing weights

            def deferred_const_loads():
                nc.sync.dma_start(out=ow_sb, in_=ow[:])
                nc.sync.dma_start(out=masks_sb, in_=masks[:])
                nc.sync.dma_start(out=gatew_sb, in_=gatew[:])
                nc.sync.dma_start(out=sel8_sb, in_=sel8[:])

            rsin = [dram.tile([8, 128, CK, 128], F16, name=f"rsin{b}") for b in range(B)]
            rsout = [dram.tile([128, CK, 128], F16, name=f"rsout{b}") for b in range(B)]

            def bcast(row_f32_ap, width=TB, act_copy=False, **pool_kw):
                """[1,width] f32 -> SBUF [128,width] f32 via K=1 f32r matmul."""
                rr = wk.tile([1, TB], F32R, tag="rwr", bufs=1)
                nc.vector.tensor_copy(out=rr[:, 0:width], in_=row_f32_ap)
                bc_ps = ps.tile([128, TB], F32, **MM)
                nc.tensor.matmul(bc_ps[:, 0:width], onesrow_r,
                                 rr[:, 0:width], start=True, stop=True)
                kw_ = pool_kw or BCS
                bc = wk.tile([128, TB], F32, **kw_)
                if act_copy:
                    nc.scalar.copy(out=bc[:, 0:width], in_=bc_ps[:, 0:width])
                else:
                    nc.vector.tensor_copy(out=bc[:, 0:width], in_=bc_ps[:, 0:width])
                return bc[:, 0:width]

            def bcast8(row_f32_ap):
                """[1,NOWN] f32 -> SBUF [E,NOWN] f32, exact (fp32 matmul so
                is_equal against the source values still holds bitwise)."""
                p8 = ps.tile([8, TB], F32, **ROW)
                nc.tensor.matmul(p8[0:E, 0:NOWN], ones8_f, row_f32_ap,
                                 start=True, stop=True)
                t8 = wk.tile([E, NOWN], F32, **SM8)
                nc.vector.tensor_copy(out=t8, in_=p8[0:E, 0:NOWN])
                return t8

            def colsum_rstd(feat_ps, scale, bias_ap):
                """rsqrt(scale*colsum(feat^2)+bias) -> [1,TB] f32 row."""
                sq = wk.tile([128, TB], BF16, **TBF)
                nc.scalar.activation(out=sq, in_=feat_ps, func=AF.Square)
                ssum = ps.tile([8, TB], F32, **ROW)
                nc.tensor.matmul(ssum[0:1, :], ones_bf, sq, start=True, stop=True)
                srow = wk.tile([1, TB], F32, **RWF)
                nc.scalar.activation(out=srow, in_=ssum[0:1, :], func=AF.Sqrt,
                                     scale=scale, bias=bias_ap)
                rrow = wk.tile([1, TB], F32, **RWF)
                nc.vector.reciprocal(out=rrow, in_=srow)
                return rrow

            def rope_plain(feat_sb, j, w_sb, out_ap):
                """out = rope(w * feat); rstd applied separately (rope is
                linear per token, so the per-token scale commutes)."""
                tcol = j * TB
                qhat = wk.tile([128, TB], BF16, **TBF)
                nc.vector.tensor_scalar(out=qhat, in0=feat_sb, scalar1=w_sb,
                                        scalar2=None, op0=OP.mult)
                rot_ps = ps.tile([128, TB], F32, **MM)
                nc.tensor.matmul(rot_ps, prot_sb, qhat, start=True, stop=True)
                qc = wk.tile([128, TB], BF16, **TBF)
                nc.vector.tensor_tensor(
                    out=qc, in0=qhat, in1=cos_sb[:, tcol:tcol + TB], op=OP.mult,
                )
                rsm = wk.tile([128, TB], BF16, **TBF)
                nc.vector.tensor_tensor(
                    out=rsm, in0=rot_ps, in1=sin_sb[:, tcol:tcol + TB], op=OP.mult,
                )
                nc.vector.tensor_tensor(out=out_ap, in0=qc, in1=rsm, op=OP.add)

            # =========================== Phase A: QKV ======================
            def phaseA(g):
                b, j = divmod(g, 2)
                n0 = g * TB
                if g == 0:
                    xtb = xtb0
                else:
                    xtb = ab.tile([128, CK, TB], BF16, tag="xtb", bufs=1)
                    nc.sync.dma_start(out=xtb[:, 0:CK // 2, :],
                                      in_=xT[:, 0:CK // 2, n0:n0 + TB])
                    nc.sync.dma_start(out=xtb[:, CK // 2:CK, :],
                                      in_=xT[:, CK // 2:CK, n0:n0 + TB])
                r1row = wk.tile([1, TB], F32, **RWF)
                nc.sync.dma_start(out=r1row, in_=rstd1[:, n0:n0 + TB])
                r1bc = bcast(r1row[:], act_copy=True)

                def accum(kind, hd):
                    p = ps.tile([128, TB], F32, **MM)
                    for kc in range(CK):
                        if kind == 'q':
                            lhs = qw_sb[:, hd, kc, :]
                        elif kind == 'k':
                            lhs = kw_sb[:, kc, :]
                        else:
                            lhs = vw_sb[:, kc, :]
                        nc.tensor.matmul(p, lhs, xtb[:, kc, :],
                                         start=(kc == 0), stop=(kc == CK - 1))
                    return p

                def stage2(kind, p):
                    # free the psum early: rope reads the bf16 SBUF copy
                    if kind == 'q':
                        row = colsum_rstd(p, 1.0, bias_q)
                    elif kind == 'k':
                        row = colsum_rstd(p, 1.0 / D, bias_eps)
                    else:
                        row = None
                    fsb = wk.tile([128, TB], BF16, tag="fsb", bufs=2)
                    nc.scalar.copy(out=fsb, in_=p)
                    return (row, fsb)

                def stage3(kind, hd, st2):
                    row, fsb = st2
                    if kind == 'q':
                        qbc = bcast(row[:], act_copy=True)
                        qr = wk.tile([128, TB], BF16, **TBF)
                        rope_plain(fsb, j, qnw_sb, qr)
                        nc.vector.tensor_tensor(
                            out=qT_sb[:, hd, j * TB:j * TB + TB],
                            in0=qr, in1=qbc, op=OP.mult)
                    elif kind == 'k':
                        rope_plain(fsb, j, knw_sb,
                                   kT_sb[:, j * TB:j * TB + TB])
                        for cc in range(TB // 128):
                            rkt = ps.tile([128, TB], F32, **MM)
                            nc.tensor.transpose(
                                rkt[:, 0:1], row[:, cc * 128:(cc + 1) * 128],
                                idf1)
                            nc.vector.tensor_copy(
                                out=rk_sb[:, 4 * j + cc:4 * j + cc + 1],
                                in_=rkt[:, 0:1])
                    else:
                        vhat = wk.tile([128, TB], BF16, **TBF)
                        nc.vector.tensor_tensor(out=vhat, in0=fsb, in1=r1bc,
                                                op=OP.mult)
                        for cc in range(TB // 128):
                            vtr = ps.tile([128, 128], BF16, **TRP)
                            nc.tensor.transpose(
                                vtr, vhat[:, cc * 128:(cc + 1) * 128], ident)
                            nc.vector.tensor_copy(
                                out=vnat_sb[:, j * 4 + cc, :], in_=vtr)

                seq = [('q', 0), ('q', 1), ('q', 2), ('q', 3),
                       ('k', None), ('v', None)]
                st = []
                for idx, (kind, hd) in enumerate(seq):
                    p = accum(kind, hd)
                    st.append([kind, hd, p, None])
                    if idx >= 1:
                        st[idx - 1][3] = stage2(st[idx - 1][0], st[idx - 1][2])
                    if idx >= 2:
                        stage3(st[idx - 2][0], st[idx - 2][1], st[idx - 2][3])
                st[-1][3] = stage2(st[-1][0], st[-1][2])
                stage3(st[-2][0], st[-2][1], st[-2][3])
                stage3(st[-1][0], st[-1][1], st[-1][3])

            # ====================== Phase B: attention =====================
            def phaseB(g):
                b, j = divmod(g, 2)
                q0 = j * TB
                ntk = 4 * j + 4
                avT = ab.tile([128, HQ, TB], BF16, tag="avT", bufs=1)

                def fin(hd, av_sb, den_ps):
                    dsb = wk.tile([1, TB], F32, **RWF)
                    nc.vector.tensor_copy(out=dsb, in_=den_ps[0:1, :])
                    rec = wk.tile([1, TB], F32, **RWF)
                    nc.vector.reciprocal(out=rec, in_=dsb)
                    rbc = bcast(rec[:])
                    nc.vector.tensor_tensor(out=avT[:, hd, :], in0=av_sb,
                                            in1=rbc, op=OP.mult)

                pend = None
                for hd in range(HQ):
                    av_ps = ps.tile([128, TB], F32, **MM)
                    den_ps = ps.tile([8, TB], F32, **ROW)
                    for i in range(ntk):
                        tk0 = i * 128
                        s = i - 4 * j
                        # diagonal tiles: columns < 128*s are fully masked, so
                        # restrict score/exp/den/av to the live column range
                        c0 = max(s, 0) * 128
                        w = TB - c0
                        sc_ps = ps.tile([128, TB], F32, **MM)
                        nc.tensor.matmul(sc_ps[:, c0:TB], kT_sb[:, tk0:tk0 + 128],
                                         qT_sb[:, hd, q0 + c0:q0 + TB],
                                         start=True, stop=True,
                                         skip_group_check=True)
                        ex = wk.tile([128, TB], BF16, **TBF)
                        if s < 0:
                            nc.scalar.activation(out=ex, in_=sc_ps, func=AF.Exp,
                                                 scale=rk_sb[:, i:i + 1])
                        else:
                            ext = wk.tile([128, TB], BF16, **TBF)
                            nc.scalar.activation(out=ext[:, c0:TB],
                                                 in_=sc_ps[:, c0:TB], func=AF.Exp,
                                                 scale=rk_sb[:, i:i + 1])
                            nc.vector.tensor_tensor(out=ex[:, c0:TB],
                                                    in0=ext[:, c0:TB],
                                                    in1=masks_sb[:, s, c0:TB],
                                                    op=OP.mult)
                        nc.tensor.matmul(den_ps[0:1, c0:TB], ones_bf,
                                         ex[:, c0:TB],
                                         start=(i == 0), stop=(i == ntk - 1),
                                         skip_group_check=True)
                        nc.tensor.matmul(av_ps[:, c0:TB], vnat_sb[:, i, :],
                                         ex[:, c0:TB],
                                         start=(i == 0), stop=(i == ntk - 1),
                                         skip_group_check=True)
                    # free the av psum early via an Act copy; fin reads SBUF
                    av_sb = wk.tile([128, TB], F32, **BCS)
                    nc.scalar.copy(out=av_sb, in_=av_ps)
                    if pend is not None:
                        fin(*pend)
                    pend = (hd, av_sb, den_ps)
                fin(*pend)
                for mq in range(4):
                    attq = ab.tile([128, 4, TB], F16, tag="attb", bufs=2)
                    for mi in range(4):
                        m = 4 * mq + mi
                        att_ps = ps.tile([128, TB], F32, **MM)
                        for hk in range(HQ):
                            nc.tensor.matmul(att_ps, ow_sb[:, m, hk, :],
                                             avT[:, hk, :], start=(hk == 0),
                                             stop=(hk == HQ - 1))
                        nc.scalar.copy(out=attq[:, mi, :], in_=att_ps)
                    for tg in range(4):
                        nc.sync.dma_start(
                            out=rsin[b][4 * j + tg, :, 4 * mq:4 * mq + 4, :],
                            in_=attq[:, :, tg * 128:(tg + 1) * 128])
                if j == 1:
                    nc.gpsimd.collective_compute(
                        "ReduceScatter", OP.add,
                        replica_groups=[list(range(N_CORES))],
                        ins=[rsin[b].opt()], outs=[rsout[b].opt()],
                    )

            # ========================= Phase C: MoE ========================
            HGU = FK * 8 * 2 * 128    # flat size of one gate (or up) half

            def load_guw_half(e, half, eng=None, after=None):
                t = wk.tile([128, FK * 8, 2, 128], F8, tag="wgu", bufs=4)
                d = (eng or nc.gpsimd).dma_start(
                    out=t,
                    in_=guw[:, e, half * HGU:(half + 1) * HGU].rearrange(
                        "p (a b c) -> p a b c", b=2, c=128))
                if after is not None:
                    _add_dep_helper(d.ins, after.ins, sync=True,
                                    reason="prefetch after startup loads")
                return t

            def load_dww(m, eng=None, after=None):
                t = wk.tile([128, E * 3, 2, 128], F8, tag="wdw", bufs=2)
                d = (eng or nc.sync).dma_start(out=t, in_=dww[:, m, :].rearrange(
                    "p (a b c) -> p a b c", b=2, c=128))
                if after is not None:
                    _add_dep_helper(d.ins, after.ins, sync=True,
                                    reason="prefetch after startup loads")
                return t

            def phaseC(pre_gu, pre_dw, moe):
                # assemble own hidden = attn partial sums (+x) for owned tokens
                for b in range(B):
                    nc.sync.dma_start(out=xh_sb[:, :, 128 * b:128 * b + 128],
                                      in_=rsout[b][:])
                for fq in range(4):
                    xow = wk.tile([128, 4, NOWN], F16, tag="xow", bufs=1)
                    nc.gpsimd.dma_start(out=xow, in_=xown[:, 4 * fq:4 * fq + 4, :])
                    for fi in range(4):
                        fc = 4 * fq + fi
                        nc.vector.tensor_tensor(out=xh_sb[:, fc, :],
                                                in0=xh_sb[:, fc, :],
                                                in1=xow[:, fi, :], op=OP.add)
                lg_ps = ps.tile([8, TB], F32, **ROW)
                den2_ps = ps.tile([8, TB], F32, **ROW)
                for fc in range(CK):
                    nc.tensor.matmul(lg_ps[0:E, 0:NOWN], gatew_sb[:, fc, :],
                                     xh_sb[:, fc, :],
                                     start=(fc == 0), stop=(fc == CK - 1))
                    sq = wk.tile([128, NOWN], BF16, **MC)
                    nc.scalar.activation(out=sq, in_=xh_sb[:, fc, :], func=AF.Square)
                    nc.tensor.matmul(den2_ps[0:1, 0:NOWN], ones_bf, sq,
                                     start=(fc == 0), stop=(fc == CK - 1))
                s2 = wk.tile([1, NOWN], F32, **SM1)
                nc.scalar.activation(out=s2, in_=den2_ps[0:1, 0:NOWN], func=AF.Sqrt,
                                     scale=1.0 / C, bias=bias_eps)
                rstd2 = wk.tile([1, NOWN], F32, tag="rstd2", bufs=1)
                nc.vector.reciprocal(out=rstd2, in_=s2)
                r2bc = bcast(rstd2[:], width=NOWN)
                for fc in range(CK):
                    nc.vector.tensor_tensor(out=xn8_sb[:, fc, :],
                                            in0=xh_sb[:, fc, :],
                                            in1=r2bc, op=OP.mult)

                # ---- top-2 routing on [E, NOWN] ----
                lg = wk.tile([E, NOWN], F32, tag="lg", bufs=1)
                nc.vector.tensor_copy(out=lg, in_=lg_ps[0:E, 0:NOWN])
                m1 = wk.tile([1, NOWN], F32, **SM1)
                nc.gpsimd.tensor_reduce(out=m1, in_=lg, axis=mybir.AxisListType.C,
                                        op=OP.max)
                m1bc = bcast8(m1[:])
                eq1 = wk.tile([E, NOWN], F32, tag="eq1", bufs=1)
                nc.vector.tensor_tensor(out=eq1, in0=lg, in1=m1bc, op=OP.is_equal)
                lg2 = wk.tile([E, NOWN], F32, **SM8)
                nc.vector.scalar_tensor_tensor(out=lg2, in0=eq1, scalar=-BIG,
                                               in1=lg, op0=OP.mult, op1=OP.add)
                m2 = wk.tile([1, NOWN], F32, **SM1)
                nc.gpsimd.tensor_reduce(out=m2, in_=lg2, axis=mybir.AxisListType.C,
                                        op=OP.max)
                m2bc = bcast8(m2[:])
                eq2 = wk.tile([E, NOWN], F32, **SM8)
                nc.vector.tensor_tensor(out=eq2, in0=lg, in1=m2bc, op=OP.is_equal)
                # dlt = (m1-m2)*rstd2 ; w1 = sigmoid(dlt); w2 = 1-w1
                dlt = wk.tile([1, NOWN], F32, **SM1)
                nc.vector.tensor_tensor(out=dlt, in0=m1, in1=m2, op=OP.subtract)
                dlts = wk.tile([1, NOWN], F32, **SM1)
                nc.vector.tensor_tensor(out=dlts, in0=dlt, in1=rstd2, op=OP.mult)
                w1 = wk.tile([1, NOWN], F32, **SM1)
                nc.scalar.activation(out=w1, in_=dlts, func=AF.Sigmoid)
                w1bc = bcast8(w1[:])
                # comb = eq1*w1 + eq2*(1-w1) = (eq1-eq2)*w1 + eq2, in place
                nc.vector.tensor_tensor(out=eq1, in0=eq1, in1=eq2, op=OP.subtract)
                nc.vector.tensor_tensor(out=eq1, in0=eq1, in1=w1bc, op=OP.mult)
                nc.vector.tensor_tensor(out=eq1, in0=eq1, in1=eq2, op=OP.add)
                # scale by PSC/WS (prod fp8 scale / up-weight descale)
                nc.vector.tensor_scalar(out=comb_row, in0=eq1, scalar1=PSC / WS,
                                        scalar2=None, op0=OP.mult)

                # ---- pass 1: gate/up + silu -> prod8 per expert ----
                def load_guw_moe(e, half, eng):
                    t = moe.tile([128, FK * 8, 2, 128], F8, tag="wgu2", bufs=5)
                    eng.dma_start(
                        out=t,
                        in_=guw[:, e, half * HGU:(half + 1) * HGU].rearrange(
                            "p (a b c) -> p a b c", b=2, c=128))
                    return t

                def load_dww_moe(m, eng):
                    t = moe.tile([128, E * 3, 2, 128], F8, tag="wdw2", bufs=3)
                    eng.dma_start(out=t, in_=dww[:, m, :].rearrange(
                        "p (a b c) -> p a b c", b=2, c=128))
                    return t

                comb_bf = wk.tile([E, NOWN], BF16, tag="combbf", bufs=1)
                nc.vector.tensor_copy(out=comb_bf, in_=comb_row)
                for e in range(E):
                    wgg = pre_gu.pop((e, 0), None) or load_guw_moe(e, 0, nc.sync)
                    wgu = pre_gu.pop((e, 1), None) or load_guw_moe(e, 1, nc.scalar)
                    cb_ps = ps.tile([128, TB], F32, **MM)
                    nc.tensor.matmul(cb_ps[:, 0:NOWN],
                                     sel8_sb[:, e * 128:(e + 1) * 128],
                                     comb_bf, start=True, stop=True)
                    cbc = wk.tile([128, NOWN], F32, tag="cbc", bufs=1)
                    nc.vector.tensor_copy(out=cbc, in_=cb_ps[:, 0:NOWN])
                    for f in range(FK):
                        g_ps = ps.tile([128, TB], F32, **MM)
                        for kp in range(8):
                            nc.tensor.matmul(
                                g_ps[:, 0:NOWN], wgg[:, f * 8 + kp, :, :],
                                xn8_sb[:, 2 * kp:2 * kp + 2, :],
                                start=(kp == 0), stop=(kp == 7),
                                perf_mode=PM.DoubleRow,
                            )
                        u_ps = ps.tile([128, TB], F32, **MM)
                        for kp in range(8):
                            nc.tensor.matmul(
                                u_ps[:, 0:NOWN], wgu[:, f * 8 + kp, :, :],
                                xn8_sb[:, 2 * kp:2 * kp + 2, :],
                                start=(kp == 0), stop=(kp == 7),
                                perf_mode=PM.DoubleRow,
                            )
                        sil = wk.tile([128, NOWN], BF16, **MC)
                        nc.scalar.activation(out=sil, in_=g_ps[:, 0:NOWN],
                                             func=AF.Silu, scale=1.0 / WS)
                        ucm = wk.tile([128, NOWN], BF16, **MC)
                        nc.vector.tensor_tensor(out=ucm, in0=u_ps[:, 0:NOWN],
                                                in1=cbc, op=OP.mult)
                        nc.vector.tensor_tensor(out=prod8_sb[:, e, f, :],
                                                in0=sil, in1=ucm, op=OP.mult)

                # ---- pass 2: down proj, accumulate experts in psum ----
                for m in range(CK):
                    wd = pre_dw.pop(m, None) or load_dww_moe(m, nc.sync)
                    eo_ps = ps.tile([128, TB], F32, **MM)
                    for e in range(E):
                        for kp in range(3):
                            nc.tensor.matmul(
                                eo_ps[:, 0:NOWN], wd[:, e * 3 + kp, :, :],
                                prod8_sb[:, e, 2 * kp:2 * kp + 2, :],
                                start=(e == 0 and kp == 0),
                                stop=(e == E - 1 and kp == 2),
                                perf_mode=PM.DoubleRow,
                            )
                    ym = wk.tile([128, NOWN], F16, tag="ymc", bufs=2)
                    nc.vector.scalar_tensor_tensor(
                        out=ym, in0=eo_ps[:, 0:NOWN],
                        scalar=1.0 / (WS * PSC), in1=xh_sb[:, m, :],
                        op0=OP.mult, op1=OP.add,
                    )
                    nc.sync.dma_start(out=y[:, m, :], in_=ym)

            pre_gu, pre_dw = {}, {}
            for g in range(NBLK):
                if 'A' in phases:
                    phaseA(g)
                if g == 0:
                    deferred_const_loads()
                    if 'C' in phases:
                        # act-queue prefetches: the Act sequencer reaches these
                        # only after A0's first Square, keeping the DMA engines
                        # free for the critical startup loads
                        for e in range(2):
                            for half in range(2):
                                pre_gu[(e, half)] = load_guw_half(
                                    e, half, nc.scalar, after=lastconst_dma)
                        pre_dw[0] = load_dww(0, nc.scalar, after=lastconst_dma)
                        pre_dw[1] = load_dww(1, nc.scalar, after=lastconst_dma)
                if 'B' in phases:
                    phaseB(g)
            ab.release()
            if 'C' in phases:
                with tc.tile_pool(name="moe", bufs=1) as moe:
                    phaseC(pre_gu, pre_dw, moe)

    _split_multi_waits(nc)
    return nc


# ---------------------------------------------------------------------------

_NC_CACHE = {}


def _get_nc():
    if "nc" not in _NC_CACHE:
        _NC_CACHE["nc"] = build_nc()
    return _NC_CACHE["nc"]


def _chunk_pm(a, nchunk):
    """[nchunk*128, free...] -> [128, nchunk, free...]"""
    return np.ascontiguousarray(
        a.reshape(nchunk, 128, *a.shape[1:]).transpose(1, 0, *range(2, a.ndim + 1))
    )


def prepare_in_maps(x, cos, sin, ln1_w, q_w, k_w, v_w, o_w, qn_w, kn_w, ln2_w,
                    gate_w, gate_up_w, down_w):
    bf = ml_dtypes.bfloat16
    f8 = ml_dtypes.float8_e4m3
    x = np.asarray(x, dtype=np.float32)
    x_flat = x.reshape(N, C)

    xT = _chunk_pm(np.ascontiguousarray(x_flat.T).astype(bf), CK)
    rstd1 = (1.0 / np.sqrt((x_flat.astype(np.float64) ** 2).mean(axis=1) + EPS)
             ).astype(np.float32)[None, :]

    ln1 = np.asarray(ln1_w, dtype=np.float32)[:, None]
    ln2 = np.asarray(ln2_w, dtype=np.float32)[:, None]
    qwf = np.asarray(q_w, dtype=np.float32) * ln1
    kwf = np.asarray(k_w, dtype=np.float32) * ln1
    vwf = np.asarray(v_w, dtype=np.float32) * ln1
    gatewf = np.asarray(gate_w, dtype=np.float32) * ln2
    guwf = np.asarray(gate_up_w, dtype=np.float32) * ln2[None]    # [E, C, 2F]
    dwf = np.asarray(down_w, dtype=np.float32)                    # [E, F, C]
    owf = np.asarray(o_w, dtype=np.float32)

    cos0 = np.asarray(cos, dtype=np.float32)[0]
    sin0 = np.asarray(sin, dtype=np.float32)[0]
    cosT = np.ascontiguousarray(cos0.T).astype(bf)
    sinT = np.ascontiguousarray(sin0.T).astype(bf)
    protm = np.zeros((128, 128), dtype=np.float32)
    for m in range(64):
        protm[m + 64, m] = -1.0
    for m in range(64, 128):
        protm[m - 64, m] = 1.0

    r = np.arange(128)[:, None]
    col = np.arange(TB)[None, :]
    masks = np.stack(
        [(col >= r + 128 * s).astype(bf) for s in range(4)], axis=1
    )

    # fp8 MoE weights, shared across cores
    # guw host layout: [128, E, FGU*8*2*128]; lhsT slice [128, 2, 128] is
    # (grp, kp) with pair index i selecting k-chunk 2kp+i.
    gu6 = (guwf * WS).astype(f8)                       # [E, C, 2F]
    gu_r = gu6.reshape(E, 8, 2, 128, FGU, 128)         # e, kp, i, p, grp, d
    guw_h = np.ascontiguousarray(
        gu_r.transpose(3, 0, 4, 1, 2, 5).reshape(128, E, FGU * 8 * 2 * 128))
    # dww host layout: [128, CK(m), E*3*2*128]; lhsT slice (e, kp) pair i
    # selects f-chunk 2kp+i; partition p = f % 128; d = c within group m.
    dw6 = (dwf * WS).astype(f8)                        # [E, F, C]
    dw_r = dw6.reshape(E, 3, 2, 128, CK, 128)          # e, kp, i, p, m, d
    dww_h = np.ascontiguousarray(
        dw_r.transpose(3, 4, 0, 1, 2, 5).reshape(128, CK, E * 3 * 2 * 128))

    gatew_h = _chunk_pm(gatewf.astype(np.float16), CK)

    in_maps = []
    for c in range(N_CORES):
        oslice = owf[512 * c:512 * (c + 1), :].astype(bf)  # [512, C]
        o4 = oslice.reshape(HQ, 128, CK, 128)              # hk, p, m, d
        ow_h = np.ascontiguousarray(o4.transpose(1, 2, 0, 3))
        # owned tokens: batch b local [128c, 128c+128)
        own_idx = np.concatenate([
            np.arange(b * T + 128 * c, b * T + 128 * (c + 1)) for b in range(B)
        ])
        xo = x_flat[own_idx, :].T                          # [C, 256]
        xown_h = _chunk_pm(np.ascontiguousarray(xo).astype(np.float16), CK)
        in_maps.append({
            "xT": xT,
            "xown": xown_h,
            "qw": np.ascontiguousarray(
                qwf[:, 512 * c:512 * (c + 1)].astype(bf)
                .reshape(CK, 128, HQ, 128).transpose(1, 2, 0, 3)),
            "kw": _chunk_pm(kwf[:, 128 * c:128 * (c + 1)].astype(bf), CK),
            "vw": _chunk_pm(vwf[:, 128 * c:128 * (c + 1)].astype(bf), CK),
            "ow": ow_h,
            "gatew": gatew_h,
            "guw": guw_h,
            "dww": dww_h,
            "cosb": cosT,
            "sinb": sinT,
            "masks": masks,
            "rstd1": rstd1,
            "qnw": np.asarray(qn_w, dtype=np.float32)[:, None],
            "knw": np.asarray(kn_w, dtype=np.float32)[:, None],
            "protb": protm.astype(bf),
            "sel8": np.kron(np.eye(E, dtype=np.float32),
                            np.ones((1, 128), dtype=np.float32)).astype(bf),
        })

    return in_maps


def combine(ys):
    out = np.zeros((N, C), dtype=np.float32)
    for c in range(N_CORES):
        yc = np.asarray(ys[c], dtype=np.float32)     # [128, CK, 256]
        # yc[p, fc, 128*b + i] -> token b*T + 128*c + i, feature fc*128+p
        feat_major = yc.transpose(1, 0, 2).reshape(C, NOWN)
        for b in range(B):
            toks = slice(b * T + 128 * c, b * T + 128 * (c + 1))
            out[toks, :] = feat_major[:, 128 * b:128 * (b + 1)].T
    return out.reshape(B, T, C)


def kernel(**inputs):
    in_maps = prepare_in_maps(**inputs)
    nc = _get_nc()
    res = run_bass_kernel_spmd(nc, in_maps, core_ids=list(range(N_CORES)))
    return combine([res.results[c]["y"] for c in range(N_CORES)])


# revision 6
# speedup vs baseline: 1.0977x; 1.0117x over previous
"""Fused attention+MoE block on 8 trn2 NeuronCores, v2.

Sharding: tensor-parallel attention (4 q-heads + 1 KV-head per core) as
before, but the attention output partials are ReduceScattered (2 per-batch
collectives) so each core ends up owning 256 tokens of the full hidden
state in feature-major layout. The MoE is then data-parallel: every core
runs all 8 experts densely on its own 256 tokens with fp8e4 DoubleRow
matmuls (2 k-chunks per instruction) and fp8 weights streamed from HBM.
Routing (top-2 over E=8) is computed on an [E, 256] tile with
partition-dim max reductions. Output y = hidden + moe for the owned
tokens; the host just concatenates core slices.
"""

import numpy as np
import ml_dtypes

import concourse.bass as bass
from concourse.bass import _add_dep_helper
import concourse.mybir as mybir
import concourse.tile as tile
from concourse.bass_utils import run_bass_kernel_spmd
from concourse.masks import make_identity
from concourse.vector_clock import ScopedClock

F32 = mybir.dt.float32
F32R = mybir.dt.float32r
F16 = mybir.dt.float16
BF16 = mybir.dt.bfloat16
F8 = mybir.dt.float8e4
U32 = mybir.dt.uint32
AF = mybir.ActivationFunctionType
OP = mybir.AluOpType
PM = mybir.MatmulPerfMode

B, T, C = 2, 1024, 2048
H, KV, D = 32, 8, 128
E, F, TOPK = 8, 768, 2
N = B * T
CK = C // 128          # 16
NBLK = 4               # token blocks of 512
TB = 512
HQ = H // 8            # 4 q heads per core
FK = F // 128          # 6
FGU = 2 * F // 128     # 12
NOWN = 256             # tokens owned per core (128 per batch)
WS = 64.0              # fp8 weight scale
PSC = 16.0             # fp8 prod scale
EPS = 1e-6
N_CORES = 8
BIG = 1e9

# ---------------------------------------------------------------------------
# walrus here rejects >1 sync-wait per instruction; split extras onto NoOps.


class _SplitDrainTileContext(tile.TileContext):
    def _drain_and_barrier(self, tick_clock, wait_clock):
        drain_inst = self.nc.sync.drain()
        wait_clock.add_sem_waits(
            drain_inst.ins, ScopedClock({None: tick_clock.global_clock})
        )
        si = drain_inst.ins.sync_info
        if si is not None and len(si.on_wait) > 1:
            ow = list(si.on_wait)
            drain_inst.ins.sync_info = mybir.SyncInfo(
                on_wait=ow[:1], on_update=list(si.on_update)
            )
            rest = ow[1:]
            while rest:
                extra = self.nc.sync.drain()
                extra.ins.sync_info = mybir.SyncInfo(on_wait=rest[:1], on_update=[])
                rest = rest[1:]
        self.nc.all_engine_barrier()
        assert self.sems is not None
        popped = self.nc._tile_sem_poison_stack.pop()
        assert popped is self._sem_poison
        self.nc.clear_and_free_semaphores(list(self.sems.allocated().values()))
        self.nc.all_engine_barrier()


def _split_multi_waits(nc):
    for bb in nc.main_func.blocks:
        insts = list(bb.instructions)
        out = []
        changed = False
        for ins in insts:
            si = ins.sync_info
            if si is not None and len(si.on_wait) > 1:
                ow = list(si.on_wait)
                for w in ow[:-1]:
                    nop = mybir.InstNoOp(name=f"waitnop-{nc.next_id()}", ins=[], outs=[])
                    nop.engine = ins.engine
                    nop.sync_info = mybir.SyncInfo(on_wait=[w], on_update=[])
                    out.append(nop)
                ins.sync_info = mybir.SyncInfo(
                    on_wait=[ow[-1]], on_update=list(si.on_update)
                )
                changed = True
            out.append(ins)
        if changed:
            bb.instructions = out


# ---------------------------------------------------------------------------


def build_nc(phases='ABC'):
    nc = bass.Bass("TRN2", target_bir_lowering=False, debug=False, num_devices=N_CORES)

    xT = nc.dram_tensor("xT", [128, CK, N], BF16, kind="ExternalInput")
    xown = nc.dram_tensor("xown", [128, CK, NOWN], F16, kind="ExternalInput")
    qw = nc.dram_tensor("qw", [128, HQ, CK, 128], BF16, kind="ExternalInput")
    kw = nc.dram_tensor("kw", [128, CK, 128], BF16, kind="ExternalInput")
    vw = nc.dram_tensor("vw", [128, CK, 128], BF16, kind="ExternalInput")
    ow = nc.dram_tensor("ow", [128, CK, HQ, 128], BF16, kind="ExternalInput")
    gatew = nc.dram_tensor("gatew", [128, CK, E], F16, kind="ExternalInput")
    guw = nc.dram_tensor("guw", [128, E, FGU * 8 * 2 * 128], F8, kind="ExternalInput")
    dww = nc.dram_tensor("dww", [128, CK, E * 3 * 2 * 128], F8, kind="ExternalInput")
    cosb = nc.dram_tensor("cosb", [128, T], BF16, kind="ExternalInput")
    sinb = nc.dram_tensor("sinb", [128, T], BF16, kind="ExternalInput")
    masks = nc.dram_tensor("masks", [128, 4, TB], BF16, kind="ExternalInput")
    rstd1 = nc.dram_tensor("rstd1", [1, N], F32, kind="ExternalInput")
    qnw = nc.dram_tensor("qnw", [128, 1], F32, kind="ExternalInput")
    knw = nc.dram_tensor("knw", [128, 1], F32, kind="ExternalInput")
    protb = nc.dram_tensor("protb", [128, 128], BF16, kind="ExternalInput")
    sel8 = nc.dram_tensor("sel8", [E, E * 128], BF16, kind="ExternalInput")

    y = nc.dram_tensor("y", [128, CK, NOWN], F16, kind="ExternalOutput")

    with _SplitDrainTileContext(nc) as tc:
        with (
            tc.tile_pool(name="const", bufs=1) as cpool,
            tc.tile_pool(name="dram", bufs=1, space="DRAM") as dram,
            tc.tile_pool(name="ps", bufs=1, space="PSUM") as ps,
            tc.tile_pool(name="big", bufs=1) as big,
            tc.tile_pool(name="work", bufs=3) as wk,
        ):
            ab = tc.alloc_tile_pool(name="ab", bufs=1)
            MM = dict(tag="mm", bufs=5)       # f32 [128,TB] psum
            ROW = dict(tag="row", bufs=2)     # f32 [<=8,TB] psum
            TRP = dict(tag="trp", bufs=1)     # bf16 [128,128] psum
            TBF = dict(tag="t512b", bufs=3)   # bf16 [128,TB] transients
            RWF = dict(tag="rowf", bufs=2)    # f32 [1,TB]
            BCS = dict(tag="bcs", bufs=3)     # f32 [128,TB] bcast results
            SM8 = dict(tag="sm8", bufs=2)     # f32 [8,NOWN] small routing tiles
            SM1 = dict(tag="sm1", bufs=3)     # f32 [1,NOWN]
            MC = dict(tag="mc", bufs=2)       # bf16 [128,NOWN] moe transients

            # ---- constants ----
            ident = cpool.tile([128, 128], BF16)
            make_identity(nc, ident)
            ones_bf = cpool.tile([128, 1], BF16)
            nc.vector.memset(ones_bf, 1.0)
            onesrow_f = cpool.tile([1, 128], F32)
            nc.vector.memset(onesrow_f, 1.0)
            onesrow_r = cpool.tile([1, 128], F32R)
            nc.vector.tensor_copy(out=onesrow_r, in_=onesrow_f)
            ones8_f = cpool.tile([1, E], F32)
            nc.vector.memset(ones8_f, 1.0)
            ones8_r = cpool.tile([1, E], F32R)
            nc.vector.tensor_copy(out=ones8_r, in_=ones8_f)
            bias_q = cpool.tile([1, 1], F32)
            nc.vector.memset(bias_q, float(D) * EPS)
            bias_eps = cpool.tile([1, 1], F32)
            nc.vector.memset(bias_eps, EPS)
            idf1 = cpool.tile([1, 1], F32)
            nc.vector.memset(idf1, 1.0)

            xtb0 = ab.tile([128, CK, TB], BF16, tag="xtb", bufs=1, name="xtb0")
            nc.sync.dma_start(out=xtb0[:, 0:CK // 2, :], in_=xT[:, 0:CK // 2, 0:TB])
            x0dma = nc.sync.dma_start(out=xtb0[:, CK // 2:CK, :],
                                      in_=xT[:, CK // 2:CK, 0:TB])
            qw_sb = ab.tile([128, HQ, CK, 128], BF16)
            for _hd in range(HQ):
                nc.sync.dma_start(out=qw_sb[:, _hd, :, :], in_=qw[:, _hd, :, :])
            kw_sb = ab.tile([128, CK, 128], BF16)
            nc.sync.dma_start(out=kw_sb, in_=kw[:])
            vw_sb = ab.tile([128, CK, 128], BF16)
            nc.sync.dma_start(out=vw_sb, in_=vw[:])
            prot_sb = cpool.tile([128, 128], BF16)
            nc.sync.dma_start(out=prot_sb, in_=protb[:])
            cos_sb = cpool.tile([128, T], BF16)
            nc.sync.dma_start(out=cos_sb, in_=cosb[:])
            sin_sb = cpool.tile([128, T], BF16)
            nc.sync.dma_start(out=sin_sb, in_=sinb[:])
            masks_sb = cpool.tile([128, 4, TB], BF16)
            qnw_sb = cpool.tile([128, 1], F32)
            nc.sync.dma_start(out=qnw_sb, in_=qnw[:])
            knw_sb = cpool.tile([128, 1], F32)
            lastconst_dma = nc.sync.dma_start(out=knw_sb, in_=knw[:])
            gatew_sb = cpool.tile([128, CK, E], F16)
            sel8_sb = cpool.tile([E, E * 128], BF16)

            ow_sb = ab.tile([128, CK, HQ, 128], BF16)

            qT_sb = ab.tile([128, HQ, T], BF16)            # per-batch
            kT_sb = ab.tile([128, T], BF16)
            vnat_sb = ab.tile([128, T // 128, 128], BF16)
            rk_sb = ab.tile([128, T // 128], F32)     # per-key-tile rstd_k
            rv_sb = ab.tile([128, T // 128], F32)     # per-key-tile rstd1
            xh_sb = big.tile([128, CK, NOWN], F16)          # own hidden (x+attn)
            xn8_sb = big.tile([128, CK, NOWN], F8)
            prod8_sb = big.tile([128, E, FK, NOWN], F8)
            comb_row = big.tile([E, NOWN], F32)             # routing weights

            def deferred_const_loads():
                nc.sync.dma_start(out=ow_sb, in_=ow[:])
                nc.sync.dma_start(out=masks_sb, in_=masks[:])
                nc.sync.dma_start(out=gatew_sb, in_=gatew[:])
                nc.sync.dma_start(out=sel8_sb, in_=sel8[:])

            rsin = [dram.tile([8, 128, CK, 128], F16, name=f"rsin{b}") for b in range(B)]
            rsout = [dram.tile([128, CK, 128], F16, name=f"rsout{b}") for b in range(B)]

            def bcast(row_f32_ap, width=TB, act_copy=False, **pool_kw):
                """[1,width] f32 -> SBUF [128,width] f32 via K=1 f32r matmul."""
                rr = wk.tile([1, TB], F32R, tag="rwr", bufs=1)
                nc.vector.tensor_copy(out=rr[:, 0:width], in_=row_f32_ap)
                bc_ps = ps.tile([128, TB], F32, **MM)
                nc.tensor.matmul(bc_ps[:, 0:width], onesrow_r,
                                 rr[:, 0:width], start=True, stop=True)
                kw_ = pool_kw or BCS
                bc = wk.tile([128, TB], F32, **kw_)
                if act_copy:
                    nc.scalar.copy(out=bc[:, 0:width], in_=bc_ps[:, 0:width])
                else:
                    nc.vector.tensor_copy(out=bc[:, 0:width], in_=bc_ps[:, 0:width])
                return bc[:, 0:width]

            def bcast8(row_f32_ap):
                """[1,NOWN] f32 -> SBUF [E,NOWN] f32, exact (fp32 matmul so
                is_equal against the source values still holds bitwise)."""
                p8 = ps.tile([8, TB], F32, **ROW)
                nc.tensor.matmul(p8[0:E, 0:NOWN], ones8_f, row_f32_ap,
                                 start=True, stop=True)
                t8 = wk.tile([E, NOWN], F32, **SM8)
                nc.vector.tensor_copy(out=t8, in_=p8[0:E, 0:NOWN])
                return t8

            def colsum_rstd(feat_ps, scale, bias_ap):
                """rsqrt(scale*colsum(feat^2)+bias) -> [1,TB] f32 row."""
                sq = wk.tile([128, TB], BF16, **TBF)
                nc.scalar.activation(out=sq, in_=feat_ps, func=AF.Square)
                ssum = ps.tile([8, TB], F32, **ROW)
                nc.tensor.matmul(ssum[0:1, :], ones_bf, sq, start=True, stop=True)
                srow = wk.tile([1, TB], F32, **RWF)
                nc.scalar.activation(out=srow, in_=ssum[0:1, :], func=AF.Sqrt,
                                     scale=scale, bias=bias_ap)
                rrow = wk.tile([1, TB], F32, **RWF)
                nc.vector.reciprocal(out=rrow, in_=srow)
                return rrow

            def rope_plain(feat_sb, j, w_sb, out_ap):
                """out = rope(w * feat); rstd applied separately (rope is
                linear per token, so the per-token scale commutes)."""
                tcol = j * TB
                qhat = wk.tile([128, TB], BF16, **TBF)
                nc.vector.tensor_scalar(out=qhat, in0=feat_sb, scalar1=w_sb,
                                        scalar2=None, op0=OP.mult)
                rot_ps = ps.tile([128, TB], F32, **MM)
                nc.tensor.matmul(rot_ps, prot_sb, qhat, start=True, stop=True)
                qc = wk.tile([128, TB], BF16, **TBF)
                nc.vector.tensor_tensor(
                    out=qc, in0=qhat, in1=cos_sb[:, tcol:tcol + TB], op=OP.mult,
                )
                rsm = wk.tile([128, TB], BF16, **TBF)
                nc.vector.tensor_tensor(
                    out=rsm, in0=rot_ps, in1=sin_sb[:, tcol:tcol + TB], op=OP.mult,
                )
                nc.vector.tensor_tensor(out=out_ap, in0=qc, in1=rsm, op=OP.add)

            # =========================== Phase A: QKV ======================
            def phaseA(g):
                b, j = divmod(g, 2)
                n0 = g * TB
                if g == 0:
                    xtb = xtb0
                else:
                    xtb = ab.tile([128, CK, TB], BF16, tag="xtb", bufs=1)
                    nc.sync.dma_start(out=xtb[:, 0:CK // 2, :],
                                      in_=xT[:, 0:CK // 2, n0:n0 + TB])
                    nc.sync.dma_start(out=xtb[:, CK // 2:CK, :],
                                      in_=xT[:, CK // 2:CK, n0:n0 + TB])
                r1row = wk.tile([1, TB], F32, **RWF)
                nc.sync.dma_start(out=r1row, in_=rstd1[:, n0:n0 + TB])
                for cc in range(TB // 128):
                    r1t = ps.tile([128, TB], F32, **MM)
                    nc.tensor.transpose(r1t[:, 0:1],
                                        r1row[:, cc * 128:(cc + 1) * 128], idf1)
                    nc.vector.tensor_copy(
                        out=rv_sb[:, 4 * j + cc:4 * j + cc + 1],
                        in_=r1t[:, 0:1])

                def accum(kind, hd):
                    p = ps.tile([128, TB], F32, **MM)
                    for kc in range(CK):
                        if kind == 'q':
                            lhs = qw_sb[:, hd, kc, :]
                        elif kind == 'k':
                            lhs = kw_sb[:, kc, :]
                        else:
                            lhs = vw_sb[:, kc, :]
                        nc.tensor.matmul(p, lhs, xtb[:, kc, :],
                                         start=(kc == 0), stop=(kc == CK - 1))
                    return p

                def stage2(kind, p):
                    # free the psum early: rope reads the bf16 SBUF copy
                    if kind == 'q':
                        row = colsum_rstd(p, 1.0, bias_q)
                    elif kind == 'k':
                        row = colsum_rstd(p, 1.0 / D, bias_eps)
                    else:
                        row = None
                    fsb = wk.tile([128, TB], BF16, tag="fsb", bufs=2)
                    nc.scalar.copy(out=fsb, in_=p)
                    return (row, fsb)

                def stage3(kind, hd, st2):
                    row, fsb = st2
                    if kind == 'q':
                        qbc = bcast(row[:], act_copy=True)
                        qr = wk.tile([128, TB], BF16, **TBF)
                        rope_plain(fsb, j, qnw_sb, qr)
                        nc.vector.tensor_tensor(
                            out=qT_sb[:, hd, j * TB:j * TB + TB],
                            in0=qr, in1=qbc, op=OP.mult)
                    elif kind == 'k':
                        rope_plain(fsb, j, knw_sb,
                                   kT_sb[:, j * TB:j * TB + TB])
                        for cc in range(TB // 128):
                            rkt = ps.tile([128, TB], F32, **MM)
                            nc.tensor.transpose(
                                rkt[:, 0:1], row[:, cc * 128:(cc + 1) * 128],
                                idf1)
                            nc.vector.tensor_copy(
                                out=rk_sb[:, 4 * j + cc:4 * j + cc + 1],
                                in_=rkt[:, 0:1])
                    else:
                        # x's rstd applied post-transpose, where token is the
                        # partition dim (per-partition scalar, no broadcast)
                        for cc in range(TB // 128):
                            vtr = ps.tile([128, 128], BF16, **TRP)
                            nc.tensor.transpose(
                                vtr, fsb[:, cc * 128:(cc + 1) * 128], ident)
                            nc.vector.tensor_scalar(
                                out=vnat_sb[:, j * 4 + cc, :], in0=vtr,
                                scalar1=rv_sb[:, 4 * j + cc:4 * j + cc + 1],
                                scalar2=None, op0=OP.mult)

                seq = [('q', 0), ('q', 1), ('q', 2), ('q', 3),
                       ('k', None), ('v', None)]
                st = []
                for idx, (kind, hd) in enumerate(seq):
                    p = accum(kind, hd)
                    st.append([kind, hd, p, None])
                    if idx >= 1:
                        st[idx - 1][3] = stage2(st[idx - 1][0], st[idx - 1][2])
                    if idx >= 2:
                        stage3(st[idx - 2][0], st[idx - 2][1], st[idx - 2][3])
                st[-1][3] = stage2(st[-1][0], st[-1][2])
                stage3(st[-2][0], st[-2][1], st[-2][3])
                stage3(st[-1][0], st[-1][1], st[-1][3])

            # ====================== Phase B: attention =====================
            def phaseB(g):
                b, j = divmod(g, 2)
                q0 = j * TB
                ntk = 4 * j + 4
                avT = ab.tile([128, HQ, TB], BF16, tag="avT", bufs=1)

                def fin(hd, av_sb, den_ps):
                    dsb = wk.tile([1, TB], F32, **RWF)
                    nc.vector.tensor_copy(out=dsb, in_=den_ps[0:1, :])
                    rec = wk.tile([1, TB], F32, **RWF)
                    nc.vector.reciprocal(out=rec, in_=dsb)
                    rbc = bcast(rec[:])
                    nc.vector.tensor_tensor(out=avT[:, hd, :], in0=av_sb,
                                            in1=rbc, op=OP.mult)

                pend = None
                for hd in range(HQ):
                    av_ps = ps.tile([128, TB], F32, **MM)
                    den_ps = ps.tile([8, TB], F32, **ROW)
                    for i in range(ntk):
                        tk0 = i * 128
                        s = i - 4 * j
                        # diagonal tiles: columns < 128*s are fully masked, so
                        # restrict score/exp/den/av to the live column range
                        c0 = max(s, 0) * 128
                        w = TB - c0
                        sc_ps = ps.tile([128, TB], F32, **MM)
                        nc.tensor.matmul(sc_ps[:, c0:TB], kT_sb[:, tk0:tk0 + 128],
                                         qT_sb[:, hd, q0 + c0:q0 + TB],
                                         start=True, stop=True,
                                         skip_group_check=True)
                        ex = wk.tile([128, TB], BF16, **TBF)
                        if s < 0:
                            nc.scalar.activation(out=ex, in_=sc_ps, func=AF.Exp,
                                                 scale=rk_sb[:, i:i + 1])
                        else:
                            ext = wk.tile([128, TB], BF16, **TBF)
                            nc.scalar.activation(out=ext[:, c0:TB],
                                                 in_=sc_ps[:, c0:TB], func=AF.Exp,
                                                 scale=rk_sb[:, i:i + 1])
                            nc.vector.tensor_tensor(out=ex[:, c0:TB],
                                                    in0=ext[:, c0:TB],
                                                    in1=masks_sb[:, s, c0:TB],
                                                    op=OP.mult)
                        nc.tensor.matmul(den_ps[0:1, c0:TB], ones_bf,
                                         ex[:, c0:TB],
                                         start=(i == 0), stop=(i == ntk - 1),
                                         skip_group_check=True)
                        nc.tensor.matmul(av_ps[:, c0:TB], vnat_sb[:, i, :],
                                         ex[:, c0:TB],
                                         start=(i == 0), stop=(i == ntk - 1),
                                         skip_group_check=True)
                    # free the av psum early via an Act copy; fin reads SBUF
                    av_sb = wk.tile([128, TB], F32, **BCS)
                    nc.scalar.copy(out=av_sb, in_=av_ps)
                    if pend is not None:
                        fin(*pend)
                    pend = (hd, av_sb, den_ps)
                fin(*pend)
                for mq in range(4):
                    attq = ab.tile([128, 4, TB], F16, tag="attb", bufs=2)
                    for mi in range(4):
                        m = 4 * mq + mi
                        att_ps = ps.tile([128, TB], F32, **MM)
                        for hk in range(HQ):
                            nc.tensor.matmul(att_ps, ow_sb[:, m, hk, :],
                                             avT[:, hk, :], start=(hk == 0),
                                             stop=(hk == HQ - 1))
                        nc.scalar.copy(out=attq[:, mi, :], in_=att_ps)
                    for tg in range(4):
                        nc.sync.dma_start(
                            out=rsin[b][4 * j + tg, :, 4 * mq:4 * mq + 4, :],
                            in_=attq[:, :, tg * 128:(tg + 1) * 128])
                if j == 1:
                    nc.gpsimd.collective_compute(
                        "ReduceScatter", OP.add,
                        replica_groups=[list(range(N_CORES))],
                        ins=[rsin[b].opt()], outs=[rsout[b].opt()],
                    )

            # ========================= Phase C: MoE ========================
            HGU = FK * 8 * 2 * 128    # flat size of one gate (or up) half

            def load_guw_half(e, half, eng=None, after=None):
                t = wk.tile([128, FK * 8, 2, 128], F8, tag="wgu", bufs=4)
                d = (eng or nc.gpsimd).dma_start(
                    out=t,
                    in_=guw[:, e, half * HGU:(half + 1) * HGU].rearrange(
                        "p (a b c) -> p a b c", b=2, c=128))
                if after is not None:
                    _add_dep_helper(d.ins, after.ins, sync=True,
                                    reason="prefetch after startup loads")
                return t

            def load_dww(m, eng=None, after=None):
                t = wk.tile([128, E * 3, 2, 128], F8, tag="wdw", bufs=2)
                d = (eng or nc.sync).dma_start(out=t, in_=dww[:, m, :].rearrange(
                    "p (a b c) -> p a b c", b=2, c=128))
                if after is not None:
                    _add_dep_helper(d.ins, after.ins, sync=True,
                                    reason="prefetch after startup loads")
                return t

            def phaseC(pre_gu, pre_dw, moe):
                # assemble own hidden = attn partial sums (+x) for owned tokens
                for b in range(B):
                    nc.sync.dma_start(out=xh_sb[:, :, 128 * b:128 * b + 128],
                                      in_=rsout[b][:])
                for fq in range(4):
                    xow = wk.tile([128, 4, NOWN], F16, tag="xow", bufs=1)
                    nc.gpsimd.dma_start(out=xow, in_=xown[:, 4 * fq:4 * fq + 4, :])
                    for fi in range(4):
                        fc = 4 * fq + fi
                        nc.vector.tensor_tensor(out=xh_sb[:, fc, :],
                                                in0=xh_sb[:, fc, :],
                                                in1=xow[:, fi, :], op=OP.add)
                lg_ps = ps.tile([8, TB], F32, **ROW)
                den2_ps = ps.tile([8, TB], F32, **ROW)
                for fc in range(CK):
                    nc.tensor.matmul(lg_ps[0:E, 0:NOWN], gatew_sb[:, fc, :],
                                     xh_sb[:, fc, :],
                                     start=(fc == 0), stop=(fc == CK - 1))
                    sq = wk.tile([128, NOWN], BF16, **MC)
                    nc.scalar.activation(out=sq, in_=xh_sb[:, fc, :], func=AF.Square)
                    nc.tensor.matmul(den2_ps[0:1, 0:NOWN], ones_bf, sq,
                                     start=(fc == 0), stop=(fc == CK - 1))
                s2 = wk.tile([1, NOWN], F32, **SM1)
                nc.scalar.activation(out=s2, in_=den2_ps[0:1, 0:NOWN], func=AF.Sqrt,
                                     scale=1.0 / C, bias=bias_eps)
                rstd2 = wk.tile([1, NOWN], F32, tag="rstd2", bufs=1)
                nc.vector.reciprocal(out=rstd2, in_=s2)
                r2bc = bcast(rstd2[:], width=NOWN)
                for fc in range(CK):
                    nc.vector.tensor_tensor(out=xn8_sb[:, fc, :],
                                            in0=xh_sb[:, fc, :],
                                            in1=r2bc, op=OP.mult)

                # ---- top-2 routing on [E, NOWN] ----
                lg = wk.tile([E, NOWN], F32, tag="lg", bufs=1)
                nc.vector.tensor_copy(out=lg, in_=lg_ps[0:E, 0:NOWN])
                m1 = wk.tile([1, NOWN], F32, **SM1)
                nc.gpsimd.tensor_reduce(out=m1, in_=lg, axis=mybir.AxisListType.C,
                                        op=OP.max)
                m1bc = bcast8(m1[:])
                eq1 = wk.tile([E, NOWN], F32, tag="eq1", bufs=1)
                nc.vector.tensor_tensor(out=eq1, in0=lg, in1=m1bc, op=OP.is_equal)
                lg2 = wk.tile([E, NOWN], F32, **SM8)
                nc.vector.scalar_tensor_tensor(out=lg2, in0=eq1, scalar=-BIG,
                                               in1=lg, op0=OP.mult, op1=OP.add)
                m2 = wk.tile([1, NOWN], F32, **SM1)
                nc.gpsimd.tensor_reduce(out=m2, in_=lg2, axis=mybir.AxisListType.C,
                                        op=OP.max)
                m2bc = bcast8(m2[:])
                eq2 = wk.tile([E, NOWN], F32, **SM8)
                nc.vector.tensor_tensor(out=eq2, in0=lg, in1=m2bc, op=OP.is_equal)
                # dlt = (m1-m2)*rstd2 ; w1 = sigmoid(dlt); w2 = 1-w1
                dlt = wk.tile([1, NOWN], F32, **SM1)
                nc.vector.tensor_tensor(out=dlt, in0=m1, in1=m2, op=OP.subtract)
                dlts = wk.tile([1, NOWN], F32, **SM1)
                nc.vector.tensor_tensor(out=dlts, in0=dlt, in1=rstd2, op=OP.mult)
                w1 = wk.tile([1, NOWN], F32, **SM1)
                nc.scalar.activation(out=w1, in_=dlts, func=AF.Sigmoid)
                w1bc = bcast8(w1[:])
                # comb = eq1*w1 + eq2*(1-w1) = (eq1-eq2)*w1 + eq2, in place
                nc.vector.tensor_tensor(out=eq1, in0=eq1, in1=eq2, op=OP.subtract)
                nc.vector.tensor_tensor(out=eq1, in0=eq1, in1=w1bc, op=OP.mult)
                nc.vector.tensor_tensor(out=eq1, in0=eq1, in1=eq2, op=OP.add)
                # scale by PSC/WS (prod fp8 scale / up-weight descale)
                nc.vector.tensor_scalar(out=comb_row, in0=eq1, scalar1=PSC / WS,
                                        scalar2=None, op0=OP.mult)

                # ---- pass 1: gate/up + silu -> prod8 per expert ----
                def load_guw_moe(e, half, eng):
                    t = moe.tile([128, FK * 8, 2, 128], F8, tag="wgu2", bufs=5)
                    eng.dma_start(
                        out=t,
                        in_=guw[:, e, half * HGU:(half + 1) * HGU].rearrange(
                            "p (a b c) -> p a b c", b=2, c=128))
                    return t

                def load_dww_moe(m, eng):
                    t = moe.tile([128, E * 3, 2, 128], F8, tag="wdw2", bufs=3)
                    eng.dma_start(out=t, in_=dww[:, m, :].rearrange(
                        "p (a b c) -> p a b c", b=2, c=128))
                    return t

                comb_bf = wk.tile([E, NOWN], BF16, tag="combbf", bufs=1)
                nc.vector.tensor_copy(out=comb_bf, in_=comb_row)
                for e in range(E):
                    wgg = pre_gu.pop((e, 0), None) or load_guw_moe(e, 0, nc.sync)
                    wgu = pre_gu.pop((e, 1), None) or load_guw_moe(e, 1, nc.scalar)
                    cb_ps = ps.tile([128, TB], F32, **MM)
                    nc.tensor.matmul(cb_ps[:, 0:NOWN],
                                     sel8_sb[:, e * 128:(e + 1) * 128],
                                     comb_bf, start=True, stop=True)
                    cbc = wk.tile([128, NOWN], F32, tag="cbc", bufs=1)
                    nc.vector.tensor_copy(out=cbc, in_=cb_ps[:, 0:NOWN])
                    for f in range(FK):
                        g_ps = ps.tile([128, TB], F32, **MM)
                        for kp in range(8):
                            nc.tensor.matmul(
                                g_ps[:, 0:NOWN], wgg[:, f * 8 + kp, :, :],
                                xn8_sb[:, 2 * kp:2 * kp + 2, :],
                                start=(kp == 0), stop=(kp == 7),
                                perf_mode=PM.DoubleRow,
                            )
                        u_ps = ps.tile([128, TB], F32, **MM)
                        for kp in range(8):
                            nc.tensor.matmul(
                                u_ps[:, 0:NOWN], wgu[:, f * 8 + kp, :, :],
                                xn8_sb[:, 2 * kp:2 * kp + 2, :],
                                start=(kp == 0), stop=(kp == 7),
                                perf_mode=PM.DoubleRow,
                            )
                        sil = wk.tile([128, NOWN], BF16, **MC)
                        nc.scalar.activation(out=sil, in_=g_ps[:, 0:NOWN],
                                             func=AF.Silu, scale=1.0 / WS)
                        ucm = wk.tile([128, NOWN], BF16, **MC)
                        nc.vector.tensor_tensor(out=ucm, in0=u_ps[:, 0:NOWN],
                                                in1=cbc, op=OP.mult)
                        nc.vector.tensor_tensor(out=prod8_sb[:, e, f, :],
                                                in0=sil, in1=ucm, op=OP.mult)

                # ---- pass 2: down proj, accumulate experts in psum ----
                for m in range(CK):
                    wd = pre_dw.pop(m, None) or load_dww_moe(m, nc.sync)
                    eo_ps = ps.tile([128, TB], F32, **MM)
                    for e in range(E):
                        for kp in range(3):
                            nc.tensor.matmul(
                                eo_ps[:, 0:NOWN], wd[:, e * 3 + kp, :, :],
                                prod8_sb[:, e, 2 * kp:2 * kp + 2, :],
                                start=(e == 0 and kp == 0),
                                stop=(e == E - 1 and kp == 2),
                                perf_mode=PM.DoubleRow,
                            )
                    ym = wk.tile([128, NOWN], F16, tag="ymc", bufs=2)
                    nc.vector.scalar_tensor_tensor(
                        out=ym, in0=eo_ps[:, 0:NOWN],
                        scalar=1.0 / (WS * PSC), in1=xh_sb[:, m, :],
                        op0=OP.mult, op1=OP.add,
                    )
                    nc.sync.dma_start(out=y[:, m, :], in_=ym)

            pre_gu, pre_dw = {}, {}
            for g in range(NBLK):
                if 'A' in phases:
                    phaseA(g)
                if g == 0:
                    deferred_const_loads()
                    if 'C' in phases:
                        # act-queue prefetches: the Act sequencer reaches these
                        # only after A0's first Square, keeping the DMA engines
                        # free for the critical startup loads
                        for e in range(2):
                            for half in range(2):
                                pre_gu[(e, half)] = load_guw_half(
                                    e, half, nc.scalar, after=lastconst_dma)
                        pre_dw[0] = load_dww(0, nc.scalar, after=lastconst_dma)
                        pre_dw[1] = load_dww(1, nc.scalar, after=lastconst_dma)
                if 'B' in phases:
                    phaseB(g)
            ab.release()
            if 'C' in phases:
                with tc.tile_pool(name="moe", bufs=1) as moe:
                    phaseC(pre_gu, pre_dw, moe)

    _split_multi_waits(nc)
    return nc


# ---------------------------------------------------------------------------

_NC_CACHE = {}


def _get_nc():
    if "nc" not in _NC_CACHE:
        _NC_CACHE["nc"] = build_nc()
    return _NC_CACHE["nc"]


def _chunk_pm(a, nchunk):
    """[nchunk*128, free...] -> [128, nchunk, free...]"""
    return np.ascontiguousarray(
        a.reshape(nchunk, 128, *a.shape[1:]).transpose(1, 0, *range(2, a.ndim + 1))
    )


def prepare_in_maps(x, cos, sin, ln1_w, q_w, k_w, v_w, o_w, qn_w, kn_w, ln2_w,
                    gate_w, gate_up_w, down_w):
    bf = ml_dtypes.bfloat16
    f8 = ml_dtypes.float8_e4m3
    x = np.asarray(x, dtype=np.float32)
    x_flat = x.reshape(N, C)

    xT = _chunk_pm(np.ascontiguousarray(x_flat.T).astype(bf), CK)
    rstd1 = (1.0 / np.sqrt((x_flat.astype(np.float64) ** 2).mean(axis=1) + EPS)
             ).astype(np.float32)[None, :]

    ln1 = np.asarray(ln1_w, dtype=np.float32)[:, None]
    ln2 = np.asarray(ln2_w, dtype=np.float32)[:, None]
    qwf = np.asarray(q_w, dtype=np.float32) * ln1
    kwf = np.asarray(k_w, dtype=np.float32) * ln1
    vwf = np.asarray(v_w, dtype=np.float32) * ln1
    gatewf = np.asarray(gate_w, dtype=np.float32) * ln2
    guwf = np.asarray(gate_up_w, dtype=np.float32) * ln2[None]    # [E, C, 2F]
    dwf = np.asarray(down_w, dtype=np.float32)                    # [E, F, C]
    owf = np.asarray(o_w, dtype=np.float32)

    cos0 = np.asarray(cos, dtype=np.float32)[0]
    sin0 = np.asarray(sin, dtype=np.float32)[0]
    cosT = np.ascontiguousarray(cos0.T).astype(bf)
    sinT = np.ascontiguousarray(sin0.T).astype(bf)
    protm = np.zeros((128, 128), dtype=np.float32)
    for m in range(64):
        protm[m + 64, m] = -1.0
    for m in range(64, 128):
        protm[m - 64, m] = 1.0

    r = np.arange(128)[:, None]
    col = np.arange(TB)[None, :]
    masks = np.stack(
        [(col >= r + 128 * s).astype(bf) for s in range(4)], axis=1
    )

    # fp8 MoE weights, shared across cores
    # guw host layout: [128, E, FGU*8*2*128]; lhsT slice [128, 2, 128] is
    # (grp, kp) with pair index i selecting k-chunk 2kp+i.
    gu6 = (guwf * WS).astype(f8)                       # [E, C, 2F]
    gu_r = gu6.reshape(E, 8, 2, 128, FGU, 128)         # e, kp, i, p, grp, d
    guw_h = np.ascontiguousarray(
        gu_r.transpose(3, 0, 4, 1, 2, 5).reshape(128, E, FGU * 8 * 2 * 128))
    # dww host layout: [128, CK(m), E*3*2*128]; lhsT slice (e, kp) pair i
    # selects f-chunk 2kp+i; partition p = f % 128; d = c within group m.
    dw6 = (dwf * WS).astype(f8)                        # [E, F, C]
    dw_r = dw6.reshape(E, 3, 2, 128, CK, 128)          # e, kp, i, p, m, d
    dww_h = np.ascontiguousarray(
        dw_r.transpose(3, 4, 0, 1, 2, 5).reshape(128, CK, E * 3 * 2 * 128))

    gatew_h = _chunk_pm(gatewf.astype(np.float16), CK)

    in_maps = []
    for c in range(N_CORES):
        oslice = owf[512 * c:512 * (c + 1), :].astype(bf)  # [512, C]
        o4 = oslice.reshape(HQ, 128, CK, 128)              # hk, p, m, d
        ow_h = np.ascontiguousarray(o4.transpose(1, 2, 0, 3))
        # owned tokens: batch b local [128c, 128c+128)
        own_idx = np.concatenate([
            np.arange(b * T + 128 * c, b * T + 128 * (c + 1)) for b in range(B)
        ])
        xo = x_flat[own_idx, :].T                          # [C, 256]
        xown_h = _chunk_pm(np.ascontiguousarray(xo).astype(np.float16), CK)
        in_maps.append({
            "xT": xT,
            "xown": xown_h,
            "qw": np.ascontiguousarray(
                qwf[:, 512 * c:512 * (c + 1)].astype(bf)
                .reshape(CK, 128, HQ, 128).transpose(1, 2, 0, 3)),
            "kw": _chunk_pm(kwf[:, 128 * c:128 * (c + 1)].astype(bf), CK),
            "vw": _chunk_pm(vwf[:, 128 * c:128 * (c + 1)].astype(bf), CK),
            "ow": ow_h,
            "gatew": gatew_h,
            "guw": guw_h,
            "dww": dww_h,
            "cosb": cosT,
            "sinb": sinT,
            "masks": masks,
            "rstd1": rstd1,
            "qnw": np.asarray(qn_w, dtype=np.float32)[:, None],
            "knw": np.asarray(kn_w, dtype=np.float32)[:, None],
            "protb": protm.astype(bf),
            "sel8": np.kron(np.eye(E, dtype=np.float32),
                            np.ones((1, 128), dtype=np.float32)).astype(bf),
        })

    return in_maps


def combine(ys):
    out = np.zeros((N, C), dtype=np.float32)
    for c in range(N_CORES):
        yc = np.asarray(ys[c], dtype=np.float32)     # [128, CK, 256]
        # yc[p, fc, 128*b + i] -> token b*T + 128*c + i, feature fc*128+p
        feat_major = yc.transpose(1, 0, 2).reshape(C, NOWN)
        for b in range(B):
            toks = slice(b * T + 128 * c, b * T + 128 * (c + 1))
            out[toks, :] = feat_major[:, 128 * b:128 * (b + 1)].T
    return out.reshape(B, T, C)


def kernel(**inputs):
    in_maps = prepare_in_maps(**inputs)
    nc = _get_nc()
    res = run_bass_kernel_spmd(nc, in_maps, core_ids=list(range(N_CORES)))
    return combine([res.results[c]["y"] for c in range(N_CORES)])


# revision 7
# speedup vs baseline: 1.1365x; 1.0354x over previous
"""Fused attention+MoE block on 8 trn2 NeuronCores, v2.

Sharding: tensor-parallel attention (4 q-heads + 1 KV-head per core) as
before, but the attention output partials are ReduceScattered (2 per-batch
collectives) so each core ends up owning 256 tokens of the full hidden
state in feature-major layout. The MoE is then data-parallel: every core
runs all 8 experts densely on its own 256 tokens with fp8e4 DoubleRow
matmuls (2 k-chunks per instruction) and fp8 weights streamed from HBM.
Routing (top-2 over E=8) is computed on an [E, 256] tile with
partition-dim max reductions. Output y = hidden + moe for the owned
tokens; the host just concatenates core slices.
"""

import numpy as np
import ml_dtypes

import concourse.bass as bass
from concourse.bass import _add_dep_helper
import concourse.mybir as mybir
import concourse.tile as tile
from concourse.bass_utils import run_bass_kernel_spmd
from concourse.masks import make_identity
from concourse.vector_clock import ScopedClock

F32 = mybir.dt.float32
F32R = mybir.dt.float32r
F16 = mybir.dt.float16
BF16 = mybir.dt.bfloat16
F8 = mybir.dt.float8e4
U32 = mybir.dt.uint32
AF = mybir.ActivationFunctionType
OP = mybir.AluOpType
PM = mybir.MatmulPerfMode

B, T, C = 2, 1024, 2048
H, KV, D = 32, 8, 128
E, F, TOPK = 8, 768, 2
N = B * T
CK = C // 128          # 16
NBLK = 4               # token blocks of 512
TB = 512
HQ = H // 8            # 4 q heads per core
FK = F // 128          # 6
FGU = 2 * F // 128     # 12
NOWN = 256             # tokens owned per core (128 per batch)
WS = 64.0              # fp8 weight scale
PSC = 16.0             # fp8 prod scale
EPS = 1e-6
N_CORES = 8
BIG = 1e9

# ---------------------------------------------------------------------------
# walrus here rejects >1 sync-wait per instruction; split extras onto NoOps.


class _SplitDrainTileContext(tile.TileContext):
    def _drain_and_barrier(self, tick_clock, wait_clock):
        drain_inst = self.nc.sync.drain()
        wait_clock.add_sem_waits(
            drain_inst.ins, ScopedClock({None: tick_clock.global_clock})
        )
        si = drain_inst.ins.sync_info
        if si is not None and len(si.on_wait) > 1:
            ow = list(si.on_wait)
            drain_inst.ins.sync_info = mybir.SyncInfo(
                on_wait=ow[:1], on_update=list(si.on_update)
            )
            rest = ow[1:]
            while rest:
                extra = self.nc.sync.drain()
                extra.ins.sync_info = mybir.SyncInfo(on_wait=rest[:1], on_update=[])
                rest = rest[1:]
        self.nc.all_engine_barrier()
        assert self.sems is not None
        popped = self.nc._tile_sem_poison_stack.pop()
        assert popped is self._sem_poison
        self.nc.clear_and_free_semaphores(list(self.sems.allocated().values()))
        self.nc.all_engine_barrier()


def _split_multi_waits(nc):
    for bb in nc.main_func.blocks:
        insts = list(bb.instructions)
        out = []
        changed = False
        for ins in insts:
            si = ins.sync_info
            if si is not None and len(si.on_wait) > 1:
                ow = list(si.on_wait)
                for w in ow[:-1]:
                    nop = mybir.InstNoOp(name=f"waitnop-{nc.next_id()}", ins=[], outs=[])
                    nop.engine = ins.engine
                    nop.sync_info = mybir.SyncInfo(on_wait=[w], on_update=[])
                    out.append(nop)
                ins.sync_info = mybir.SyncInfo(
                    on_wait=[ow[-1]], on_update=list(si.on_update)
                )
                changed = True
            out.append(ins)
        if changed:
            bb.instructions = out


# ---------------------------------------------------------------------------


def build_nc(phases='ABC'):
    nc = bass.Bass("TRN2", target_bir_lowering=False, debug=False, num_devices=N_CORES)

    xT = nc.dram_tensor("xT", [128, CK, N], BF16, kind="ExternalInput")
    xown = nc.dram_tensor("xown", [128, CK, NOWN], F16, kind="ExternalInput")
    qw = nc.dram_tensor("qw", [128, HQ, CK, 128], BF16, kind="ExternalInput")
    kw = nc.dram_tensor("kw", [128, CK, 128], BF16, kind="ExternalInput")
    vw = nc.dram_tensor("vw", [128, CK, 128], BF16, kind="ExternalInput")
    ow = nc.dram_tensor("ow", [128, CK, HQ, 128], BF16, kind="ExternalInput")
    gatew = nc.dram_tensor("gatew", [128, CK, E], F16, kind="ExternalInput")
    guw = nc.dram_tensor("guw", [128, E, FGU * 8 * 2 * 128], F8, kind="ExternalInput")
    dww = nc.dram_tensor("dww", [128, CK, E * 3 * 2 * 128], F8, kind="ExternalInput")
    cosb = nc.dram_tensor("cosb", [128, T], BF16, kind="ExternalInput")
    sinb = nc.dram_tensor("sinb", [128, T], BF16, kind="ExternalInput")
    masks = nc.dram_tensor("masks", [128, 4, TB], BF16, kind="ExternalInput")
    rstd1 = nc.dram_tensor("rstd1", [1, N], F32, kind="ExternalInput")
    qnw = nc.dram_tensor("qnw", [128, 1], F32, kind="ExternalInput")
    knw = nc.dram_tensor("knw", [128, 1], F32, kind="ExternalInput")
    protb = nc.dram_tensor("protb", [128, 128], BF16, kind="ExternalInput")
    sel8 = nc.dram_tensor("sel8", [E, E * 128], BF16, kind="ExternalInput")

    y = nc.dram_tensor("y", [128, CK, NOWN], F16, kind="ExternalOutput")

    with _SplitDrainTileContext(nc) as tc:
        with (
            tc.tile_pool(name="const", bufs=1) as cpool,
            tc.tile_pool(name="dram", bufs=1, space="DRAM") as dram,
            tc.tile_pool(name="ps", bufs=1, space="PSUM") as ps,
            tc.tile_pool(name="big", bufs=1) as big,
            tc.tile_pool(name="work", bufs=3) as wk,
        ):
            ab = tc.alloc_tile_pool(name="ab", bufs=1)
            MM = dict(tag="mm", bufs=5)       # f32 [128,TB] psum
            ROW = dict(tag="row", bufs=2)     # f32 [<=8,TB] psum
            TRP = dict(tag="trp", bufs=1)     # bf16 [128,128] psum
            TBF = dict(tag="t512b", bufs=3)   # bf16 [128,TB] transients
            RWF = dict(tag="rowf", bufs=2)    # f32 [1,TB]
            BCS = dict(tag="bcs", bufs=3)     # f32 [128,TB] bcast results
            SM8 = dict(tag="sm8", bufs=2)     # f32 [8,NOWN] small routing tiles
            SM1 = dict(tag="sm1", bufs=3)     # f32 [1,NOWN]
            MC = dict(tag="mc", bufs=2)       # bf16 [128,NOWN] moe transients

            # ---- constants ----
            ident = cpool.tile([128, 128], BF16)
            make_identity(nc, ident)
            ones_bf = cpool.tile([128, 1], BF16)
            nc.vector.memset(ones_bf, 1.0)
            onesrow_f = cpool.tile([1, 128], F32)
            nc.vector.memset(onesrow_f, 1.0)
            onesrow_r = cpool.tile([1, 128], F32R)
            nc.vector.tensor_copy(out=onesrow_r, in_=onesrow_f)
            ones8_f = cpool.tile([1, E], F32)
            nc.vector.memset(ones8_f, 1.0)
            ones8_r = cpool.tile([1, E], F32R)
            nc.vector.tensor_copy(out=ones8_r, in_=ones8_f)
            bias_q = cpool.tile([1, 1], F32)
            nc.vector.memset(bias_q, float(D) * EPS)
            bias_eps = cpool.tile([1, 1], F32)
            nc.vector.memset(bias_eps, EPS)
            idf1 = cpool.tile([1, 1], F32)
            nc.vector.memset(idf1, 1.0)

            xtb0 = ab.tile([128, CK, TB], BF16, tag="xtb", bufs=1, name="xtb0")
            nc.sync.dma_start(out=xtb0[:, 0:CK // 2, :], in_=xT[:, 0:CK // 2, 0:TB])
            x0dma = nc.sync.dma_start(out=xtb0[:, CK // 2:CK, :],
                                      in_=xT[:, CK // 2:CK, 0:TB])
            qw_sb = ab.tile([128, HQ, CK, 128], BF16)
            for _hd in range(HQ):
                nc.sync.dma_start(out=qw_sb[:, _hd, :, :], in_=qw[:, _hd, :, :])
            kw_sb = ab.tile([128, CK, 128], BF16)
            nc.sync.dma_start(out=kw_sb, in_=kw[:])
            vw_sb = ab.tile([128, CK, 128], BF16)
            nc.sync.dma_start(out=vw_sb, in_=vw[:])
            prot_sb = cpool.tile([128, 128], BF16)
            nc.sync.dma_start(out=prot_sb, in_=protb[:])
            cos_sb = cpool.tile([128, T], BF16)
            nc.sync.dma_start(out=cos_sb, in_=cosb[:])
            sin_sb = cpool.tile([128, T], BF16)
            nc.sync.dma_start(out=sin_sb, in_=sinb[:])
            masks_sb = cpool.tile([128, 4, TB], BF16)
            qnw_sb = cpool.tile([128, 1], F32)
            nc.sync.dma_start(out=qnw_sb, in_=qnw[:])
            knw_sb = cpool.tile([128, 1], F32)
            lastconst_dma = nc.sync.dma_start(out=knw_sb, in_=knw[:])
            gatew_sb = cpool.tile([128, CK, E], F16)
            sel8_sb = cpool.tile([E, E * 128], BF16)

            ow_sb = ab.tile([128, CK, HQ, 128], BF16)

            qT_sb = ab.tile([128, HQ, T], BF16)            # per-batch
            kT_sb = ab.tile([128, T], BF16)
            vnat_sb = ab.tile([128, T // 128, 128], BF16)
            rk_sb = ab.tile([128, T // 128], F32)     # per-key-tile rstd_k
            rv_sb = ab.tile([128, T // 128], F32)     # per-key-tile rstd1
            xh_sb = big.tile([128, CK, NOWN], F16)          # own hidden (x+attn)
            xn8_sb = big.tile([128, CK, NOWN], F8)
            prod8_sb = big.tile([128, E, FK, NOWN], F8)
            comb_row = big.tile([E, NOWN], F32)             # routing weights

            def deferred_const_loads():
                nc.sync.dma_start(out=ow_sb, in_=ow[:])
                nc.sync.dma_start(out=masks_sb, in_=masks[:])
                nc.sync.dma_start(out=gatew_sb, in_=gatew[:])
                nc.sync.dma_start(out=sel8_sb, in_=sel8[:])

            rsin = [dram.tile([8, 128, CK, 128], F16, name=f"rsin{b}") for b in range(B)]
            rsout = [dram.tile([128, CK, 128], F16, name=f"rsout{b}") for b in range(B)]

            def bcast(row_f32_ap, width=TB, act_copy=False, **pool_kw):
                """[1,width] f32 -> SBUF [128,width] f32 via K=1 f32r matmul."""
                rr = wk.tile([1, TB], F32R, tag="rwr", bufs=1)
                nc.vector.tensor_copy(out=rr[:, 0:width], in_=row_f32_ap)
                bc_ps = ps.tile([128, TB], F32, **MM)
                nc.tensor.matmul(bc_ps[:, 0:width], onesrow_r,
                                 rr[:, 0:width], start=True, stop=True)
                kw_ = pool_kw or BCS
                bc = wk.tile([128, TB], F32, **kw_)
                if act_copy:
                    nc.scalar.copy(out=bc[:, 0:width], in_=bc_ps[:, 0:width])
                else:
                    nc.vector.tensor_copy(out=bc[:, 0:width], in_=bc_ps[:, 0:width])
                return bc[:, 0:width]

            def bcast8(row_f32_ap):
                """[1,NOWN] f32 -> SBUF [E,NOWN] f32, exact (fp32 matmul so
                is_equal against the source values still holds bitwise)."""
                p8 = ps.tile([8, TB], F32, **ROW)
                nc.tensor.matmul(p8[0:E, 0:NOWN], ones8_f, row_f32_ap,
                                 start=True, stop=True)
                t8 = wk.tile([E, NOWN], F32, **SM8)
                nc.vector.tensor_copy(out=t8, in_=p8[0:E, 0:NOWN])
                return t8

            def colsum_rstd(feat_ps, scale, bias_ap):
                """rsqrt(scale*colsum(feat^2)+bias) -> [1,TB] f32 row."""
                sq = wk.tile([128, TB], BF16, **TBF)
                nc.scalar.activation(out=sq, in_=feat_ps, func=AF.Square)
                ssum = ps.tile([8, TB], F32, **ROW)
                nc.tensor.matmul(ssum[0:1, :], ones_bf, sq, start=True, stop=True)
                srow = wk.tile([1, TB], F32, **RWF)
                nc.scalar.activation(out=srow, in_=ssum[0:1, :], func=AF.Sqrt,
                                     scale=scale, bias=bias_ap)
                rrow = wk.tile([1, TB], F32, **RWF)
                nc.vector.reciprocal(out=rrow, in_=srow)
                return rrow

            def rope_plain(feat_sb, j, w_sb, out_ap):
                """out = rope(w * feat); rstd applied separately (rope is
                linear per token, so the per-token scale commutes)."""
                tcol = j * TB
                qhat = wk.tile([128, TB], BF16, **TBF)
                nc.vector.tensor_scalar(out=qhat, in0=feat_sb, scalar1=w_sb,
                                        scalar2=None, op0=OP.mult)
                rot_ps = ps.tile([128, TB], F32, **MM)
                nc.tensor.matmul(rot_ps, prot_sb, qhat, start=True, stop=True)
                qc = wk.tile([128, TB], BF16, **TBF)
                nc.vector.tensor_tensor(
                    out=qc, in0=qhat, in1=cos_sb[:, tcol:tcol + TB], op=OP.mult,
                )
                rsm = wk.tile([128, TB], BF16, **TBF)
                nc.vector.tensor_tensor(
                    out=rsm, in0=rot_ps, in1=sin_sb[:, tcol:tcol + TB], op=OP.mult,
                )
                nc.vector.tensor_tensor(out=out_ap, in0=qc, in1=rsm, op=OP.add)

            # =========================== Phase A: QKV ======================
            def phaseA(g):
                b, j = divmod(g, 2)
                n0 = g * TB
                if g == 0:
                    xtb = xtb0
                else:
                    xtb = ab.tile([128, CK, TB], BF16, tag="xtb", bufs=1)
                    nc.sync.dma_start(out=xtb[:, 0:CK // 2, :],
                                      in_=xT[:, 0:CK // 2, n0:n0 + TB])
                    nc.sync.dma_start(out=xtb[:, CK // 2:CK, :],
                                      in_=xT[:, CK // 2:CK, n0:n0 + TB])
                r1row = wk.tile([1, TB], F32, **RWF)
                nc.sync.dma_start(out=r1row, in_=rstd1[:, n0:n0 + TB])
                for cc in range(TB // 128):
                    r1t = ps.tile([128, TB], F32, **MM)
                    nc.tensor.transpose(r1t[:, 0:1],
                                        r1row[:, cc * 128:(cc + 1) * 128], idf1)
                    nc.vector.tensor_copy(
                        out=rv_sb[:, 4 * j + cc:4 * j + cc + 1],
                        in_=r1t[:, 0:1])

                def accum(kind, hd):
                    p = ps.tile([128, TB], F32, **MM)
                    for kc in range(CK):
                        if kind == 'q':
                            lhs = qw_sb[:, hd, kc, :]
                        elif kind == 'k':
                            lhs = kw_sb[:, kc, :]
                        else:
                            lhs = vw_sb[:, kc, :]
                        nc.tensor.matmul(p, lhs, xtb[:, kc, :],
                                         start=(kc == 0), stop=(kc == CK - 1))
                    return p

                def stage2(kind, p):
                    # free the psum early: rope reads the bf16 SBUF copy
                    if kind == 'q':
                        row = colsum_rstd(p, 1.0, bias_q)
                    elif kind == 'k':
                        row = colsum_rstd(p, 1.0 / D, bias_eps)
                    else:
                        row = None
                    fsb = wk.tile([128, TB], BF16, tag="fsb", bufs=2)
                    nc.scalar.copy(out=fsb, in_=p)
                    return (row, fsb)

                def stage3(kind, hd, st2):
                    row, fsb = st2
                    if kind == 'q':
                        qbc = bcast(row[:], act_copy=True)
                        qr = wk.tile([128, TB], BF16, **TBF)
                        rope_plain(fsb, j, qnw_sb, qr)
                        nc.vector.tensor_tensor(
                            out=qT_sb[:, hd, j * TB:j * TB + TB],
                            in0=qr, in1=qbc, op=OP.mult)
                    elif kind == 'k':
                        rope_plain(fsb, j, knw_sb,
                                   kT_sb[:, j * TB:j * TB + TB])
                        for cc in range(TB // 128):
                            rkt = ps.tile([128, TB], F32, **MM)
                            nc.tensor.transpose(
                                rkt[:, 0:1], row[:, cc * 128:(cc + 1) * 128],
                                idf1)
                            nc.vector.tensor_copy(
                                out=rk_sb[:, 4 * j + cc:4 * j + cc + 1],
                                in_=rkt[:, 0:1])
                    else:
                        # x's rstd applied post-transpose, where token is the
                        # partition dim (per-partition scalar, no broadcast)
                        for cc in range(TB // 128):
                            vtr = ps.tile([128, 128], BF16, **TRP)
                            nc.tensor.transpose(
                                vtr, fsb[:, cc * 128:(cc + 1) * 128], ident)
                            nc.vector.tensor_scalar(
                                out=vnat_sb[:, j * 4 + cc, :], in0=vtr,
                                scalar1=rv_sb[:, 4 * j + cc:4 * j + cc + 1],
                                scalar2=None, op0=OP.mult)

                seq = [('q', 0), ('q', 1), ('q', 2), ('q', 3),
                       ('k', None), ('v', None)]
                st = []
                for idx, (kind, hd) in enumerate(seq):
                    p = accum(kind, hd)
                    st.append([kind, hd, p, None])
                    if idx >= 1:
                        st[idx - 1][3] = stage2(st[idx - 1][0], st[idx - 1][2])
                    if idx >= 2:
                        stage3(st[idx - 2][0], st[idx - 2][1], st[idx - 2][3])
                st[-1][3] = stage2(st[-1][0], st[-1][2])
                stage3(st[-2][0], st[-2][1], st[-2][3])
                stage3(st[-1][0], st[-1][1], st[-1][3])

            # ====================== Phase B: attention =====================
            def phaseB(g):
                b, j = divmod(g, 2)
                q0 = j * TB
                ntk = 4 * j + 4
                avT = ab.tile([128, HQ, TB], BF16, tag="avT", bufs=1)

                def fin(hd, av_sb, den_ps):
                    dsb = wk.tile([1, TB], F32, **RWF)
                    nc.vector.tensor_copy(out=dsb, in_=den_ps[0:1, :])
                    rec = wk.tile([1, TB], F32, **RWF)
                    nc.vector.reciprocal(out=rec, in_=dsb)
                    rbc = bcast(rec[:])
                    nc.vector.tensor_tensor(out=avT[:, hd, :], in0=av_sb,
                                            in1=rbc, op=OP.mult)

                pend = None
                for hd in range(HQ):
                    av_ps = ps.tile([128, TB], F32, **MM)
                    den_ps = ps.tile([8, TB], F32, **ROW)
                    for i in range(ntk):
                        tk0 = i * 128
                        s = i - 4 * j
                        # diagonal tiles: columns < 128*s are fully masked, so
                        # restrict score/exp/den/av to the live column range
                        c0 = max(s, 0) * 128
                        w = TB - c0
                        sc_ps = ps.tile([128, TB], F32, **MM)
                        nc.tensor.matmul(sc_ps[:, c0:TB], kT_sb[:, tk0:tk0 + 128],
                                         qT_sb[:, hd, q0 + c0:q0 + TB],
                                         start=True, stop=True,
                                         skip_group_check=True)
                        ex = wk.tile([128, TB], BF16, **TBF)
                        if s < 0:
                            nc.scalar.activation(out=ex, in_=sc_ps, func=AF.Exp,
                                                 scale=rk_sb[:, i:i + 1])
                        else:
                            ext = wk.tile([128, TB], BF16, **TBF)
                            nc.scalar.activation(out=ext[:, c0:TB],
                                                 in_=sc_ps[:, c0:TB], func=AF.Exp,
                                                 scale=rk_sb[:, i:i + 1])
                            nc.vector.tensor_tensor(out=ex[:, c0:TB],
                                                    in0=ext[:, c0:TB],
                                                    in1=masks_sb[:, s, c0:TB],
                                                    op=OP.mult)
                        nc.tensor.matmul(den_ps[0:1, c0:TB], ones_bf,
                                         ex[:, c0:TB],
                                         start=(i == 0), stop=(i == ntk - 1),
                                         skip_group_check=True)
                        nc.tensor.matmul(av_ps[:, c0:TB], vnat_sb[:, i, :],
                                         ex[:, c0:TB],
                                         start=(i == 0), stop=(i == ntk - 1),
                                         skip_group_check=True)
                    # free the av psum early via an Act copy; fin reads SBUF
                    av_sb = wk.tile([128, TB], F32, **BCS)
                    nc.scalar.copy(out=av_sb, in_=av_ps)
                    if pend is not None:
                        fin(*pend)
                    pend = (hd, av_sb, den_ps)
                fin(*pend)
                for mq in range(4):
                    attq = ab.tile([128, 4, TB], F16, tag="attb", bufs=2)
                    for mi in range(4):
                        m = 4 * mq + mi
                        att_ps = ps.tile([128, TB], F32, **MM)
                        for hk in range(HQ):
                            nc.tensor.matmul(att_ps, ow_sb[:, m, hk, :],
                                             avT[:, hk, :], start=(hk == 0),
                                             stop=(hk == HQ - 1))
                        nc.scalar.copy(out=attq[:, mi, :], in_=att_ps)
                    for tg in range(4):
                        nc.sync.dma_start(
                            out=rsin[b][4 * j + tg, :, 4 * mq:4 * mq + 4, :],
                            in_=attq[:, :, tg * 128:(tg + 1) * 128])
                if j == 1:
                    nc.gpsimd.collective_compute(
                        "ReduceScatter", OP.add,
                        replica_groups=[list(range(N_CORES))],
                        ins=[rsin[b].opt()], outs=[rsout[b].opt()],
                    )

            # ========================= Phase C: MoE ========================
            HGU = FK * 8 * 2 * 128    # flat size of one gate (or up) half

            def load_guw_half(e, half, eng=None, after=None):
                t = wk.tile([128, FK * 8, 2, 128], F8, tag="wgu", bufs=4)
                d = (eng or nc.gpsimd).dma_start(
                    out=t,
                    in_=guw[:, e, half * HGU:(half + 1) * HGU].rearrange(
                        "p (a b c) -> p a b c", b=2, c=128))
                if after is not None:
                    _add_dep_helper(d.ins, after.ins, sync=True,
                                    reason="prefetch after startup loads")
                return t

            def load_dww(m, eng=None, after=None):
                t = wk.tile([128, E * 3, 2, 128], F8, tag="wdw", bufs=2)
                d = (eng or nc.sync).dma_start(out=t, in_=dww[:, m, :].rearrange(
                    "p (a b c) -> p a b c", b=2, c=128))
                if after is not None:
                    _add_dep_helper(d.ins, after.ins, sync=True,
                                    reason="prefetch after startup loads")
                return t

            def phaseC(pre_gu, pre_dw, moe):
                # assemble own hidden = attn partial sums (+x) for owned tokens
                for b in range(B):
                    nc.sync.dma_start(out=xh_sb[:, :, 128 * b:128 * b + 128],
                                      in_=rsout[b][:])
                xow = moe.tile([128, CK, NOWN], F16, name="xow_all")
                nc.sync.dma_start(out=xow, in_=xown[:])
                for fc in range(CK):
                    nc.vector.tensor_tensor(out=xh_sb[:, fc, :],
                                            in0=xh_sb[:, fc, :],
                                            in1=xow[:, fc, :], op=OP.add)
                lg_ps = ps.tile([8, TB], F32, **ROW)
                den2_ps = ps.tile([8, TB], F32, **ROW)
                for fc in range(CK):
                    nc.tensor.matmul(lg_ps[0:E, 0:NOWN], gatew_sb[:, fc, :],
                                     xh_sb[:, fc, :],
                                     start=(fc == 0), stop=(fc == CK - 1))
                    sq = wk.tile([128, NOWN], BF16, **MC)
                    nc.scalar.activation(out=sq, in_=xh_sb[:, fc, :], func=AF.Square)
                    nc.tensor.matmul(den2_ps[0:1, 0:NOWN], ones_bf, sq,
                                     start=(fc == 0), stop=(fc == CK - 1))
                s2 = wk.tile([1, NOWN], F32, **SM1)
                nc.scalar.activation(out=s2, in_=den2_ps[0:1, 0:NOWN], func=AF.Sqrt,
                                     scale=1.0 / C, bias=bias_eps)
                rstd2 = wk.tile([1, NOWN], F32, tag="rstd2", bufs=1)
                nc.vector.reciprocal(out=rstd2, in_=s2)
                r2bc = bcast(rstd2[:], width=NOWN)
                for fc in range(CK):
                    nc.vector.tensor_tensor(out=xn8_sb[:, fc, :],
                                            in0=xh_sb[:, fc, :],
                                            in1=r2bc, op=OP.mult)

                # ---- top-2 routing on [E, NOWN] ----
                lg = wk.tile([E, NOWN], F32, tag="lg", bufs=1)
                nc.vector.tensor_copy(out=lg, in_=lg_ps[0:E, 0:NOWN])
                m1 = wk.tile([1, NOWN], F32, **SM1)
                nc.gpsimd.tensor_reduce(out=m1, in_=lg, axis=mybir.AxisListType.C,
                                        op=OP.max)
                m1bc = bcast8(m1[:])
                eq1 = wk.tile([E, NOWN], F32, tag="eq1", bufs=1)
                nc.vector.tensor_tensor(out=eq1, in0=lg, in1=m1bc, op=OP.is_equal)
                lg2 = wk.tile([E, NOWN], F32, **SM8)
                nc.vector.scalar_tensor_tensor(out=lg2, in0=eq1, scalar=-BIG,
                                               in1=lg, op0=OP.mult, op1=OP.add)
                m2 = wk.tile([1, NOWN], F32, **SM1)
                nc.gpsimd.tensor_reduce(out=m2, in_=lg2, axis=mybir.AxisListType.C,
                                        op=OP.max)
                m2bc = bcast8(m2[:])
                eq2 = wk.tile([E, NOWN], F32, **SM8)
                nc.vector.tensor_tensor(out=eq2, in0=lg, in1=m2bc, op=OP.is_equal)
                # dlt = (m1-m2)*rstd2 ; w1 = sigmoid(dlt); w2 = 1-w1
                dlt = wk.tile([1, NOWN], F32, **SM1)
                nc.vector.tensor_tensor(out=dlt, in0=m1, in1=m2, op=OP.subtract)
                dlts = wk.tile([1, NOWN], F32, **SM1)
                nc.vector.tensor_tensor(out=dlts, in0=dlt, in1=rstd2, op=OP.mult)
                w1 = wk.tile([1, NOWN], F32, **SM1)
                nc.scalar.activation(out=w1, in_=dlts, func=AF.Sigmoid)
                w1bc = bcast8(w1[:])
                # comb = eq1*w1 + eq2*(1-w1) = (eq1-eq2)*w1 + eq2, in place
                nc.vector.tensor_tensor(out=eq1, in0=eq1, in1=eq2, op=OP.subtract)
                nc.vector.tensor_tensor(out=eq1, in0=eq1, in1=w1bc, op=OP.mult)
                nc.vector.tensor_tensor(out=eq1, in0=eq1, in1=eq2, op=OP.add)
                # scale by PSC/WS (prod fp8 scale / up-weight descale)
                nc.vector.tensor_scalar(out=comb_row, in0=eq1, scalar1=PSC / WS,
                                        scalar2=None, op0=OP.mult)

                # ---- pass 1: gate/up + silu -> prod8 per expert ----
                def load_guw_moe(e, half, eng):
                    t = moe.tile([128, FK * 8, 2, 128], F8, tag="wgu2", bufs=4)
                    eng.dma_start(
                        out=t,
                        in_=guw[:, e, half * HGU:(half + 1) * HGU].rearrange(
                            "p (a b c) -> p a b c", b=2, c=128))
                    return t

                def load_dww_moe(m, eng):
                    t = moe.tile([128, E * 3, 2, 128], F8, tag="wdw2", bufs=4)
                    eng.dma_start(out=t, in_=dww[:, m, :].rearrange(
                        "p (a b c) -> p a b c", b=2, c=128))
                    return t

                comb_bf = wk.tile([E, NOWN], BF16, tag="combbf", bufs=1)
                nc.vector.tensor_copy(out=comb_bf, in_=comb_row)
                for e in range(E):
                    wgg = pre_gu.pop((e, 0), None) or load_guw_moe(e, 0, nc.sync)
                    wgu = pre_gu.pop((e, 1), None) or load_guw_moe(e, 1, nc.scalar)
                    cb_ps = ps.tile([128, TB], F32, **MM)
                    nc.tensor.matmul(cb_ps[:, 0:NOWN],
                                     sel8_sb[:, e * 128:(e + 1) * 128],
                                     comb_bf, start=True, stop=True)
                    cbc = wk.tile([128, NOWN], F32, tag="cbc", bufs=1)
                    nc.vector.tensor_copy(out=cbc, in_=cb_ps[:, 0:NOWN])
                    for f in range(FK):
                        g_ps = ps.tile([128, TB], F32, **MM)
                        for kp in range(8):
                            nc.tensor.matmul(
                                g_ps[:, 0:NOWN], wgg[:, f * 8 + kp, :, :],
                                xn8_sb[:, 2 * kp:2 * kp + 2, :],
                                start=(kp == 0), stop=(kp == 7),
                                perf_mode=PM.DoubleRow,
                            )
                        u_ps = ps.tile([128, TB], F32, **MM)
                        for kp in range(8):
                            nc.tensor.matmul(
                                u_ps[:, 0:NOWN], wgu[:, f * 8 + kp, :, :],
                                xn8_sb[:, 2 * kp:2 * kp + 2, :],
                                start=(kp == 0), stop=(kp == 7),
                                perf_mode=PM.DoubleRow,
                            )
                        sil = wk.tile([128, NOWN], BF16, **MC)
                        nc.scalar.activation(out=sil, in_=g_ps[:, 0:NOWN],
                                             func=AF.Silu, scale=1.0 / WS)
                        ucm = wk.tile([128, NOWN], BF16, **MC)
                        nc.vector.tensor_tensor(out=ucm, in0=u_ps[:, 0:NOWN],
                                                in1=cbc, op=OP.mult)
                        nc.vector.tensor_tensor(out=prod8_sb[:, e, f, :],
                                                in0=sil, in1=ucm, op=OP.mult)

                # ---- pass 2: down proj, accumulate experts in psum ----
                for m in range(CK):
                    wd = pre_dw.pop(m, None) or load_dww_moe(m, nc.sync)
                    eo_ps = ps.tile([128, TB], F32, **MM)
                    for e in range(E):
                        for kp in range(3):
                            nc.tensor.matmul(
                                eo_ps[:, 0:NOWN], wd[:, e * 3 + kp, :, :],
                                prod8_sb[:, e, 2 * kp:2 * kp + 2, :],
                                start=(e == 0 and kp == 0),
                                stop=(e == E - 1 and kp == 2),
                                perf_mode=PM.DoubleRow,
                            )
                    ym = wk.tile([128, NOWN], F16, tag="ymc", bufs=2)
                    nc.vector.scalar_tensor_tensor(
                        out=ym, in0=eo_ps[:, 0:NOWN],
                        scalar=1.0 / (WS * PSC), in1=xh_sb[:, m, :],
                        op0=OP.mult, op1=OP.add,
                    )
                    nc.sync.dma_start(out=y[:, m, :], in_=ym)

            pre_gu, pre_dw = {}, {}
            for g in range(NBLK):
                if 'A' in phases:
                    phaseA(g)
                if g == 0:
                    deferred_const_loads()
                    if 'C' in phases:
                        # act-queue prefetches: the Act sequencer reaches these
                        # only after A0's first Square, keeping the DMA engines
                        # free for the critical startup loads
                        for e in range(2):
                            for half in range(2):
                                pre_gu[(e, half)] = load_guw_half(
                                    e, half, nc.scalar, after=lastconst_dma)
                        pre_dw[0] = load_dww(0, nc.scalar, after=lastconst_dma)
                        pre_dw[1] = load_dww(1, nc.scalar, after=lastconst_dma)
                if 'B' in phases:
                    phaseB(g)
            ab.release()
            if 'C' in phases:
                with tc.tile_pool(name="moe", bufs=1) as moe:
                    phaseC(pre_gu, pre_dw, moe)

    _split_multi_waits(nc)
    return nc


# ---------------------------------------------------------------------------

_NC_CACHE = {}


def _get_nc():
    if "nc" not in _NC_CACHE:
        _NC_CACHE["nc"] = build_nc()
    return _NC_CACHE["nc"]


def _chunk_pm(a, nchunk):
    """[nchunk*128, free...] -> [128, nchunk, free...]"""
    return np.ascontiguousarray(
        a.reshape(nchunk, 128, *a.shape[1:]).transpose(1, 0, *range(2, a.ndim + 1))
    )


def prepare_in_maps(x, cos, sin, ln1_w, q_w, k_w, v_w, o_w, qn_w, kn_w, ln2_w,
                    gate_w, gate_up_w, down_w):
    bf = ml_dtypes.bfloat16
    f8 = ml_dtypes.float8_e4m3
    x = np.asarray(x, dtype=np.float32)
    x_flat = x.reshape(N, C)

    xT = _chunk_pm(np.ascontiguousarray(x_flat.T).astype(bf), CK)
    rstd1 = (1.0 / np.sqrt((x_flat.astype(np.float64) ** 2).mean(axis=1) + EPS)
             ).astype(np.float32)[None, :]

    ln1 = np.asarray(ln1_w, dtype=np.float32)[:, None]
    ln2 = np.asarray(ln2_w, dtype=np.float32)[:, None]
    qwf = np.asarray(q_w, dtype=np.float32) * ln1
    kwf = np.asarray(k_w, dtype=np.float32) * ln1
    vwf = np.asarray(v_w, dtype=np.float32) * ln1
    gatewf = np.asarray(gate_w, dtype=np.float32) * ln2
    guwf = np.asarray(gate_up_w, dtype=np.float32) * ln2[None]    # [E, C, 2F]
    dwf = np.asarray(down_w, dtype=np.float32)                    # [E, F, C]
    owf = np.asarray(o_w, dtype=np.float32)

    cos0 = np.asarray(cos, dtype=np.float32)[0]
    sin0 = np.asarray(sin, dtype=np.float32)[0]
    cosT = np.ascontiguousarray(cos0.T).astype(bf)
    sinT = np.ascontiguousarray(sin0.T).astype(bf)
    protm = np.zeros((128, 128), dtype=np.float32)
    for m in range(64):
        protm[m + 64, m] = -1.0
    for m in range(64, 128):
        protm[m - 64, m] = 1.0

    r = np.arange(128)[:, None]
    col = np.arange(TB)[None, :]
    masks = np.stack(
        [(col >= r + 128 * s).astype(bf) for s in range(4)], axis=1
    )

    # fp8 MoE weights, shared across cores
    # guw host layout: [128, E, FGU*8*2*128]; lhsT slice [128, 2, 128] is
    # (grp, kp) with pair index i selecting k-chunk 2kp+i.
    gu6 = (guwf * WS).astype(f8)                       # [E, C, 2F]
    gu_r = gu6.reshape(E, 8, 2, 128, FGU, 128)         # e, kp, i, p, grp, d
    guw_h = np.ascontiguousarray(
        gu_r.transpose(3, 0, 4, 1, 2, 5).reshape(128, E, FGU * 8 * 2 * 128))
    # dww host layout: [128, CK(m), E*3*2*128]; lhsT slice (e, kp) pair i
    # selects f-chunk 2kp+i; partition p = f % 128; d = c within group m.
    dw6 = (dwf * WS).astype(f8)                        # [E, F, C]
    dw_r = dw6.reshape(E, 3, 2, 128, CK, 128)          # e, kp, i, p, m, d
    dww_h = np.ascontiguousarray(
        dw_r.transpose(3, 4, 0, 1, 2, 5).reshape(128, CK, E * 3 * 2 * 128))

    gatew_h = _chunk_pm(gatewf.astype(np.float16), CK)

    in_maps = []
    for c in range(N_CORES):
        oslice = owf[512 * c:512 * (c + 1), :].astype(bf)  # [512, C]
        o4 = oslice.reshape(HQ, 128, CK, 128)              # hk, p, m, d
        ow_h = np.ascontiguousarray(o4.transpose(1, 2, 0, 3))
        # owned tokens: batch b local [128c, 128c+128)
        own_idx = np.concatenate([
            np.arange(b * T + 128 * c, b * T + 128 * (c + 1)) for b in range(B)
        ])
        xo = x_flat[own_idx, :].T                          # [C, 256]
        xown_h = _chunk_pm(np.ascontiguousarray(xo).astype(np.float16), CK)
        in_maps.append({
            "xT": xT,
            "xown": xown_h,
            "qw": np.ascontiguousarray(
                qwf[:, 512 * c:512 * (c + 1)].astype(bf)
                .reshape(CK, 128, HQ, 128).transpose(1, 2, 0, 3)),
            "kw": _chunk_pm(kwf[:, 128 * c:128 * (c + 1)].astype(bf), CK),
            "vw": _chunk_pm(vwf[:, 128 * c:128 * (c + 1)].astype(bf), CK),
            "ow": ow_h,
            "gatew": gatew_h,
            "guw": guw_h,
            "dww": dww_h,
            "cosb": cosT,
            "sinb": sinT,
            "masks": masks,
            "rstd1": rstd1,
            "qnw": np.asarray(qn_w, dtype=np.float32)[:, None],
            "knw": np.asarray(kn_w, dtype=np.float32)[:, None],
            "protb": protm.astype(bf),
            "sel8": np.kron(np.eye(E, dtype=np.float32),
                            np.ones((1, 128), dtype=np.float32)).astype(bf),
        })

    return in_maps


def combine(ys):
    out = np.zeros((N, C), dtype=np.float32)
    for c in range(N_CORES):
        yc = np.asarray(ys[c], dtype=np.float32)     # [128, CK, 256]
        # yc[p, fc, 128*b + i] -> token b*T + 128*c + i, feature fc*128+p
        feat_major = yc.transpose(1, 0, 2).reshape(C, NOWN)
        for b in range(B):
            toks = slice(b * T + 128 * c, b * T + 128 * (c + 1))
            out[toks, :] = feat_major[:, 128 * b:128 * (b + 1)].T
    return out.reshape(B, T, C)


def kernel(**inputs):
    in_maps = prepare_in_maps(**inputs)
    nc = _get_nc()
    res = run_bass_kernel_spmd(nc, in_maps, core_ids=list(range(N_CORES)))
    return combine([res.results[c]["y"] for c in range(N_CORES)])


# revision 8
# speedup vs baseline: 1.1407x; 1.0037x over previous
"""Fused attention+MoE block on 8 trn2 NeuronCores, v2.

Sharding: tensor-parallel attention (4 q-heads + 1 KV-head per core) as
before, but the attention output partials are ReduceScattered (2 per-batch
collectives) so each core ends up owning 256 tokens of the full hidden
state in feature-major layout. The MoE is then data-parallel: every core
runs all 8 experts densely on its own 256 tokens with fp8e4 DoubleRow
matmuls (2 k-chunks per instruction) and fp8 weights streamed from HBM.
Routing (top-2 over E=8) is computed on an [E, 256] tile with
partition-dim max reductions. Output y = hidden + moe for the owned
tokens; the host just concatenates core slices.
"""

import numpy as np
import ml_dtypes

import concourse.bass as bass
from concourse.bass import _add_dep_helper
import concourse.mybir as mybir
import concourse.tile as tile
from concourse.bass_utils import run_bass_kernel_spmd
from concourse.masks import make_identity
from concourse.vector_clock import ScopedClock

F32 = mybir.dt.float32
F32R = mybir.dt.float32r
F16 = mybir.dt.float16
BF16 = mybir.dt.bfloat16
F8 = mybir.dt.float8e4
U32 = mybir.dt.uint32
AF = mybir.ActivationFunctionType
OP = mybir.AluOpType
PM = mybir.MatmulPerfMode

B, T, C = 2, 1024, 2048
H, KV, D = 32, 8, 128
E, F, TOPK = 8, 768, 2
N = B * T
CK = C // 128          # 16
NBLK = 4               # token blocks of 512
TB = 512
HQ = H // 8            # 4 q heads per core
FK = F // 128          # 6
FGU = 2 * F // 128     # 12
NOWN = 256             # tokens owned per core (128 per batch)
WS = 64.0              # fp8 weight scale
PSC = 16.0             # fp8 prod scale
EPS = 1e-6
N_CORES = 8
BIG = 1e9

# ---------------------------------------------------------------------------
# walrus here rejects >1 sync-wait per instruction; split extras onto NoOps.


class _SplitDrainTileContext(tile.TileContext):
    def _drain_and_barrier(self, tick_clock, wait_clock):
        drain_inst = self.nc.sync.drain()
        wait_clock.add_sem_waits(
            drain_inst.ins, ScopedClock({None: tick_clock.global_clock})
        )
        si = drain_inst.ins.sync_info
        if si is not None and len(si.on_wait) > 1:
            ow = list(si.on_wait)
            drain_inst.ins.sync_info = mybir.SyncInfo(
                on_wait=ow[:1], on_update=list(si.on_update)
            )
            rest = ow[1:]
            while rest:
                extra = self.nc.sync.drain()
                extra.ins.sync_info = mybir.SyncInfo(on_wait=rest[:1], on_update=[])
                rest = rest[1:]
        self.nc.all_engine_barrier()
        assert self.sems is not None
        popped = self.nc._tile_sem_poison_stack.pop()
        assert popped is self._sem_poison
        self.nc.clear_and_free_semaphores(list(self.sems.allocated().values()))
        self.nc.all_engine_barrier()


def _split_multi_waits(nc):
    for bb in nc.main_func.blocks:
        insts = list(bb.instructions)
        out = []
        changed = False
        for ins in insts:
            si = ins.sync_info
            if si is not None and len(si.on_wait) > 1:
                ow = list(si.on_wait)
                for w in ow[:-1]:
                    nop = mybir.InstNoOp(name=f"waitnop-{nc.next_id()}", ins=[], outs=[])
                    nop.engine = ins.engine
                    nop.sync_info = mybir.SyncInfo(on_wait=[w], on_update=[])
                    out.append(nop)
                ins.sync_info = mybir.SyncInfo(
                    on_wait=[ow[-1]], on_update=list(si.on_update)
                )
                changed = True
            out.append(ins)
        if changed:
            bb.instructions = out


# ---------------------------------------------------------------------------


def build_nc(phases='ABC'):
    nc = bass.Bass("TRN2", target_bir_lowering=False, debug=False, num_devices=N_CORES)

    xT = nc.dram_tensor("xT", [128, CK, N], BF16, kind="ExternalInput")
    xown = nc.dram_tensor("xown", [128, CK, NOWN], F16, kind="ExternalInput")
    qw = nc.dram_tensor("qw", [128, HQ, CK, 128], BF16, kind="ExternalInput")
    kw = nc.dram_tensor("kw", [128, CK, 128], BF16, kind="ExternalInput")
    vw = nc.dram_tensor("vw", [128, CK, 128], BF16, kind="ExternalInput")
    ow = nc.dram_tensor("ow", [128, CK, HQ, 128], BF16, kind="ExternalInput")
    gatew = nc.dram_tensor("gatew", [128, CK, E], F16, kind="ExternalInput")
    guw = nc.dram_tensor("guw", [128, E, FGU * 8 * 2 * 128], F8, kind="ExternalInput")
    dww = nc.dram_tensor("dww", [128, CK, E * 3 * 2 * 128], F8, kind="ExternalInput")
    cosb = nc.dram_tensor("cosb", [128, T], BF16, kind="ExternalInput")
    sinb = nc.dram_tensor("sinb", [128, T], BF16, kind="ExternalInput")
    masks = nc.dram_tensor("masks", [128, 4, TB], BF16, kind="ExternalInput")
    rstd1 = nc.dram_tensor("rstd1", [1, N], F32, kind="ExternalInput")
    qnw = nc.dram_tensor("qnw", [128, 1], F32, kind="ExternalInput")
    knw = nc.dram_tensor("knw", [128, 1], F32, kind="ExternalInput")
    protb = nc.dram_tensor("protb", [128, 128], BF16, kind="ExternalInput")
    sel8 = nc.dram_tensor("sel8", [E, E * 128], BF16, kind="ExternalInput")

    y = nc.dram_tensor("y", [128, CK, NOWN], F16, kind="ExternalOutput")

    with _SplitDrainTileContext(nc) as tc:
        with (
            tc.tile_pool(name="const", bufs=1) as cpool,
            tc.tile_pool(name="dram", bufs=1, space="DRAM") as dram,
            tc.tile_pool(name="ps", bufs=1, space="PSUM") as ps,
            tc.tile_pool(name="big", bufs=1) as big,
            tc.tile_pool(name="work", bufs=3) as wk,
        ):
            ab = tc.alloc_tile_pool(name="ab", bufs=1)
            MM = dict(tag="mm", bufs=5)       # f32 [128,TB] psum
            ROW = dict(tag="row", bufs=2)     # f32 [<=8,TB] psum
            TRP = dict(tag="trp", bufs=1)     # bf16 [128,128] psum
            TBF = dict(tag="t512b", bufs=4)   # bf16 [128,TB] transients
            RWF = dict(tag="rowf", bufs=2)    # f32 [1,TB]
            BCS = dict(tag="bcs", bufs=3)     # f32 [128,TB] bcast results
            SM8 = dict(tag="sm8", bufs=2)     # f32 [8,NOWN] small routing tiles
            SM1 = dict(tag="sm1", bufs=3)     # f32 [1,NOWN]
            MC = dict(tag="mc", bufs=4)       # bf16 [128,NOWN] moe transients

            # ---- constants ----
            ident = cpool.tile([128, 128], BF16)
            make_identity(nc, ident)
            ones_bf = cpool.tile([128, 1], BF16)
            nc.vector.memset(ones_bf, 1.0)
            onesrow_f = cpool.tile([1, 128], F32)
            nc.vector.memset(onesrow_f, 1.0)
            onesrow_r = cpool.tile([1, 128], F32R)
            nc.vector.tensor_copy(out=onesrow_r, in_=onesrow_f)
            ones8_f = cpool.tile([1, E], F32)
            nc.vector.memset(ones8_f, 1.0)
            ones8_r = cpool.tile([1, E], F32R)
            nc.vector.tensor_copy(out=ones8_r, in_=ones8_f)
            bias_q = cpool.tile([1, 1], F32)
            nc.vector.memset(bias_q, float(D) * EPS)
            bias_eps = cpool.tile([1, 1], F32)
            nc.vector.memset(bias_eps, EPS)
            idf1 = cpool.tile([1, 1], F32)
            nc.vector.memset(idf1, 1.0)

            xtb0 = ab.tile([128, CK, TB], BF16, tag="xtb", bufs=1, name="xtb0")
            nc.sync.dma_start(out=xtb0[:, 0:CK // 2, :], in_=xT[:, 0:CK // 2, 0:TB])
            x0dma = nc.sync.dma_start(out=xtb0[:, CK // 2:CK, :],
                                      in_=xT[:, CK // 2:CK, 0:TB])
            qw_sb = ab.tile([128, HQ, CK, 128], BF16)
            for _hd in range(HQ):
                nc.sync.dma_start(out=qw_sb[:, _hd, :, :], in_=qw[:, _hd, :, :])
            kw_sb = ab.tile([128, CK, 128], BF16)
            nc.sync.dma_start(out=kw_sb, in_=kw[:])
            vw_sb = ab.tile([128, CK, 128], BF16)
            nc.sync.dma_start(out=vw_sb, in_=vw[:])
            prot_sb = cpool.tile([128, 128], BF16)
            nc.sync.dma_start(out=prot_sb, in_=protb[:])
            cos_sb = cpool.tile([128, T], BF16)
            nc.sync.dma_start(out=cos_sb, in_=cosb[:])
            sin_sb = cpool.tile([128, T], BF16)
            nc.sync.dma_start(out=sin_sb, in_=sinb[:])
            masks_sb = cpool.tile([128, 4, TB], BF16)
            qnw_sb = cpool.tile([128, 1], F32)
            nc.sync.dma_start(out=qnw_sb, in_=qnw[:])
            knw_sb = cpool.tile([128, 1], F32)
            lastconst_dma = nc.sync.dma_start(out=knw_sb, in_=knw[:])
            gatew_sb = cpool.tile([128, CK, E], F16)
            sel8_sb = cpool.tile([E, E * 128], BF16)

            ow_sb = ab.tile([128, CK, HQ, 128], BF16)

            qT_sb = ab.tile([128, HQ, T], BF16)            # per-batch
            kT_sb = ab.tile([128, T], BF16)
            vnat_sb = ab.tile([128, T // 128, 128], BF16)
            rk_sb = ab.tile([128, T // 128], F32)     # per-key-tile rstd_k
            rv_sb = ab.tile([128, T // 128], F32)     # per-key-tile rstd1
            xh_sb = big.tile([128, CK, NOWN], F16)          # own hidden (x+attn)
            xn8_sb = big.tile([128, CK, NOWN], F8)
            prod8_sb = big.tile([128, E, FK, NOWN], F8)
            comb_row = big.tile([E, NOWN], F32)             # routing weights

            def deferred_const_loads():
                nc.sync.dma_start(out=ow_sb, in_=ow[:])
                nc.sync.dma_start(out=masks_sb, in_=masks[:])
                nc.sync.dma_start(out=gatew_sb, in_=gatew[:])
                nc.sync.dma_start(out=sel8_sb, in_=sel8[:])

            rsin = [dram.tile([8, 128, CK, 128], F16, name=f"rsin{b}") for b in range(B)]
            rsout = [dram.tile([128, CK, 128], F16, name=f"rsout{b}") for b in range(B)]

            def bcast(row_f32_ap, width=TB, act_copy=False, **pool_kw):
                """[1,width] f32 -> SBUF [128,width] f32 via K=1 f32r matmul."""
                rr = wk.tile([1, TB], F32R, tag="rwr", bufs=1)
                nc.vector.tensor_copy(out=rr[:, 0:width], in_=row_f32_ap)
                bc_ps = ps.tile([128, TB], F32, **MM)
                nc.tensor.matmul(bc_ps[:, 0:width], onesrow_r,
                                 rr[:, 0:width], start=True, stop=True)
                kw_ = pool_kw or BCS
                bc = wk.tile([128, TB], F32, **kw_)
                if act_copy:
                    nc.scalar.copy(out=bc[:, 0:width], in_=bc_ps[:, 0:width])
                else:
                    nc.vector.tensor_copy(out=bc[:, 0:width], in_=bc_ps[:, 0:width])
                return bc[:, 0:width]

            def bcast8(row_f32_ap):
                """[1,NOWN] f32 -> SBUF [E,NOWN] f32, exact (fp32 matmul so
                is_equal against the source values still holds bitwise)."""
                p8 = ps.tile([8, TB], F32, **ROW)
                nc.tensor.matmul(p8[0:E, 0:NOWN], ones8_f, row_f32_ap,
                                 start=True, stop=True)
                t8 = wk.tile([E, NOWN], F32, **SM8)
                nc.vector.tensor_copy(out=t8, in_=p8[0:E, 0:NOWN])
                return t8

            def colsum_rstd(feat_ps, scale, bias_ap):
                """rsqrt(scale*colsum(feat^2)+bias) -> [1,TB] f32 row."""
                sq = wk.tile([128, TB], BF16, **TBF)
                nc.scalar.activation(out=sq, in_=feat_ps, func=AF.Square)
                ssum = ps.tile([8, TB], F32, **ROW)
                nc.tensor.matmul(ssum[0:1, :], ones_bf, sq, start=True, stop=True)
                srow = wk.tile([1, TB], F32, **RWF)
                nc.scalar.activation(out=srow, in_=ssum[0:1, :], func=AF.Sqrt,
                                     scale=scale, bias=bias_ap)
                rrow = wk.tile([1, TB], F32, **RWF)
                nc.vector.reciprocal(out=rrow, in_=srow)
                return rrow

            def rope_plain(feat_sb, j, w_sb, out_ap):
                """out = rope(w * feat); rstd applied separately (rope is
                linear per token, so the per-token scale commutes)."""
                tcol = j * TB
                qhat = wk.tile([128, TB], BF16, **TBF)
                nc.vector.tensor_scalar(out=qhat, in0=feat_sb, scalar1=w_sb,
                                        scalar2=None, op0=OP.mult)
                rot_ps = ps.tile([128, TB], F32, **MM)
                nc.tensor.matmul(rot_ps, prot_sb, qhat, start=True, stop=True)
                qc = wk.tile([128, TB], BF16, **TBF)
                nc.vector.tensor_tensor(
                    out=qc, in0=qhat, in1=cos_sb[:, tcol:tcol + TB], op=OP.mult,
                )
                rsm = wk.tile([128, TB], BF16, **TBF)
                nc.vector.tensor_tensor(
                    out=rsm, in0=rot_ps, in1=sin_sb[:, tcol:tcol + TB], op=OP.mult,
                )
                nc.vector.tensor_tensor(out=out_ap, in0=qc, in1=rsm, op=OP.add)

            # =========================== Phase A: QKV ======================
            def phaseA(g):
                b, j = divmod(g, 2)
                n0 = g * TB
                if g == 0:
                    xtb = xtb0
                else:
                    xtb = ab.tile([128, CK, TB], BF16, tag="xtb", bufs=1)
                    nc.sync.dma_start(out=xtb[:, 0:CK // 2, :],
                                      in_=xT[:, 0:CK // 2, n0:n0 + TB])
                    nc.sync.dma_start(out=xtb[:, CK // 2:CK, :],
                                      in_=xT[:, CK // 2:CK, n0:n0 + TB])
                r1row = wk.tile([1, TB], F32, **RWF)
                nc.sync.dma_start(out=r1row, in_=rstd1[:, n0:n0 + TB])
                for cc in range(TB // 128):
                    r1t = ps.tile([128, TB], F32, **MM)
                    nc.tensor.transpose(r1t[:, 0:1],
                                        r1row[:, cc * 128:(cc + 1) * 128], idf1)
                    nc.vector.tensor_copy(
                        out=rv_sb[:, 4 * j + cc:4 * j + cc + 1],
                        in_=r1t[:, 0:1])

                def accum(kind, hd):
                    p = ps.tile([128, TB], F32, **MM)
                    for kc in range(CK):
                        if kind == 'q':
                            lhs = qw_sb[:, hd, kc, :]
                        elif kind == 'k':
                            lhs = kw_sb[:, kc, :]
                        else:
                            lhs = vw_sb[:, kc, :]
                        nc.tensor.matmul(p, lhs, xtb[:, kc, :],
                                         start=(kc == 0), stop=(kc == CK - 1))
                    return p

                def stage2(kind, p):
                    # free the psum early: rope reads the bf16 SBUF copy
                    if kind == 'q':
                        row = colsum_rstd(p, 1.0, bias_q)
                    elif kind == 'k':
                        row = colsum_rstd(p, 1.0 / D, bias_eps)
                    else:
                        row = None
                    fsb = wk.tile([128, TB], BF16, tag="fsb", bufs=2)
                    nc.scalar.copy(out=fsb, in_=p)
                    return (row, fsb)

                def stage3(kind, hd, st2):
                    row, fsb = st2
                    if kind == 'q':
                        qbc = bcast(row[:], act_copy=True)
                        qr = wk.tile([128, TB], BF16, **TBF)
                        rope_plain(fsb, j, qnw_sb, qr)
                        nc.vector.tensor_tensor(
                            out=qT_sb[:, hd, j * TB:j * TB + TB],
                            in0=qr, in1=qbc, op=OP.mult)
                    elif kind == 'k':
                        rope_plain(fsb, j, knw_sb,
                                   kT_sb[:, j * TB:j * TB + TB])
                        for cc in range(TB // 128):
                            rkt = ps.tile([128, TB], F32, **MM)
                            nc.tensor.transpose(
                                rkt[:, 0:1], row[:, cc * 128:(cc + 1) * 128],
                                idf1)
                            nc.vector.tensor_copy(
                                out=rk_sb[:, 4 * j + cc:4 * j + cc + 1],
                                in_=rkt[:, 0:1])
                    else:
                        # x's rstd applied post-transpose, where token is the
                        # partition dim (per-partition scalar, no broadcast)
                        for cc in range(TB // 128):
                            vtr = ps.tile([128, 128], BF16, **TRP)
                            nc.tensor.transpose(
                                vtr, fsb[:, cc * 128:(cc + 1) * 128], ident)
                            nc.vector.tensor_scalar(
                                out=vnat_sb[:, j * 4 + cc, :], in0=vtr,
                                scalar1=rv_sb[:, 4 * j + cc:4 * j + cc + 1],
                                scalar2=None, op0=OP.mult)

                seq = [('q', 0), ('q', 1), ('q', 2), ('q', 3),
                       ('k', None), ('v', None)]
                st = []
                for idx, (kind, hd) in enumerate(seq):
                    p = accum(kind, hd)
                    st.append([kind, hd, p, None])
                    if idx >= 1:
                        st[idx - 1][3] = stage2(st[idx - 1][0], st[idx - 1][2])
                    if idx >= 2:
                        stage3(st[idx - 2][0], st[idx - 2][1], st[idx - 2][3])
                st[-1][3] = stage2(st[-1][0], st[-1][2])
                stage3(st[-2][0], st[-2][1], st[-2][3])
                stage3(st[-1][0], st[-1][1], st[-1][3])

            # ====================== Phase B: attention =====================
            def phaseB(g):
                b, j = divmod(g, 2)
                q0 = j * TB
                ntk = 4 * j + 4
                avT = ab.tile([128, HQ, TB], BF16, tag="avT", bufs=1)

                def fin(hd, av_sb, den_ps):
                    dsb = wk.tile([1, TB], F32, **RWF)
                    nc.vector.tensor_copy(out=dsb, in_=den_ps[0:1, :])
                    rec = wk.tile([1, TB], F32, **RWF)
                    nc.vector.reciprocal(out=rec, in_=dsb)
                    rbc = bcast(rec[:])
                    nc.vector.tensor_tensor(out=avT[:, hd, :], in0=av_sb,
                                            in1=rbc, op=OP.mult)

                pend = None
                for hd in range(HQ):
                    av_ps = ps.tile([128, TB], F32, **MM)
                    den_ps = ps.tile([8, TB], F32, **ROW)
                    for i in range(ntk):
                        tk0 = i * 128
                        s = i - 4 * j
                        # diagonal tiles: columns < 128*s are fully masked, so
                        # restrict score/exp/den/av to the live column range
                        c0 = max(s, 0) * 128
                        w = TB - c0
                        sc_ps = ps.tile([128, TB], F32, **MM)
                        nc.tensor.matmul(sc_ps[:, c0:TB], kT_sb[:, tk0:tk0 + 128],
                                         qT_sb[:, hd, q0 + c0:q0 + TB],
                                         start=True, stop=True,
                                         skip_group_check=True)
                        ex = wk.tile([128, TB], BF16, **TBF)
                        if s < 0:
                            nc.scalar.activation(out=ex, in_=sc_ps, func=AF.Exp,
                                                 scale=rk_sb[:, i:i + 1])
                        else:
                            ext = wk.tile([128, TB], BF16, **TBF)
                            nc.scalar.activation(out=ext[:, c0:TB],
                                                 in_=sc_ps[:, c0:TB], func=AF.Exp,
                                                 scale=rk_sb[:, i:i + 1])
                            nc.vector.tensor_tensor(out=ex[:, c0:TB],
                                                    in0=ext[:, c0:TB],
                                                    in1=masks_sb[:, s, c0:TB],
                                                    op=OP.mult)
                        nc.tensor.matmul(den_ps[0:1, c0:TB], ones_bf,
                                         ex[:, c0:TB],
                                         start=(i == 0), stop=(i == ntk - 1),
                                         skip_group_check=True)
                        nc.tensor.matmul(av_ps[:, c0:TB], vnat_sb[:, i, :],
                                         ex[:, c0:TB],
                                         start=(i == 0), stop=(i == ntk - 1),
                                         skip_group_check=True)
                    # free the av psum early via an Act copy; fin reads SBUF
                    av_sb = wk.tile([128, TB], F32, **BCS)
                    nc.scalar.copy(out=av_sb, in_=av_ps)
                    if pend is not None:
                        fin(*pend)
                    pend = (hd, av_sb, den_ps)
                fin(*pend)
                for mq in range(4):
                    attq = ab.tile([128, 4, TB], F16, tag="attb", bufs=2)
                    for mi in range(4):
                        m = 4 * mq + mi
                        att_ps = ps.tile([128, TB], F32, **MM)
                        for hk in range(HQ):
                            nc.tensor.matmul(att_ps, ow_sb[:, m, hk, :],
                                             avT[:, hk, :], start=(hk == 0),
                                             stop=(hk == HQ - 1))
                        nc.scalar.copy(out=attq[:, mi, :], in_=att_ps)
                    for tg in range(4):
                        nc.sync.dma_start(
                            out=rsin[b][4 * j + tg, :, 4 * mq:4 * mq + 4, :],
                            in_=attq[:, :, tg * 128:(tg + 1) * 128])
                if j == 1:
                    nc.gpsimd.collective_compute(
                        "ReduceScatter", OP.add,
                        replica_groups=[list(range(N_CORES))],
                        ins=[rsin[b].opt()], outs=[rsout[b].opt()],
                    )

            # ========================= Phase C: MoE ========================
            HGU = FK * 8 * 2 * 128    # flat size of one gate (or up) half

            def load_guw_half(e, half, eng=None, after=None):
                t = wk.tile([128, FK * 8, 2, 128], F8, tag="wgu", bufs=4)
                d = (eng or nc.gpsimd).dma_start(
                    out=t,
                    in_=guw[:, e, half * HGU:(half + 1) * HGU].rearrange(
                        "p (a b c) -> p a b c", b=2, c=128))
                if after is not None:
                    _add_dep_helper(d.ins, after.ins, sync=True,
                                    reason="prefetch after startup loads")
                return t

            def load_dww(m, eng=None, after=None):
                t = wk.tile([128, E * 3, 2, 128], F8, tag="wdw", bufs=2)
                d = (eng or nc.sync).dma_start(out=t, in_=dww[:, m, :].rearrange(
                    "p (a b c) -> p a b c", b=2, c=128))
                if after is not None:
                    _add_dep_helper(d.ins, after.ins, sync=True,
                                    reason="prefetch after startup loads")
                return t

            def phaseC(pre_gu, pre_dw, moe):
                # assemble own hidden = attn partial sums (+x) for owned tokens
                for b in range(B):
                    nc.sync.dma_start(out=xh_sb[:, :, 128 * b:128 * b + 128],
                                      in_=rsout[b][:])
                xow = moe.tile([128, CK, NOWN], F16, name="xow_all")
                nc.sync.dma_start(out=xow, in_=xown[:])
                for fc in range(CK):
                    nc.vector.tensor_tensor(out=xh_sb[:, fc, :],
                                            in0=xh_sb[:, fc, :],
                                            in1=xow[:, fc, :], op=OP.add)
                lg_ps = ps.tile([8, TB], F32, **ROW)
                den2_ps = ps.tile([8, TB], F32, **ROW)
                for fc in range(CK):
                    nc.tensor.matmul(lg_ps[0:E, 0:NOWN], gatew_sb[:, fc, :],
                                     xh_sb[:, fc, :],
                                     start=(fc == 0), stop=(fc == CK - 1))
                    sq = wk.tile([128, NOWN], BF16, **MC)
                    nc.scalar.activation(out=sq, in_=xh_sb[:, fc, :], func=AF.Square)
                    nc.tensor.matmul(den2_ps[0:1, 0:NOWN], ones_bf, sq,
                                     start=(fc == 0), stop=(fc == CK - 1))
                s2 = wk.tile([1, NOWN], F32, **SM1)
                nc.scalar.activation(out=s2, in_=den2_ps[0:1, 0:NOWN], func=AF.Sqrt,
                                     scale=1.0 / C, bias=bias_eps)
                rstd2 = wk.tile([1, NOWN], F32, tag="rstd2", bufs=1)
                nc.vector.reciprocal(out=rstd2, in_=s2)
                r2bc = bcast(rstd2[:], width=NOWN)
                for fc in range(CK):
                    nc.vector.tensor_tensor(out=xn8_sb[:, fc, :],
                                            in0=xh_sb[:, fc, :],
                                            in1=r2bc, op=OP.mult)

                # ---- top-2 routing on [E, NOWN] ----
                lg = wk.tile([E, NOWN], F32, tag="lg", bufs=1)
                nc.vector.tensor_copy(out=lg, in_=lg_ps[0:E, 0:NOWN])
                m1 = wk.tile([1, NOWN], F32, **SM1)
                nc.gpsimd.tensor_reduce(out=m1, in_=lg, axis=mybir.AxisListType.C,
                                        op=OP.max)
                m1bc = bcast8(m1[:])
                eq1 = wk.tile([E, NOWN], F32, tag="eq1", bufs=1)
                nc.vector.tensor_tensor(out=eq1, in0=lg, in1=m1bc, op=OP.is_equal)
                lg2 = wk.tile([E, NOWN], F32, **SM8)
                nc.vector.scalar_tensor_tensor(out=lg2, in0=eq1, scalar=-BIG,
                                               in1=lg, op0=OP.mult, op1=OP.add)
                m2 = wk.tile([1, NOWN], F32, **SM1)
                nc.gpsimd.tensor_reduce(out=m2, in_=lg2, axis=mybir.AxisListType.C,
                                        op=OP.max)
                m2bc = bcast8(m2[:])
                eq2 = wk.tile([E, NOWN], F32, **SM8)
                nc.vector.tensor_tensor(out=eq2, in0=lg, in1=m2bc, op=OP.is_equal)
                # dlt = (m1-m2)*rstd2 ; w1 = sigmoid(dlt); w2 = 1-w1
                dlt = wk.tile([1, NOWN], F32, **SM1)
                nc.vector.tensor_tensor(out=dlt, in0=m1, in1=m2, op=OP.subtract)
                dlts = wk.tile([1, NOWN], F32, **SM1)
                nc.vector.tensor_tensor(out=dlts, in0=dlt, in1=rstd2, op=OP.mult)
                w1 = wk.tile([1, NOWN], F32, **SM1)
                nc.scalar.activation(out=w1, in_=dlts, func=AF.Sigmoid)
                w1bc = bcast8(w1[:])
                # comb = eq1*w1 + eq2*(1-w1) = (eq1-eq2)*w1 + eq2, in place
                nc.vector.tensor_tensor(out=eq1, in0=eq1, in1=eq2, op=OP.subtract)
                nc.vector.tensor_tensor(out=eq1, in0=eq1, in1=w1bc, op=OP.mult)
                nc.vector.tensor_tensor(out=eq1, in0=eq1, in1=eq2, op=OP.add)
                # scale by PSC/WS (prod fp8 scale / up-weight descale)
                nc.vector.tensor_scalar(out=comb_row, in0=eq1, scalar1=PSC / WS,
                                        scalar2=None, op0=OP.mult)

                # ---- pass 1: gate/up + silu -> prod8 per expert ----
                def load_guw_moe(e, half, eng):
                    t = moe.tile([128, FK * 8, 2, 128], F8, tag="wgu2", bufs=4)
                    eng.dma_start(
                        out=t,
                        in_=guw[:, e, half * HGU:(half + 1) * HGU].rearrange(
                            "p (a b c) -> p a b c", b=2, c=128))
                    return t

                def load_dww_moe(m, eng):
                    t = moe.tile([128, E * 3, 2, 128], F8, tag="wdw2", bufs=4)
                    eng.dma_start(out=t, in_=dww[:, m, :].rearrange(
                        "p (a b c) -> p a b c", b=2, c=128))
                    return t

                comb_bf = wk.tile([E, NOWN], BF16, tag="combbf", bufs=1)
                nc.vector.tensor_copy(out=comb_bf, in_=comb_row)
                for e in range(E):
                    wgg = pre_gu.pop((e, 0), None) or load_guw_moe(e, 0, nc.sync)
                    wgu = pre_gu.pop((e, 1), None) or load_guw_moe(e, 1, nc.scalar)
                    cb_ps = ps.tile([128, TB], F32, **MM)
                    nc.tensor.matmul(cb_ps[:, 0:NOWN],
                                     sel8_sb[:, e * 128:(e + 1) * 128],
                                     comb_bf, start=True, stop=True)
                    cbc = wk.tile([128, NOWN], F32, tag="cbc", bufs=1)
                    nc.vector.tensor_copy(out=cbc, in_=cb_ps[:, 0:NOWN])
                    for f in range(FK):
                        g_ps = ps.tile([128, TB], F32, **MM)
                        for kp in range(8):
                            nc.tensor.matmul(
                                g_ps[:, 0:NOWN], wgg[:, f * 8 + kp, :, :],
                                xn8_sb[:, 2 * kp:2 * kp + 2, :],
                                start=(kp == 0), stop=(kp == 7),
                                perf_mode=PM.DoubleRow,
                            )
                        u_ps = ps.tile([128, TB], F32, **MM)
                        for kp in range(8):
                            nc.tensor.matmul(
                                u_ps[:, 0:NOWN], wgu[:, f * 8 + kp, :, :],
                                xn8_sb[:, 2 * kp:2 * kp + 2, :],
                                start=(kp == 0), stop=(kp == 7),
                                perf_mode=PM.DoubleRow,
                            )
                        sil = wk.tile([128, NOWN], BF16, **MC)
                        nc.scalar.activation(out=sil, in_=g_ps[:, 0:NOWN],
                                             func=AF.Silu, scale=1.0 / WS)
                        ucm = wk.tile([128, NOWN], BF16, **MC)
                        nc.vector.tensor_tensor(out=ucm, in0=u_ps[:, 0:NOWN],
                                                in1=cbc, op=OP.mult)
                        nc.vector.tensor_tensor(out=prod8_sb[:, e, f, :],
                                                in0=sil, in1=ucm, op=OP.mult)

                # ---- pass 2: down proj, accumulate experts in psum ----
                for m in range(CK):
                    wd = pre_dw.pop(m, None) or load_dww_moe(m, nc.sync)
                    eo_ps = ps.tile([128, TB], F32, **MM)
                    for e in range(E):
                        for kp in range(3):
                            nc.tensor.matmul(
                                eo_ps[:, 0:NOWN], wd[:, e * 3 + kp, :, :],
                                prod8_sb[:, e, 2 * kp:2 * kp + 2, :],
                                start=(e == 0 and kp == 0),
                                stop=(e == E - 1 and kp == 2),
                                perf_mode=PM.DoubleRow,
                            )
                    ym = wk.tile([128, NOWN], F16, tag="ymc", bufs=2)
                    nc.vector.scalar_tensor_tensor(
                        out=ym, in0=eo_ps[:, 0:NOWN],
                        scalar=1.0 / (WS * PSC), in1=xh_sb[:, m, :],
                        op0=OP.mult, op1=OP.add,
                    )
                    nc.sync.dma_start(out=y[:, m, :], in_=ym)

            pre_gu, pre_dw = {}, {}
            for g in range(NBLK):
                if 'A' in phases:
                    phaseA(g)
                if g == 0:
                    deferred_const_loads()
                    if 'C' in phases:
                        # act-queue prefetches: the Act sequencer reaches these
                        # only after A0's first Square, keeping the DMA engines
                        # free for the critical startup loads
                        for e in range(2):
                            for half in range(2):
                                pre_gu[(e, half)] = load_guw_half(
                                    e, half, nc.scalar, after=lastconst_dma)
                        pre_dw[0] = load_dww(0, nc.scalar, after=lastconst_dma)
                        pre_dw[1] = load_dww(1, nc.scalar, after=lastconst_dma)
                if 'B' in phases:
                    phaseB(g)
            ab.release()
            if 'C' in phases:
                with tc.tile_pool(name="moe", bufs=1) as moe:
                    phaseC(pre_gu, pre_dw, moe)

    _split_multi_waits(nc)
    return nc


# ---------------------------------------------------------------------------

_NC_CACHE = {}


def _get_nc():
    if "nc" not in _NC_CACHE:
        _NC_CACHE["nc"] = build_nc()
    return _NC_CACHE["nc"]


def _chunk_pm(a, nchunk):
    """[nchunk*128, free...] -> [128, nchunk, free...]"""
    return np.ascontiguousarray(
        a.reshape(nchunk, 128, *a.shape[1:]).transpose(1, 0, *range(2, a.ndim + 1))
    )


def prepare_in_maps(x, cos, sin, ln1_w, q_w, k_w, v_w, o_w, qn_w, kn_w, ln2_w,
                    gate_w, gate_up_w, down_w):
    bf = ml_dtypes.bfloat16
    f8 = ml_dtypes.float8_e4m3
    x = np.asarray(x, dtype=np.float32)
    x_flat = x.reshape(N, C)

    xT = _chunk_pm(np.ascontiguousarray(x_flat.T).astype(bf), CK)
    rstd1 = (1.0 / np.sqrt((x_flat.astype(np.float64) ** 2).mean(axis=1) + EPS)
             ).astype(np.float32)[None, :]

    ln1 = np.asarray(ln1_w, dtype=np.float32)[:, None]
    ln2 = np.asarray(ln2_w, dtype=np.float32)[:, None]
    qwf = np.asarray(q_w, dtype=np.float32) * ln1
    kwf = np.asarray(k_w, dtype=np.float32) * ln1
    vwf = np.asarray(v_w, dtype=np.float32) * ln1
    gatewf = np.asarray(gate_w, dtype=np.float32) * ln2
    guwf = np.asarray(gate_up_w, dtype=np.float32) * ln2[None]    # [E, C, 2F]
    dwf = np.asarray(down_w, dtype=np.float32)                    # [E, F, C]
    owf = np.asarray(o_w, dtype=np.float32)

    cos0 = np.asarray(cos, dtype=np.float32)[0]
    sin0 = np.asarray(sin, dtype=np.float32)[0]
    cosT = np.ascontiguousarray(cos0.T).astype(bf)
    sinT = np.ascontiguousarray(sin0.T).astype(bf)
    protm = np.zeros((128, 128), dtype=np.float32)
    for m in range(64):
        protm[m + 64, m] = -1.0
    for m in range(64, 128):
        protm[m - 64, m] = 1.0

    r = np.arange(128)[:, None]
    col = np.arange(TB)[None, :]
    masks = np.stack(
        [(col >= r + 128 * s).astype(bf) for s in range(4)], axis=1
    )

    # fp8 MoE weights, shared across cores
    # guw host layout: [128, E, FGU*8*2*128]; lhsT slice [128, 2, 128] is
    # (grp, kp) with pair index i selecting k-chunk 2kp+i.
    gu6 = (guwf * WS).astype(f8)                       # [E, C, 2F]
    gu_r = gu6.reshape(E, 8, 2, 128, FGU, 128)         # e, kp, i, p, grp, d
    guw_h = np.ascontiguousarray(
        gu_r.transpose(3, 0, 4, 1, 2, 5).reshape(128, E, FGU * 8 * 2 * 128))
    # dww host layout: [128, CK(m), E*3*2*128]; lhsT slice (e, kp) pair i
    # selects f-chunk 2kp+i; partition p = f % 128; d = c within group m.
    dw6 = (dwf * WS).astype(f8)                        # [E, F, C]
    dw_r = dw6.reshape(E, 3, 2, 128, CK, 128)          # e, kp, i, p, m, d
    dww_h = np.ascontiguousarray(
        dw_r.transpose(3, 4, 0, 1, 2, 5).reshape(128, CK, E * 3 * 2 * 128))

    gatew_h = _chunk_pm(gatewf.astype(np.float16), CK)

    in_maps = []
    for c in range(N_CORES):
        oslice = owf[512 * c:512 * (c + 1), :].astype(bf)  # [512, C]
        o4 = oslice.reshape(HQ, 128, CK, 128)              # hk, p, m, d
        ow_h = np.ascontiguousarray(o4.transpose(1, 2, 0, 3))
        # owned tokens: batch b local [128c, 128c+128)
        own_idx = np.concatenate([
            np.arange(b * T + 128 * c, b * T + 128 * (c + 1)) for b in range(B)
        ])
        xo = x_flat[own_idx, :].T                          # [C, 256]
        xown_h = _chunk_pm(np.ascontiguousarray(xo).astype(np.float16), CK)
        in_maps.append({
            "xT": xT,
            "xown": xown_h,
            "qw": np.ascontiguousarray(
                qwf[:, 512 * c:512 * (c + 1)].astype(bf)
                .reshape(CK, 128, HQ, 128).transpose(1, 2, 0, 3)),
            "kw": _chunk_pm(kwf[:, 128 * c:128 * (c + 1)].astype(bf), CK),
            "vw": _chunk_pm(vwf[:, 128 * c:128 * (c + 1)].astype(bf), CK),
            "ow": ow_h,
            "gatew": gatew_h,
            "guw": guw_h,
            "dww": dww_h,
            "cosb": cosT,
            "sinb": sinT,
            "masks": masks,
            "rstd1": rstd1,
            "qnw": np.asarray(qn_w, dtype=np.float32)[:, None],
            "knw": np.asarray(kn_w, dtype=np.float32)[:, None],
            "protb": protm.astype(bf),
            "sel8": np.kron(np.eye(E, dtype=np.float32),
                            np.ones((1, 128), dtype=np.float32)).astype(bf),
        })

    return in_maps


def combine(ys):
    out = np.zeros((N, C), dtype=np.float32)
    for c in range(N_CORES):
        yc = np.asarray(ys[c], dtype=np.float32)     # [128, CK, 256]
        # yc[p, fc, 128*b + i] -> token b*T + 128*c + i, feature fc*128+p
        feat_major = yc.transpose(1, 0, 2).reshape(C, NOWN)
        for b in range(B):
            toks = slice(b * T + 128 * c, b * T + 128 * (c + 1))
            out[toks, :] = feat_major[:, 128 * b:128 * (b + 1)].T
    return out.reshape(B, T, C)


def kernel(**inputs):
    in_maps = prepare_in_maps(**inputs)
    nc = _get_nc()
    res = run_bass_kernel_spmd(nc, in_maps, core_ids=list(range(N_CORES)))
    return combine([res.results[c]["y"] for c in range(N_CORES)])


# revision 9
# speedup vs baseline: 1.1420x; 1.0011x over previous
"""Fused attention+MoE block on 8 trn2 NeuronCores, v2.

Sharding: tensor-parallel attention (4 q-heads + 1 KV-head per core) as
before, but the attention output partials are ReduceScattered (2 per-batch
collectives) so each core ends up owning 256 tokens of the full hidden
state in feature-major layout. The MoE is then data-parallel: every core
runs all 8 experts densely on its own 256 tokens with fp8e4 DoubleRow
matmuls (2 k-chunks per instruction) and fp8 weights streamed from HBM.
Routing (top-2 over E=8) is computed on an [E, 256] tile with
partition-dim max reductions. Output y = hidden + moe for the owned
tokens; the host just concatenates core slices.
"""

import numpy as np
import ml_dtypes

import concourse.bass as bass
from concourse.bass import _add_dep_helper
import concourse.mybir as mybir
import concourse.tile as tile
from concourse.bass_utils import run_bass_kernel_spmd
from concourse.masks import make_identity
from concourse.vector_clock import ScopedClock

F32 = mybir.dt.float32
F32R = mybir.dt.float32r
F16 = mybir.dt.float16
BF16 = mybir.dt.bfloat16
F8 = mybir.dt.float8e4
U32 = mybir.dt.uint32
AF = mybir.ActivationFunctionType
OP = mybir.AluOpType
PM = mybir.MatmulPerfMode

B, T, C = 2, 1024, 2048
H, KV, D = 32, 8, 128
E, F, TOPK = 8, 768, 2
N = B * T
CK = C // 128          # 16
NBLK = 4               # token blocks of 512
TB = 512
HQ = H // 8            # 4 q heads per core
FK = F // 128          # 6
FGU = 2 * F // 128     # 12
NOWN = 256             # tokens owned per core (128 per batch)
WS = 64.0              # fp8 weight scale
PSC = 16.0             # fp8 prod scale
EPS = 1e-6
N_CORES = 8
BIG = 1e9

# ---------------------------------------------------------------------------
# walrus here rejects >1 sync-wait per instruction; split extras onto NoOps.


class _SplitDrainTileContext(tile.TileContext):
    def _drain_and_barrier(self, tick_clock, wait_clock):
        drain_inst = self.nc.sync.drain()
        wait_clock.add_sem_waits(
            drain_inst.ins, ScopedClock({None: tick_clock.global_clock})
        )
        si = drain_inst.ins.sync_info
        if si is not None and len(si.on_wait) > 1:
            ow = list(si.on_wait)
            drain_inst.ins.sync_info = mybir.SyncInfo(
                on_wait=ow[:1], on_update=list(si.on_update)
            )
            rest = ow[1:]
            while rest:
                extra = self.nc.sync.drain()
                extra.ins.sync_info = mybir.SyncInfo(on_wait=rest[:1], on_update=[])
                rest = rest[1:]
        self.nc.all_engine_barrier()
        assert self.sems is not None
        popped = self.nc._tile_sem_poison_stack.pop()
        assert popped is self._sem_poison
        self.nc.clear_and_free_semaphores(list(self.sems.allocated().values()))
        self.nc.all_engine_barrier()


def _split_multi_waits(nc):
    for bb in nc.main_func.blocks:
        insts = list(bb.instructions)
        out = []
        changed = False
        for ins in insts:
            si = ins.sync_info
            if si is not None and len(si.on_wait) > 1:
                ow = list(si.on_wait)
                for w in ow[:-1]:
                    nop = mybir.InstNoOp(name=f"waitnop-{nc.next_id()}", ins=[], outs=[])
                    nop.engine = ins.engine
                    nop.sync_info = mybir.SyncInfo(on_wait=[w], on_update=[])
                    out.append(nop)
                ins.sync_info = mybir.SyncInfo(
                    on_wait=[ow[-1]], on_update=list(si.on_update)
                )
                changed = True
            out.append(ins)
        if changed:
            bb.instructions = out


# ---------------------------------------------------------------------------


def build_nc(phases='ABC'):
    nc = bass.Bass("TRN2", target_bir_lowering=False, debug=False, num_devices=N_CORES)

    xT = nc.dram_tensor("xT", [128, CK, N], BF16, kind="ExternalInput")
    xown = nc.dram_tensor("xown", [128, CK, NOWN], F16, kind="ExternalInput")
    qw = nc.dram_tensor("qw", [128, HQ, CK, 128], BF16, kind="ExternalInput")
    kw = nc.dram_tensor("kw", [128, CK, 128], BF16, kind="ExternalInput")
    vw = nc.dram_tensor("vw", [128, CK, 128], BF16, kind="ExternalInput")
    ow = nc.dram_tensor("ow", [128, CK, HQ, 128], BF16, kind="ExternalInput")
    gatew = nc.dram_tensor("gatew", [128, CK, E], F16, kind="ExternalInput")
    guw = nc.dram_tensor("guw", [128, E, FGU * 8 * 2 * 128], F8, kind="ExternalInput")
    dww = nc.dram_tensor("dww", [128, CK, E * 3 * 2 * 128], F8, kind="ExternalInput")
    cosb = nc.dram_tensor("cosb", [128, T], BF16, kind="ExternalInput")
    sinb = nc.dram_tensor("sinb", [128, T], BF16, kind="ExternalInput")
    masks = nc.dram_tensor("masks", [128, 4, TB], BF16, kind="ExternalInput")
    rstd1 = nc.dram_tensor("rstd1", [1, N], F32, kind="ExternalInput")
    qnw = nc.dram_tensor("qnw", [128, 1], F32, kind="ExternalInput")
    knw = nc.dram_tensor("knw", [128, 1], F32, kind="ExternalInput")
    protb = nc.dram_tensor("protb", [128, 128], BF16, kind="ExternalInput")
    sel8 = nc.dram_tensor("sel8", [E, E * 128], BF16, kind="ExternalInput")

    y = nc.dram_tensor("y", [128, CK, NOWN], F16, kind="ExternalOutput")

    with _SplitDrainTileContext(nc) as tc:
        with (
            tc.tile_pool(name="const", bufs=1) as cpool,
            tc.tile_pool(name="dram", bufs=1, space="DRAM") as dram,
            tc.tile_pool(name="ps", bufs=1, space="PSUM") as ps,
            tc.tile_pool(name="big", bufs=1) as big,
            tc.tile_pool(name="work", bufs=3) as wk,
        ):
            ab = tc.alloc_tile_pool(name="ab", bufs=1)
            MM = dict(tag="mm", bufs=5)       # f32 [128,TB] psum
            ROW = dict(tag="row", bufs=2)     # f32 [<=8,TB] psum
            TRP = dict(tag="trp", bufs=1)     # bf16 [128,128] psum
            TBF = dict(tag="t512b", bufs=5)   # bf16 [128,TB] transients
            RWF = dict(tag="rowf", bufs=2)    # f32 [1,TB]
            BCS = dict(tag="bcs", bufs=3)     # f32 [128,TB] bcast results
            SM8 = dict(tag="sm8", bufs=2)     # f32 [8,NOWN] small routing tiles
            SM1 = dict(tag="sm1", bufs=3)     # f32 [1,NOWN]
            MC = dict(tag="mc", bufs=4)       # bf16 [128,NOWN] moe transients

            # ---- constants ----
            ident = cpool.tile([128, 128], BF16)
            make_identity(nc, ident)
            ones_bf = cpool.tile([128, 1], BF16)
            nc.vector.memset(ones_bf, 1.0)
            onesrow_f = cpool.tile([1, 128], F32)
            nc.vector.memset(onesrow_f, 1.0)
            onesrow_r = cpool.tile([1, 128], F32R)
            nc.vector.tensor_copy(out=onesrow_r, in_=onesrow_f)
            ones8_f = cpool.tile([1, E], F32)
            nc.vector.memset(ones8_f, 1.0)
            ones8_r = cpool.tile([1, E], F32R)
            nc.vector.tensor_copy(out=ones8_r, in_=ones8_f)
            bias_q = cpool.tile([1, 1], F32)
            nc.vector.memset(bias_q, float(D) * EPS)
            bias_eps = cpool.tile([1, 1], F32)
            nc.vector.memset(bias_eps, EPS)
            idf1 = cpool.tile([1, 1], F32)
            nc.vector.memset(idf1, 1.0)

            xtb0 = ab.tile([128, CK, TB], BF16, tag="xtb", bufs=1, name="xtb0")
            nc.sync.dma_start(out=xtb0[:, 0:CK // 2, :], in_=xT[:, 0:CK // 2, 0:TB])
            x0dma = nc.sync.dma_start(out=xtb0[:, CK // 2:CK, :],
                                      in_=xT[:, CK // 2:CK, 0:TB])
            qw_sb = ab.tile([128, HQ, CK, 128], BF16)
            for _hd in range(HQ):
                nc.sync.dma_start(out=qw_sb[:, _hd, :, :], in_=qw[:, _hd, :, :])
            kw_sb = ab.tile([128, CK, 128], BF16)
            nc.sync.dma_start(out=kw_sb, in_=kw[:])
            vw_sb = ab.tile([128, CK, 128], BF16)
            nc.sync.dma_start(out=vw_sb, in_=vw[:])
            prot_sb = cpool.tile([128, 128], BF16)
            nc.sync.dma_start(out=prot_sb, in_=protb[:])
            cos_sb = cpool.tile([128, T], BF16)
            nc.sync.dma_start(out=cos_sb, in_=cosb[:])
            sin_sb = cpool.tile([128, T], BF16)
            nc.sync.dma_start(out=sin_sb, in_=sinb[:])
            masks_sb = cpool.tile([128, 4, TB], BF16)
            qnw_sb = cpool.tile([128, 1], F32)
            nc.sync.dma_start(out=qnw_sb, in_=qnw[:])
            knw_sb = cpool.tile([128, 1], F32)
            lastconst_dma = nc.sync.dma_start(out=knw_sb, in_=knw[:])
            gatew_sb = cpool.tile([128, CK, E], F16)
            sel8_sb = cpool.tile([E, E * 128], BF16)

            ow_sb = ab.tile([128, CK, HQ, 128], BF16)

            qT_sb = ab.tile([128, HQ, T], BF16)            # per-batch
            kT_sb = ab.tile([128, T], BF16)
            vnat_sb = ab.tile([128, T // 128, 128], BF16)
            rk_sb = ab.tile([128, T // 128], F32)     # per-key-tile rstd_k
            rv_sb = ab.tile([128, T // 128], F32)     # per-key-tile rstd1
            xh_sb = big.tile([128, CK, NOWN], F16)          # own hidden (x+attn)
            xn8_sb = big.tile([128, CK, NOWN], F8)
            comb_row = big.tile([E, NOWN], F32)             # routing weights

            def deferred_const_loads():
                nc.sync.dma_start(out=ow_sb, in_=ow[:])
                nc.sync.dma_start(out=masks_sb, in_=masks[:])
                nc.sync.dma_start(out=gatew_sb, in_=gatew[:])
                nc.sync.dma_start(out=sel8_sb, in_=sel8[:])

            rsin = [dram.tile([8, 128, CK, 128], F16, name=f"rsin{b}") for b in range(B)]
            rsout = [dram.tile([128, CK, 128], F16, name=f"rsout{b}") for b in range(B)]

            def bcast(row_f32_ap, width=TB, act_copy=False, **pool_kw):
                """[1,width] f32 -> SBUF [128,width] f32 via K=1 f32r matmul."""
                rr = wk.tile([1, TB], F32R, tag="rwr", bufs=1)
                nc.vector.tensor_copy(out=rr[:, 0:width], in_=row_f32_ap)
                bc_ps = ps.tile([128, TB], F32, **MM)
                nc.tensor.matmul(bc_ps[:, 0:width], onesrow_r,
                                 rr[:, 0:width], start=True, stop=True)
                kw_ = pool_kw or BCS
                bc = wk.tile([128, TB], F32, **kw_)
                if act_copy:
                    nc.scalar.copy(out=bc[:, 0:width], in_=bc_ps[:, 0:width])
                else:
                    nc.vector.tensor_copy(out=bc[:, 0:width], in_=bc_ps[:, 0:width])
                return bc[:, 0:width]

            def bcast8(row_f32_ap):
                """[1,NOWN] f32 -> SBUF [E,NOWN] f32, exact (fp32 matmul so
                is_equal against the source values still holds bitwise)."""
                p8 = ps.tile([8, TB], F32, **ROW)
                nc.tensor.matmul(p8[0:E, 0:NOWN], ones8_f, row_f32_ap,
                                 start=True, stop=True)
                t8 = wk.tile([E, NOWN], F32, **SM8)
                nc.vector.tensor_copy(out=t8, in_=p8[0:E, 0:NOWN])
                return t8

            def colsum_rstd(feat_ps, scale, bias_ap):
                """rsqrt(scale*colsum(feat^2)+bias) -> [1,TB] f32 row."""
                sq = wk.tile([128, TB], BF16, **TBF)
                nc.scalar.activation(out=sq, in_=feat_ps, func=AF.Square)
                ssum = ps.tile([8, TB], F32, **ROW)
                nc.tensor.matmul(ssum[0:1, :], ones_bf, sq, start=True, stop=True)
                srow = wk.tile([1, TB], F32, **RWF)
                nc.scalar.activation(out=srow, in_=ssum[0:1, :], func=AF.Sqrt,
                                     scale=scale, bias=bias_ap)
                rrow = wk.tile([1, TB], F32, **RWF)
                nc.vector.reciprocal(out=rrow, in_=srow)
                return rrow

            def rope_plain(feat_sb, j, w_sb, out_ap):
                """out = rope(w * feat); rstd applied separately (rope is
                linear per token, so the per-token scale commutes)."""
                tcol = j * TB
                qhat = wk.tile([128, TB], BF16, **TBF)
                nc.vector.tensor_scalar(out=qhat, in0=feat_sb, scalar1=w_sb,
                                        scalar2=None, op0=OP.mult)
                rot_ps = ps.tile([128, TB], F32, **MM)
                nc.tensor.matmul(rot_ps, prot_sb, qhat, start=True, stop=True)
                qc = wk.tile([128, TB], BF16, **TBF)
                nc.vector.tensor_tensor(
                    out=qc, in0=qhat, in1=cos_sb[:, tcol:tcol + TB], op=OP.mult,
                )
                rsm = wk.tile([128, TB], BF16, **TBF)
                nc.vector.tensor_tensor(
                    out=rsm, in0=rot_ps, in1=sin_sb[:, tcol:tcol + TB], op=OP.mult,
                )
                nc.vector.tensor_tensor(out=out_ap, in0=qc, in1=rsm, op=OP.add)

            # =========================== Phase A: QKV ======================
            def phaseA(g):
                b, j = divmod(g, 2)
                n0 = g * TB
                if g == 0:
                    xtb = xtb0
                else:
                    xtb = ab.tile([128, CK, TB], BF16, tag="xtb", bufs=1)
                    nc.sync.dma_start(out=xtb[:, 0:CK // 2, :],
                                      in_=xT[:, 0:CK // 2, n0:n0 + TB])
                    nc.sync.dma_start(out=xtb[:, CK // 2:CK, :],
                                      in_=xT[:, CK // 2:CK, n0:n0 + TB])
                r1row = wk.tile([1, TB], F32, **RWF)
                nc.sync.dma_start(out=r1row, in_=rstd1[:, n0:n0 + TB])
                for cc in range(TB // 128):
                    r1t = ps.tile([128, TB], F32, **MM)
                    nc.tensor.transpose(r1t[:, 0:1],
                                        r1row[:, cc * 128:(cc + 1) * 128], idf1)
                    nc.vector.tensor_copy(
                        out=rv_sb[:, 4 * j + cc:4 * j + cc + 1],
                        in_=r1t[:, 0:1])

                def accum(kind, hd):
                    p = ps.tile([128, TB], F32, **MM)
                    for kc in range(CK):
                        if kind == 'q':
                            lhs = qw_sb[:, hd, kc, :]
                        elif kind == 'k':
                            lhs = kw_sb[:, kc, :]
                        else:
                            lhs = vw_sb[:, kc, :]
                        nc.tensor.matmul(p, lhs, xtb[:, kc, :],
                                         start=(kc == 0), stop=(kc == CK - 1))
                    return p

                def stage2(kind, p):
                    # free the psum early: rope reads the bf16 SBUF copy
                    if kind == 'q':
                        row = colsum_rstd(p, 1.0, bias_q)
                    elif kind == 'k':
                        row = colsum_rstd(p, 1.0 / D, bias_eps)
                    else:
                        row = None
                    fsb = wk.tile([128, TB], BF16, tag="fsb", bufs=2)
                    nc.scalar.copy(out=fsb, in_=p)
                    return (row, fsb)

                def stage3(kind, hd, st2):
                    row, fsb = st2
                    if kind == 'q':
                        qbc = bcast(row[:], act_copy=True)
                        qr = wk.tile([128, TB], BF16, **TBF)
                        rope_plain(fsb, j, qnw_sb, qr)
                        nc.vector.tensor_tensor(
                            out=qT_sb[:, hd, j * TB:j * TB + TB],
                            in0=qr, in1=qbc, op=OP.mult)
                    elif kind == 'k':
                        rope_plain(fsb, j, knw_sb,
                                   kT_sb[:, j * TB:j * TB + TB])
                        for cc in range(TB // 128):
                            rkt = ps.tile([128, TB], F32, **MM)
                            nc.tensor.transpose(
                                rkt[:, 0:1], row[:, cc * 128:(cc + 1) * 128],
                                idf1)
                            nc.vector.tensor_copy(
                                out=rk_sb[:, 4 * j + cc:4 * j + cc + 1],
                                in_=rkt[:, 0:1])
                    else:
                        # x's rstd applied post-transpose, where token is the
                        # partition dim (per-partition scalar, no broadcast)
                        for cc in range(TB // 128):
                            vtr = ps.tile([128, 128], BF16, **TRP)
                            nc.tensor.transpose(
                                vtr, fsb[:, cc * 128:(cc + 1) * 128], ident)
                            nc.vector.tensor_scalar(
                                out=vnat_sb[:, j * 4 + cc, :], in0=vtr,
                                scalar1=rv_sb[:, 4 * j + cc:4 * j + cc + 1],
                                scalar2=None, op0=OP.mult)

                seq = [('q', 0), ('q', 1), ('q', 2), ('q', 3),
                       ('k', None), ('v', None)]
                st = []
                for idx, (kind, hd) in enumerate(seq):
                    p = accum(kind, hd)
                    st.append([kind, hd, p, None])
                    if idx >= 1:
                        st[idx - 1][3] = stage2(st[idx - 1][0], st[idx - 1][2])
                    if idx >= 2:
                        stage3(st[idx - 2][0], st[idx - 2][1], st[idx - 2][3])
                st[-1][3] = stage2(st[-1][0], st[-1][2])
                stage3(st[-2][0], st[-2][1], st[-2][3])
                stage3(st[-1][0], st[-1][1], st[-1][3])

            # ====================== Phase B: attention =====================
            def phaseB(g):
                b, j = divmod(g, 2)
                q0 = j * TB
                ntk = 4 * j + 4
                avT = ab.tile([128, HQ, TB], BF16, tag="avT", bufs=1)

                def fin(hd, av_sb, den_ps):
                    dsb = wk.tile([1, TB], F32, **RWF)
                    nc.vector.tensor_copy(out=dsb, in_=den_ps[0:1, :])
                    rec = wk.tile([1, TB], F32, **RWF)
                    nc.vector.reciprocal(out=rec, in_=dsb)
                    rbc = bcast(rec[:])
                    nc.vector.tensor_tensor(out=avT[:, hd, :], in0=av_sb,
                                            in1=rbc, op=OP.mult)

                pend = None
                for hd in range(HQ):
                    av_ps = ps.tile([128, TB], F32, **MM)
                    den_ps = ps.tile([8, TB], F32, **ROW)
                    for i in range(ntk):
                        tk0 = i * 128
                        s = i - 4 * j
                        # diagonal tiles: columns < 128*s are fully masked, so
                        # restrict score/exp/den/av to the live column range
                        c0 = max(s, 0) * 128
                        w = TB - c0
                        sc_ps = ps.tile([128, TB], F32, **MM)
                        nc.tensor.matmul(sc_ps[:, c0:TB], kT_sb[:, tk0:tk0 + 128],
                                         qT_sb[:, hd, q0 + c0:q0 + TB],
                                         start=True, stop=True,
                                         skip_group_check=True)
                        ex = wk.tile([128, TB], BF16, **TBF)
                        if s < 0:
                            nc.scalar.activation(out=ex, in_=sc_ps, func=AF.Exp,
                                                 scale=rk_sb[:, i:i + 1])
                        else:
                            ext = wk.tile([128, TB], BF16, **TBF)
                            nc.scalar.activation(out=ext[:, c0:TB],
                                                 in_=sc_ps[:, c0:TB], func=AF.Exp,
                                                 scale=rk_sb[:, i:i + 1])
                            nc.vector.tensor_tensor(out=ex[:, c0:TB],
                                                    in0=ext[:, c0:TB],
                                                    in1=masks_sb[:, s, c0:TB],
                                                    op=OP.mult)
                        nc.tensor.matmul(den_ps[0:1, c0:TB], ones_bf,
                                         ex[:, c0:TB],
                                         start=(i == 0), stop=(i == ntk - 1),
                                         skip_group_check=True)
                        nc.tensor.matmul(av_ps[:, c0:TB], vnat_sb[:, i, :],
                                         ex[:, c0:TB],
                                         start=(i == 0), stop=(i == ntk - 1),
                                         skip_group_check=True)
                    # free the av psum early via an Act copy; fin reads SBUF
                    av_sb = wk.tile([128, TB], F32, **BCS)
                    nc.scalar.copy(out=av_sb, in_=av_ps)
                    if pend is not None:
                        fin(*pend)
                    pend = (hd, av_sb, den_ps)
                fin(*pend)
                for mq in range(4):
                    attq = ab.tile([128, 4, TB], F16, tag="attb", bufs=2)
                    for mi in range(4):
                        m = 4 * mq + mi
                        att_ps = ps.tile([128, TB], F32, **MM)
                        for hk in range(HQ):
                            nc.tensor.matmul(att_ps, ow_sb[:, m, hk, :],
                                             avT[:, hk, :], start=(hk == 0),
                                             stop=(hk == HQ - 1))
                        nc.scalar.copy(out=attq[:, mi, :], in_=att_ps)
                    for tg in range(4):
                        nc.sync.dma_start(
                            out=rsin[b][4 * j + tg, :, 4 * mq:4 * mq + 4, :],
                            in_=attq[:, :, tg * 128:(tg + 1) * 128])
                if j == 1:
                    nc.gpsimd.collective_compute(
                        "ReduceScatter", OP.add,
                        replica_groups=[list(range(N_CORES))],
                        ins=[rsin[b].opt()], outs=[rsout[b].opt()],
                    )

            # ========================= Phase C: MoE ========================
            HGU = FK * 8 * 2 * 128    # flat size of one gate (or up) half

            def load_guw_half(e, half, eng=None, after=None):
                t = wk.tile([128, FK * 8, 2, 128], F8, tag="wgu", bufs=4)
                d = (eng or nc.gpsimd).dma_start(
                    out=t,
                    in_=guw[:, e, half * HGU:(half + 1) * HGU].rearrange(
                        "p (a b c) -> p a b c", b=2, c=128))
                if after is not None:
                    _add_dep_helper(d.ins, after.ins, sync=True,
                                    reason="prefetch after startup loads")
                return t

            def load_dww(m, eng=None, after=None):
                t = wk.tile([128, E * 3, 2, 128], F8, tag="wdw", bufs=2)
                d = (eng or nc.sync).dma_start(out=t, in_=dww[:, m, :].rearrange(
                    "p (a b c) -> p a b c", b=2, c=128))
                if after is not None:
                    _add_dep_helper(d.ins, after.ins, sync=True,
                                    reason="prefetch after startup loads")
                return t

            def phaseC(pre_gu, pre_dw, moe):
                # assemble own hidden = attn partial sums (+x) for owned tokens
                for b in range(B):
                    nc.sync.dma_start(out=xh_sb[:, :, 128 * b:128 * b + 128],
                                      in_=rsout[b][:])
                prod8_sb = moe.tile([128, E, FK, NOWN], F8, name="prod8_sb")
                xow = moe.tile([128, CK, NOWN], F16, name="xow_all")
                nc.sync.dma_start(out=xow, in_=xown[:])
                for fc in range(CK):
                    nc.vector.tensor_tensor(out=xh_sb[:, fc, :],
                                            in0=xh_sb[:, fc, :],
                                            in1=xow[:, fc, :], op=OP.add)
                lg_ps = ps.tile([8, TB], F32, **ROW)
                den2_ps = ps.tile([8, TB], F32, **ROW)
                for fc in range(CK):
                    nc.tensor.matmul(lg_ps[0:E, 0:NOWN], gatew_sb[:, fc, :],
                                     xh_sb[:, fc, :],
                                     start=(fc == 0), stop=(fc == CK - 1))
                    sq = wk.tile([128, NOWN], BF16, **MC)
                    nc.scalar.activation(out=sq, in_=xh_sb[:, fc, :], func=AF.Square)
                    nc.tensor.matmul(den2_ps[0:1, 0:NOWN], ones_bf, sq,
                                     start=(fc == 0), stop=(fc == CK - 1))
                s2 = wk.tile([1, NOWN], F32, **SM1)
                nc.scalar.activation(out=s2, in_=den2_ps[0:1, 0:NOWN], func=AF.Sqrt,
                                     scale=1.0 / C, bias=bias_eps)
                rstd2 = wk.tile([1, NOWN], F32, tag="rstd2", bufs=1)
                nc.vector.reciprocal(out=rstd2, in_=s2)
                r2bc = bcast(rstd2[:], width=NOWN)
                for fc in range(CK):
                    nc.vector.tensor_tensor(out=xn8_sb[:, fc, :],
                                            in0=xh_sb[:, fc, :],
                                            in1=r2bc, op=OP.mult)

                # ---- top-2 routing on [E, NOWN] ----
                lg = wk.tile([E, NOWN], F32, tag="lg", bufs=1)
                nc.vector.tensor_copy(out=lg, in_=lg_ps[0:E, 0:NOWN])
                m1 = wk.tile([1, NOWN], F32, **SM1)
                nc.gpsimd.tensor_reduce(out=m1, in_=lg, axis=mybir.AxisListType.C,
                                        op=OP.max)
                m1bc = bcast8(m1[:])
                eq1 = wk.tile([E, NOWN], F32, tag="eq1", bufs=1)
                nc.vector.tensor_tensor(out=eq1, in0=lg, in1=m1bc, op=OP.is_equal)
                lg2 = wk.tile([E, NOWN], F32, **SM8)
                nc.vector.scalar_tensor_tensor(out=lg2, in0=eq1, scalar=-BIG,
                                               in1=lg, op0=OP.mult, op1=OP.add)
                m2 = wk.tile([1, NOWN], F32, **SM1)
                nc.gpsimd.tensor_reduce(out=m2, in_=lg2, axis=mybir.AxisListType.C,
                                        op=OP.max)
                m2bc = bcast8(m2[:])
                eq2 = wk.tile([E, NOWN], F32, **SM8)
                nc.vector.tensor_tensor(out=eq2, in0=lg, in1=m2bc, op=OP.is_equal)
                # dlt = (m1-m2)*rstd2 ; w1 = sigmoid(dlt); w2 = 1-w1
                dlt = wk.tile([1, NOWN], F32, **SM1)
                nc.vector.tensor_tensor(out=dlt, in0=m1, in1=m2, op=OP.subtract)
                dlts = wk.tile([1, NOWN], F32, **SM1)
                nc.vector.tensor_tensor(out=dlts, in0=dlt, in1=rstd2, op=OP.mult)
                w1 = wk.tile([1, NOWN], F32, **SM1)
                nc.scalar.activation(out=w1, in_=dlts, func=AF.Sigmoid)
                w1bc = bcast8(w1[:])
                # comb = eq1*w1 + eq2*(1-w1) = (eq1-eq2)*w1 + eq2, in place
                nc.vector.tensor_tensor(out=eq1, in0=eq1, in1=eq2, op=OP.subtract)
                nc.vector.tensor_tensor(out=eq1, in0=eq1, in1=w1bc, op=OP.mult)
                nc.vector.tensor_tensor(out=eq1, in0=eq1, in1=eq2, op=OP.add)
                # scale by PSC/WS (prod fp8 scale / up-weight descale)
                nc.vector.tensor_scalar(out=comb_row, in0=eq1, scalar1=PSC / WS,
                                        scalar2=None, op0=OP.mult)

                # ---- pass 1: gate/up + silu -> prod8 per expert ----
                def load_guw_moe(e, half, eng):
                    t = moe.tile([128, FK * 8, 2, 128], F8, tag="wgu2", bufs=4)
                    eng.dma_start(
                        out=t,
                        in_=guw[:, e, half * HGU:(half + 1) * HGU].rearrange(
                            "p (a b c) -> p a b c", b=2, c=128))
                    return t

                def load_dww_moe(m, eng):
                    t = moe.tile([128, E * 3, 2, 128], F8, tag="wdw2", bufs=3)
                    eng.dma_start(out=t, in_=dww[:, m, :].rearrange(
                        "p (a b c) -> p a b c", b=2, c=128))
                    return t

                comb_bf = wk.tile([E, NOWN], BF16, tag="combbf", bufs=1)
                nc.vector.tensor_copy(out=comb_bf, in_=comb_row)
                for e in range(E):
                    wgg = pre_gu.pop((e, 0), None) or load_guw_moe(e, 0, nc.sync)
                    wgu = pre_gu.pop((e, 1), None) or load_guw_moe(e, 1, nc.scalar)
                    cb_ps = ps.tile([128, TB], F32, **MM)
                    nc.tensor.matmul(cb_ps[:, 0:NOWN],
                                     sel8_sb[:, e * 128:(e + 1) * 128],
                                     comb_bf, start=True, stop=True)
                    cbc = wk.tile([128, NOWN], F32, tag="cbc", bufs=1)
                    nc.vector.tensor_copy(out=cbc, in_=cb_ps[:, 0:NOWN])
                    for f in range(FK):
                        g_ps = ps.tile([128, TB], F32, **MM)
                        for kp in range(8):
                            nc.tensor.matmul(
                                g_ps[:, 0:NOWN], wgg[:, f * 8 + kp, :, :],
                                xn8_sb[:, 2 * kp:2 * kp + 2, :],
                                start=(kp == 0), stop=(kp == 7),
                                perf_mode=PM.DoubleRow,
                            )
                        u_ps = ps.tile([128, TB], F32, **MM)
                        for kp in range(8):
                            nc.tensor.matmul(
                                u_ps[:, 0:NOWN], wgu[:, f * 8 + kp, :, :],
                                xn8_sb[:, 2 * kp:2 * kp + 2, :],
                                start=(kp == 0), stop=(kp == 7),
                                perf_mode=PM.DoubleRow,
                            )
                        sil = wk.tile([128, NOWN], BF16, **MC)
                        nc.scalar.activation(out=sil, in_=g_ps[:, 0:NOWN],
                                             func=AF.Silu, scale=1.0 / WS)
                        ucm = wk.tile([128, NOWN], BF16, **MC)
                        nc.vector.tensor_tensor(out=ucm, in0=u_ps[:, 0:NOWN],
                                                in1=cbc, op=OP.mult)
                        nc.vector.tensor_tensor(out=prod8_sb[:, e, f, :],
                                                in0=sil, in1=ucm, op=OP.mult)

                # ---- pass 2: down proj, accumulate experts in psum ----
                for m in range(CK):
                    wd = pre_dw.pop(m, None) or load_dww_moe(m, nc.sync)
                    eo_ps = ps.tile([128, TB], F32, **MM)
                    for e in range(E):
                        for kp in range(3):
                            nc.tensor.matmul(
                                eo_ps[:, 0:NOWN], wd[:, e * 3 + kp, :, :],
                                prod8_sb[:, e, 2 * kp:2 * kp + 2, :],
                                start=(e == 0 and kp == 0),
                                stop=(e == E - 1 and kp == 2),
                                perf_mode=PM.DoubleRow,
                            )
                    ym = wk.tile([128, NOWN], F16, tag="ymc", bufs=2)
                    nc.vector.scalar_tensor_tensor(
                        out=ym, in0=eo_ps[:, 0:NOWN],
                        scalar=1.0 / (WS * PSC), in1=xh_sb[:, m, :],
                        op0=OP.mult, op1=OP.add,
                    )
                    nc.sync.dma_start(out=y[:, m, :], in_=ym)

            pre_gu, pre_dw = {}, {}
            for g in range(NBLK):
                if 'A' in phases:
                    phaseA(g)
                if g == 0:
                    deferred_const_loads()
                    if 'C' in phases:
                        # act-queue prefetches: the Act sequencer reaches these
                        # only after A0's first Square, keeping the DMA engines
                        # free for the critical startup loads
                        for e in range(2):
                            for half in range(2):
                                pre_gu[(e, half)] = load_guw_half(
                                    e, half, nc.scalar, after=lastconst_dma)
                        pre_dw[0] = load_dww(0, nc.scalar, after=lastconst_dma)
                        pre_dw[1] = load_dww(1, nc.scalar, after=lastconst_dma)
                if 'B' in phases:
                    phaseB(g)
            ab.release()
            if 'C' in phases:
                with tc.tile_pool(name="moe", bufs=1) as moe:
                    phaseC(pre_gu, pre_dw, moe)

    _split_multi_waits(nc)
    return nc


# ---------------------------------------------------------------------------

_NC_CACHE = {}


def _get_nc():
    if "nc" not in _NC_CACHE:
        _NC_CACHE["nc"] = build_nc()
    return _NC_CACHE["nc"]


def _chunk_pm(a, nchunk):
    """[nchunk*128, free...] -> [128, nchunk, free...]"""
    return np.ascontiguousarray(
        a.reshape(nchunk, 128, *a.shape[1:]).transpose(1, 0, *range(2, a.ndim + 1))
    )


def prepare_in_maps(x, cos, sin, ln1_w, q_w, k_w, v_w, o_w, qn_w, kn_w, ln2_w,
                    gate_w, gate_up_w, down_w):
    bf = ml_dtypes.bfloat16
    f8 = ml_dtypes.float8_e4m3
    x = np.asarray(x, dtype=np.float32)
    x_flat = x.reshape(N, C)

    xT = _chunk_pm(np.ascontiguousarray(x_flat.T).astype(bf), CK)
    rstd1 = (1.0 / np.sqrt((x_flat.astype(np.float64) ** 2).mean(axis=1) + EPS)
             ).astype(np.float32)[None, :]

    ln1 = np.asarray(ln1_w, dtype=np.float32)[:, None]
    ln2 = np.asarray(ln2_w, dtype=np.float32)[:, None]
    qwf = np.asarray(q_w, dtype=np.float32) * ln1
    kwf = np.asarray(k_w, dtype=np.float32) * ln1
    vwf = np.asarray(v_w, dtype=np.float32) * ln1
    gatewf = np.asarray(gate_w, dtype=np.float32) * ln2
    guwf = np.asarray(gate_up_w, dtype=np.float32) * ln2[None]    # [E, C, 2F]
    dwf = np.asarray(down_w, dtype=np.float32)                    # [E, F, C]
    owf = np.asarray(o_w, dtype=np.float32)

    cos0 = np.asarray(cos, dtype=np.float32)[0]
    sin0 = np.asarray(sin, dtype=np.float32)[0]
    cosT = np.ascontiguousarray(cos0.T).astype(bf)
    sinT = np.ascontiguousarray(sin0.T).astype(bf)
    protm = np.zeros((128, 128), dtype=np.float32)
    for m in range(64):
        protm[m + 64, m] = -1.0
    for m in range(64, 128):
        protm[m - 64, m] = 1.0

    r = np.arange(128)[:, None]
    col = np.arange(TB)[None, :]
    masks = np.stack(
        [(col >= r + 128 * s).astype(bf) for s in range(4)], axis=1
    )

    # fp8 MoE weights, shared across cores
    # guw host layout: [128, E, FGU*8*2*128]; lhsT slice [128, 2, 128] is
    # (grp, kp) with pair index i selecting k-chunk 2kp+i.
    gu6 = (guwf * WS).astype(f8)                       # [E, C, 2F]
    gu_r = gu6.reshape(E, 8, 2, 128, FGU, 128)         # e, kp, i, p, grp, d
    guw_h = np.ascontiguousarray(
        gu_r.transpose(3, 0, 4, 1, 2, 5).reshape(128, E, FGU * 8 * 2 * 128))
    # dww host layout: [128, CK(m), E*3*2*128]; lhsT slice (e, kp) pair i
    # selects f-chunk 2kp+i; partition p = f % 128; d = c within group m.
    dw6 = (dwf * WS).astype(f8)                        # [E, F, C]
    dw_r = dw6.reshape(E, 3, 2, 128, CK, 128)          # e, kp, i, p, m, d
    dww_h = np.ascontiguousarray(
        dw_r.transpose(3, 4, 0, 1, 2, 5).reshape(128, CK, E * 3 * 2 * 128))

    gatew_h = _chunk_pm(gatewf.astype(np.float16), CK)

    in_maps = []
    for c in range(N_CORES):
        oslice = owf[512 * c:512 * (c + 1), :].astype(bf)  # [512, C]
        o4 = oslice.reshape(HQ, 128, CK, 128)              # hk, p, m, d
        ow_h = np.ascontiguousarray(o4.transpose(1, 2, 0, 3))
        # owned tokens: batch b local [128c, 128c+128)
        own_idx = np.concatenate([
            np.arange(b * T + 128 * c, b * T + 128 * (c + 1)) for b in range(B)
        ])
        xo = x_flat[own_idx, :].T                          # [C, 256]
        xown_h = _chunk_pm(np.ascontiguousarray(xo).astype(np.float16), CK)
        in_maps.append({
            "xT": xT,
            "xown": xown_h,
            "qw": np.ascontiguousarray(
                qwf[:, 512 * c:512 * (c + 1)].astype(bf)
                .reshape(CK, 128, HQ, 128).transpose(1, 2, 0, 3)),
            "kw": _chunk_pm(kwf[:, 128 * c:128 * (c + 1)].astype(bf), CK),
            "vw": _chunk_pm(vwf[:, 128 * c:128 * (c + 1)].astype(bf), CK),
            "ow": ow_h,
            "gatew": gatew_h,
            "guw": guw_h,
            "dww": dww_h,
            "cosb": cosT,
            "sinb": sinT,
            "masks": masks,
            "rstd1": rstd1,
            "qnw": np.asarray(qn_w, dtype=np.float32)[:, None],
            "knw": np.asarray(kn_w, dtype=np.float32)[:, None],
            "protb": protm.astype(bf),
            "sel8": np.kron(np.eye(E, dtype=np.float32),
                            np.ones((1, 128), dtype=np.float32)).astype(bf),
        })

    return in_maps


def combine(ys):
    out = np.zeros((N, C), dtype=np.float32)
    for c in range(N_CORES):
        yc = np.asarray(ys[c], dtype=np.float32)     # [128, CK, 256]
        # yc[p, fc, 128*b + i] -> token b*T + 128*c + i, feature fc*128+p
        feat_major = yc.transpose(1, 0, 2).reshape(C, NOWN)
        for b in range(B):
            toks = slice(b * T + 128 * c, b * T + 128 * (c + 1))
            out[toks, :] = feat_major[:, 128 * b:128 * (b + 1)].T
    return out.reshape(B, T, C)


def kernel(**inputs):
    in_maps = prepare_in_maps(**inputs)
    nc = _get_nc()
    res = run_bass_kernel_spmd(nc, in_maps, core_ids=list(range(N_CORES)))
    return combine([res.results[c]["y"] for c in range(N_CORES)])


# revision 10
# speedup vs baseline: 1.1542x; 1.0107x over previous
"""Fused attention+MoE block on 8 trn2 NeuronCores, v2.

Sharding: tensor-parallel attention (4 q-heads + 1 KV-head per core) as
before, but the attention output partials are ReduceScattered (2 per-batch
collectives) so each core ends up owning 256 tokens of the full hidden
state in feature-major layout. The MoE is then data-parallel: every core
runs all 8 experts densely on its own 256 tokens with fp8e4 DoubleRow
matmuls (2 k-chunks per instruction) and fp8 weights streamed from HBM.
Routing (top-2 over E=8) is computed on an [E, 256] tile with
partition-dim max reductions. Output y = hidden + moe for the owned
tokens; the host just concatenates core slices.
"""

import numpy as np
import ml_dtypes

import concourse.bass as bass
from concourse.bass import _add_dep_helper
import concourse.mybir as mybir
import concourse.tile as tile
from concourse.bass_utils import run_bass_kernel_spmd
from concourse.masks import make_identity
from concourse.vector_clock import ScopedClock

F32 = mybir.dt.float32
F32R = mybir.dt.float32r
F16 = mybir.dt.float16
BF16 = mybir.dt.bfloat16
F8 = mybir.dt.float8e4
U32 = mybir.dt.uint32
AF = mybir.ActivationFunctionType
OP = mybir.AluOpType
PM = mybir.MatmulPerfMode

B, T, C = 2, 1024, 2048
H, KV, D = 32, 8, 128
E, F, TOPK = 8, 768, 2
N = B * T
CK = C // 128          # 16
NBLK = 4               # token blocks of 512
TB = 512
HQ = H // 8            # 4 q heads per core
FK = F // 128          # 6
FGU = 2 * F // 128     # 12
NOWN = 256             # tokens owned per core (128 per batch)
WS = 64.0              # fp8 weight scale
PSC = 16.0             # fp8 prod scale
EPS = 1e-6
N_CORES = 8
BIG = 1e9

# ---------------------------------------------------------------------------
# walrus here rejects >1 sync-wait per instruction; split extras onto NoOps.


class _SplitDrainTileContext(tile.TileContext):
    def _drain_and_barrier(self, tick_clock, wait_clock):
        drain_inst = self.nc.sync.drain()
        wait_clock.add_sem_waits(
            drain_inst.ins, ScopedClock({None: tick_clock.global_clock})
        )
        si = drain_inst.ins.sync_info
        if si is not None and len(si.on_wait) > 1:
            ow = list(si.on_wait)
            drain_inst.ins.sync_info = mybir.SyncInfo(
                on_wait=ow[:1], on_update=list(si.on_update)
            )
            rest = ow[1:]
            while rest:
                extra = self.nc.sync.drain()
                extra.ins.sync_info = mybir.SyncInfo(on_wait=rest[:1], on_update=[])
                rest = rest[1:]
        self.nc.all_engine_barrier()
        assert self.sems is not None
        popped = self.nc._tile_sem_poison_stack.pop()
        assert popped is self._sem_poison
        self.nc.clear_and_free_semaphores(list(self.sems.allocated().values()))
        self.nc.all_engine_barrier()


def _split_multi_waits(nc):
    for bb in nc.main_func.blocks:
        insts = list(bb.instructions)
        out = []
        changed = False
        for ins in insts:
            si = ins.sync_info
            if si is not None and len(si.on_wait) > 1:
                ow = list(si.on_wait)
                for w in ow[:-1]:
                    nop = mybir.InstNoOp(name=f"waitnop-{nc.next_id()}", ins=[], outs=[])
                    nop.engine = ins.engine
                    nop.sync_info = mybir.SyncInfo(on_wait=[w], on_update=[])
                    out.append(nop)
                ins.sync_info = mybir.SyncInfo(
                    on_wait=[ow[-1]], on_update=list(si.on_update)
                )
                changed = True
            out.append(ins)
        if changed:
            bb.instructions = out


# ---------------------------------------------------------------------------


def build_nc(phases='ABC'):
    nc = bass.Bass("TRN2", target_bir_lowering=False, debug=False, num_devices=N_CORES)

    xT = nc.dram_tensor("xT", [128, CK, N], BF16, kind="ExternalInput")
    xown = nc.dram_tensor("xown", [128, CK, NOWN], F16, kind="ExternalInput")
    qw = nc.dram_tensor("qw", [128, HQ, CK, 128], BF16, kind="ExternalInput")
    kw = nc.dram_tensor("kw", [128, CK, 128], BF16, kind="ExternalInput")
    vw = nc.dram_tensor("vw", [128, CK, 128], BF16, kind="ExternalInput")
    ow = nc.dram_tensor("ow", [128, CK, HQ, 128], BF16, kind="ExternalInput")
    gatew = nc.dram_tensor("gatew", [128, CK, E], F16, kind="ExternalInput")
    guw = nc.dram_tensor("guw", [128, E, FGU * 8 * 2 * 128], F8, kind="ExternalInput")
    dww = nc.dram_tensor("dww", [128, CK, E * 3 * 2 * 128], F8, kind="ExternalInput")
    cosb = nc.dram_tensor("cosb", [128, T], BF16, kind="ExternalInput")
    sinb = nc.dram_tensor("sinb", [128, T], BF16, kind="ExternalInput")
    masks = nc.dram_tensor("masks", [128, 4, TB], BF16, kind="ExternalInput")
    rstd1 = nc.dram_tensor("rstd1", [1, N], F32, kind="ExternalInput")
    qnw = nc.dram_tensor("qnw", [128, 1], F32, kind="ExternalInput")
    knw = nc.dram_tensor("knw", [128, 1], F32, kind="ExternalInput")
    protb = nc.dram_tensor("protb", [128, 128], BF16, kind="ExternalInput")
    sel8 = nc.dram_tensor("sel8", [E, E * 128], BF16, kind="ExternalInput")

    y = nc.dram_tensor("y", [128, CK, NOWN], F16, kind="ExternalOutput")

    with _SplitDrainTileContext(nc) as tc:
        with (
            tc.tile_pool(name="const", bufs=1) as cpool,
            tc.tile_pool(name="dram", bufs=1, space="DRAM") as dram,
            tc.tile_pool(name="ps", bufs=1, space="PSUM") as ps,
            tc.tile_pool(name="big", bufs=1) as big,
            tc.tile_pool(name="work", bufs=3) as wk,
        ):
            ab = tc.alloc_tile_pool(name="ab", bufs=1)
            MM = dict(tag="mm", bufs=5)       # f32 [128,TB] psum
            ROW = dict(tag="row", bufs=2)     # f32 [<=8,TB] psum
            TRP = dict(tag="trp", bufs=1)     # bf16 [128,128] psum
            TBF = dict(tag="t512b", bufs=5)   # bf16 [128,TB] transients
            RWF = dict(tag="rowf", bufs=3)    # f32 [1,TB]
            BCS = dict(tag="bcs", bufs=3)     # f32 [128,TB] bcast results
            SM8 = dict(tag="sm8", bufs=2)     # f32 [8,NOWN] small routing tiles
            SM1 = dict(tag="sm1", bufs=3)     # f32 [1,NOWN]
            MC = dict(tag="mc", bufs=4)       # bf16 [128,NOWN] moe transients

            # ---- constants ----
            ident = cpool.tile([128, 128], BF16)
            make_identity(nc, ident)
            ones_bf = cpool.tile([128, 1], BF16)
            nc.vector.memset(ones_bf, 1.0)
            onesrow_f = cpool.tile([1, 128], F32)
            nc.vector.memset(onesrow_f, 1.0)
            onesrow_r = cpool.tile([1, 128], F32R)
            nc.vector.tensor_copy(out=onesrow_r, in_=onesrow_f)
            ones8_f = cpool.tile([1, E], F32)
            nc.vector.memset(ones8_f, 1.0)
            ones8_r = cpool.tile([1, E], F32R)
            nc.vector.tensor_copy(out=ones8_r, in_=ones8_f)
            bias_q = cpool.tile([1, 1], F32)
            nc.vector.memset(bias_q, float(D) * EPS)
            bias_eps = cpool.tile([1, 1], F32)
            nc.vector.memset(bias_eps, EPS)
            idf1 = cpool.tile([1, 1], F32)
            nc.vector.memset(idf1, 1.0)

            xtb0 = ab.tile([128, CK, TB], BF16, tag="xtb", bufs=1, name="xtb0")
            nc.sync.dma_start(out=xtb0[:, 0:CK // 2, :], in_=xT[:, 0:CK // 2, 0:TB])
            x0dma = nc.sync.dma_start(out=xtb0[:, CK // 2:CK, :],
                                      in_=xT[:, CK // 2:CK, 0:TB])
            qw_sb = ab.tile([128, HQ, CK, 128], BF16)
            for _hd in range(HQ):
                nc.sync.dma_start(out=qw_sb[:, _hd, :, :], in_=qw[:, _hd, :, :])
            kw_sb = ab.tile([128, CK, 128], BF16)
            nc.sync.dma_start(out=kw_sb, in_=kw[:])
            vw_sb = ab.tile([128, CK, 128], BF16)
            nc.sync.dma_start(out=vw_sb, in_=vw[:])
            prot_sb = cpool.tile([128, 128], BF16)
            nc.sync.dma_start(out=prot_sb, in_=protb[:])
            cos_sb = cpool.tile([128, T], BF16)
            nc.sync.dma_start(out=cos_sb, in_=cosb[:])
            sin_sb = cpool.tile([128, T], BF16)
            nc.sync.dma_start(out=sin_sb, in_=sinb[:])
            masks_sb = cpool.tile([128, 4, TB], BF16)
            qnw_sb = cpool.tile([128, 1], F32)
            nc.sync.dma_start(out=qnw_sb, in_=qnw[:])
            knw_sb = cpool.tile([128, 1], F32)
            lastconst_dma = nc.sync.dma_start(out=knw_sb, in_=knw[:])
            gatew_sb = cpool.tile([128, CK, E], F16)
            sel8_sb = cpool.tile([E, E * 128], BF16)

            ow_sb = ab.tile([128, CK, HQ, 128], BF16)

            qT_sb = ab.tile([128, HQ, T], BF16)            # per-batch
            kT_sb = ab.tile([128, T], BF16)
            vnat_sb = ab.tile([128, T // 128, 128], BF16)
            rk_sb = ab.tile([128, T // 128], F32)     # per-key-tile rstd_k
            rv_sb = ab.tile([128, T // 128], F32)     # per-key-tile rstd1
            xh_sb = big.tile([128, CK, NOWN], F16)          # own hidden (x+attn)
            xn8_sb = big.tile([128, CK, NOWN], F8)
            comb_row = big.tile([E, NOWN], F32)             # routing weights

            def deferred_const_loads():
                nc.sync.dma_start(out=ow_sb, in_=ow[:])
                nc.sync.dma_start(out=masks_sb, in_=masks[:])
                nc.sync.dma_start(out=gatew_sb, in_=gatew[:])
                nc.sync.dma_start(out=sel8_sb, in_=sel8[:])

            rsin = [dram.tile([8, 128, CK, 128], F16, name=f"rsin{b}") for b in range(B)]
            rsout = [dram.tile([128, CK, 128], F16, name=f"rsout{b}") for b in range(B)]

            def bcast(row_f32_ap, width=TB, act_copy=False, **pool_kw):
                """[1,width] f32 -> SBUF [128,width] f32 via K=1 f32r matmul."""
                rr = wk.tile([1, TB], F32R, tag="rwr", bufs=2)
                nc.vector.tensor_copy(out=rr[:, 0:width], in_=row_f32_ap)
                bc_ps = ps.tile([128, TB], F32, **MM)
                nc.tensor.matmul(bc_ps[:, 0:width], onesrow_r,
                                 rr[:, 0:width], start=True, stop=True)
                kw_ = pool_kw or BCS
                bc = wk.tile([128, TB], F32, **kw_)
                if act_copy:
                    nc.scalar.copy(out=bc[:, 0:width], in_=bc_ps[:, 0:width])
                else:
                    nc.vector.tensor_copy(out=bc[:, 0:width], in_=bc_ps[:, 0:width])
                return bc[:, 0:width]

            def bcast8(row_f32_ap):
                """[1,NOWN] f32 -> SBUF [E,NOWN] f32, exact (fp32 matmul so
                is_equal against the source values still holds bitwise)."""
                p8 = ps.tile([8, TB], F32, **ROW)
                nc.tensor.matmul(p8[0:E, 0:NOWN], ones8_f, row_f32_ap,
                                 start=True, stop=True)
                t8 = wk.tile([E, NOWN], F32, **SM8)
                nc.vector.tensor_copy(out=t8, in_=p8[0:E, 0:NOWN])
                return t8

            def colsum_rstd(feat_ps, scale, bias_ap):
                """rsqrt(scale*colsum(feat^2)+bias) -> [1,TB] f32 row."""
                sq = wk.tile([128, TB], BF16, **TBF)
                nc.scalar.activation(out=sq, in_=feat_ps, func=AF.Square)
                ssum = ps.tile([8, TB], F32, **ROW)
                nc.tensor.matmul(ssum[0:1, :], ones_bf, sq, start=True, stop=True)
                srow = wk.tile([1, TB], F32, **RWF)
                nc.scalar.activation(out=srow, in_=ssum[0:1, :], func=AF.Sqrt,
                                     scale=scale, bias=bias_ap)
                rrow = wk.tile([1, TB], F32, **RWF)
                nc.vector.reciprocal(out=rrow, in_=srow)
                return rrow

            def rope_plain(feat_sb, j, w_sb, out_ap):
                """out = rope(w * feat); rstd applied separately (rope is
                linear per token, so the per-token scale commutes)."""
                tcol = j * TB
                qhat = wk.tile([128, TB], BF16, **TBF)
                nc.vector.tensor_scalar(out=qhat, in0=feat_sb, scalar1=w_sb,
                                        scalar2=None, op0=OP.mult)
                rot_ps = ps.tile([128, TB], F32, **MM)
                nc.tensor.matmul(rot_ps, prot_sb, qhat, start=True, stop=True)
                qc = wk.tile([128, TB], BF16, **TBF)
                nc.vector.tensor_tensor(
                    out=qc, in0=qhat, in1=cos_sb[:, tcol:tcol + TB], op=OP.mult,
                )
                rsm = wk.tile([128, TB], BF16, **TBF)
                nc.vector.tensor_tensor(
                    out=rsm, in0=rot_ps, in1=sin_sb[:, tcol:tcol + TB], op=OP.mult,
                )
                nc.vector.tensor_tensor(out=out_ap, in0=qc, in1=rsm, op=OP.add)

            # =========================== Phase A: QKV ======================
            def phaseA(g):
                b, j = divmod(g, 2)
                n0 = g * TB
                if g == 0:
                    xtb = xtb0
                else:
                    xtb = ab.tile([128, CK, TB], BF16, tag="xtb", bufs=1)
                    nc.sync.dma_start(out=xtb[:, 0:CK // 2, :],
                                      in_=xT[:, 0:CK // 2, n0:n0 + TB])
                    nc.sync.dma_start(out=xtb[:, CK // 2:CK, :],
                                      in_=xT[:, CK // 2:CK, n0:n0 + TB])
                r1row = wk.tile([1, TB], F32, **RWF)
                nc.sync.dma_start(out=r1row, in_=rstd1[:, n0:n0 + TB])
                for cc in range(TB // 128):
                    r1t = ps.tile([128, TB], F32, **MM)
                    nc.tensor.transpose(r1t[:, 0:1],
                                        r1row[:, cc * 128:(cc + 1) * 128], idf1)
                    nc.vector.tensor_copy(
                        out=rv_sb[:, 4 * j + cc:4 * j + cc + 1],
                        in_=r1t[:, 0:1])

                def accum(kind, hd):
                    p = ps.tile([128, TB], F32, **MM)
                    for kc in range(CK):
                        if kind == 'q':
                            lhs = qw_sb[:, hd, kc, :]
                        elif kind == 'k':
                            lhs = kw_sb[:, kc, :]
                        else:
                            lhs = vw_sb[:, kc, :]
                        nc.tensor.matmul(p, lhs, xtb[:, kc, :],
                                         start=(kc == 0), stop=(kc == CK - 1))
                    return p

                def stage2(kind, p):
                    # free the psum early: rope reads the bf16 SBUF copy
                    if kind == 'q':
                        row = colsum_rstd(p, 1.0, bias_q)
                    elif kind == 'k':
                        row = colsum_rstd(p, 1.0 / D, bias_eps)
                    else:
                        row = None
                    fsb = wk.tile([128, TB], BF16, tag="fsb", bufs=3)
                    nc.scalar.copy(out=fsb, in_=p)
                    return (row, fsb)

                def stage3(kind, hd, st2):
                    row, fsb = st2
                    if kind == 'q':
                        qbc = bcast(row[:], act_copy=True)
                        qr = wk.tile([128, TB], BF16, **TBF)
                        rope_plain(fsb, j, qnw_sb, qr)
                        nc.vector.tensor_tensor(
                            out=qT_sb[:, hd, j * TB:j * TB + TB],
                            in0=qr, in1=qbc, op=OP.mult)
                    elif kind == 'k':
                        rope_plain(fsb, j, knw_sb,
                                   kT_sb[:, j * TB:j * TB + TB])
                        for cc in range(TB // 128):
                            rkt = ps.tile([128, TB], F32, **MM)
                            nc.tensor.transpose(
                                rkt[:, 0:1], row[:, cc * 128:(cc + 1) * 128],
                                idf1)
                            nc.vector.tensor_copy(
                                out=rk_sb[:, 4 * j + cc:4 * j + cc + 1],
                                in_=rkt[:, 0:1])
                    else:
                        # x's rstd applied post-transpose, where token is the
                        # partition dim (per-partition scalar, no broadcast)
                        for cc in range(TB // 128):
                            vtr = ps.tile([128, 128], BF16, **TRP)
                            nc.tensor.transpose(
                                vtr, fsb[:, cc * 128:(cc + 1) * 128], ident)
                            nc.vector.tensor_scalar(
                                out=vnat_sb[:, j * 4 + cc, :], in0=vtr,
                                scalar1=rv_sb[:, 4 * j + cc:4 * j + cc + 1],
                                scalar2=None, op0=OP.mult)

                seq = [('q', 0), ('q', 1), ('q', 2), ('q', 3),
                       ('k', None), ('v', None)]
                st = []
                for idx, (kind, hd) in enumerate(seq):
                    p = accum(kind, hd)
                    st.append([kind, hd, p, None])
                    if idx >= 1:
                        st[idx - 1][3] = stage2(st[idx - 1][0], st[idx - 1][2])
                    if idx >= 2:
                        stage3(st[idx - 2][0], st[idx - 2][1], st[idx - 2][3])
                st[-1][3] = stage2(st[-1][0], st[-1][2])
                stage3(st[-2][0], st[-2][1], st[-2][3])
                stage3(st[-1][0], st[-1][1], st[-1][3])

            # ====================== Phase B: attention =====================
            def phaseB(g):
                b, j = divmod(g, 2)
                q0 = j * TB
                ntk = 4 * j + 4
                avT = ab.tile([128, HQ, TB], BF16, tag="avT", bufs=1)

                def fin(hd, av_sb, den_ps):
                    dsb = wk.tile([1, TB], F32, **RWF)
                    nc.vector.tensor_copy(out=dsb, in_=den_ps[0:1, :])
                    rec = wk.tile([1, TB], F32, **RWF)
                    nc.vector.reciprocal(out=rec, in_=dsb)
                    rbc = bcast(rec[:])
                    nc.vector.tensor_tensor(out=avT[:, hd, :], in0=av_sb,
                                            in1=rbc, op=OP.mult)

                pend = None
                for hd in range(HQ):
                    av_ps = ps.tile([128, TB], F32, **MM)
                    den_ps = ps.tile([8, TB], F32, **ROW)
                    for i in range(ntk):
                        tk0 = i * 128
                        s = i - 4 * j
                        # diagonal tiles: columns < 128*s are fully masked, so
                        # restrict score/exp/den/av to the live column range
                        c0 = max(s, 0) * 128
                        w = TB - c0
                        sc_ps = ps.tile([128, TB], F32, **MM)
                        nc.tensor.matmul(sc_ps[:, c0:TB], kT_sb[:, tk0:tk0 + 128],
                                         qT_sb[:, hd, q0 + c0:q0 + TB],
                                         start=True, stop=True,
                                         skip_group_check=True)
                        ex = wk.tile([128, TB], BF16, **TBF)
                        if s < 0:
                            nc.scalar.activation(out=ex, in_=sc_ps, func=AF.Exp,
                                                 scale=rk_sb[:, i:i + 1])
                        else:
                            ext = wk.tile([128, TB], BF16, **TBF)
                            nc.scalar.activation(out=ext[:, c0:TB],
                                                 in_=sc_ps[:, c0:TB], func=AF.Exp,
                                                 scale=rk_sb[:, i:i + 1])
                            nc.vector.tensor_tensor(out=ex[:, c0:TB],
                                                    in0=ext[:, c0:TB],
                                                    in1=masks_sb[:, s, c0:TB],
                                                    op=OP.mult)
                        nc.tensor.matmul(den_ps[0:1, c0:TB], ones_bf,
                                         ex[:, c0:TB],
                                         start=(i == 0), stop=(i == ntk - 1),
                                         skip_group_check=True)
                        nc.tensor.matmul(av_ps[:, c0:TB], vnat_sb[:, i, :],
                                         ex[:, c0:TB],
                                         start=(i == 0), stop=(i == ntk - 1),
                                         skip_group_check=True)
                    # free the av psum early via an Act copy; fin reads SBUF
                    av_sb = wk.tile([128, TB], F32, **BCS)
                    nc.scalar.copy(out=av_sb, in_=av_ps)
                    if pend is not None:
                        fin(*pend)
                    pend = (hd, av_sb, den_ps)
                fin(*pend)
                for mq in range(4):
                    attq = ab.tile([128, 4, TB], F16, tag="attb", bufs=2)
                    for mi in range(4):
                        m = 4 * mq + mi
                        att_ps = ps.tile([128, TB], F32, **MM)
                        for hk in range(HQ):
                            nc.tensor.matmul(att_ps, ow_sb[:, m, hk, :],
                                             avT[:, hk, :], start=(hk == 0),
                                             stop=(hk == HQ - 1))
                        nc.scalar.copy(out=attq[:, mi, :], in_=att_ps)
                    for tg in range(4):
                        nc.sync.dma_start(
                            out=rsin[b][4 * j + tg, :, 4 * mq:4 * mq + 4, :],
                            in_=attq[:, :, tg * 128:(tg + 1) * 128])
                if j == 1:
                    nc.gpsimd.collective_compute(
                        "ReduceScatter", OP.add,
                        replica_groups=[list(range(N_CORES))],
                        ins=[rsin[b].opt()], outs=[rsout[b].opt()],
                    )

            # ========================= Phase C: MoE ========================
            HGU = FK * 8 * 2 * 128    # flat size of one gate (or up) half

            def load_guw_half(e, half, eng=None, after=None):
                t = wk.tile([128, FK * 8, 2, 128], F8, tag="wgu", bufs=4)
                d = (eng or nc.gpsimd).dma_start(
                    out=t,
                    in_=guw[:, e, half * HGU:(half + 1) * HGU].rearrange(
                        "p (a b c) -> p a b c", b=2, c=128))
                if after is not None:
                    _add_dep_helper(d.ins, after.ins, sync=True,
                                    reason="prefetch after startup loads")
                return t

            def load_dww(m, eng=None, after=None):
                t = wk.tile([128, E * 3, 2, 128], F8, tag="wdw", bufs=2)
                d = (eng or nc.sync).dma_start(out=t, in_=dww[:, m, :].rearrange(
                    "p (a b c) -> p a b c", b=2, c=128))
                if after is not None:
                    _add_dep_helper(d.ins, after.ins, sync=True,
                                    reason="prefetch after startup loads")
                return t

            def phaseC(pre_gu, pre_dw, moe):
                # assemble own hidden = attn partial sums (+x) for owned tokens
                for b in range(B):
                    nc.sync.dma_start(out=xh_sb[:, :, 128 * b:128 * b + 128],
                                      in_=rsout[b][:])
                prod8_sb = moe.tile([128, E, FK, NOWN], F8, name="prod8_sb")
                xow = moe.tile([128, CK, NOWN], F16, name="xow_all")
                nc.sync.dma_start(out=xow, in_=xown[:])
                # per-batch halves: batch 0's adds run during the RS1 wait
                for b in range(B):
                    cs = slice(128 * b, 128 * b + 128)
                    for fc in range(CK):
                        nc.vector.tensor_tensor(out=xh_sb[:, fc, cs],
                                                in0=xh_sb[:, fc, cs],
                                                in1=xow[:, fc, cs], op=OP.add)
                lg_ps = ps.tile([8, TB], F32, **ROW)
                den2_ps = ps.tile([8, TB], F32, **ROW)
                for fc in range(CK):
                    nc.tensor.matmul(lg_ps[0:E, 0:NOWN], gatew_sb[:, fc, :],
                                     xh_sb[:, fc, :],
                                     start=(fc == 0), stop=(fc == CK - 1))
                    sq = wk.tile([128, NOWN], BF16, **MC)
                    nc.scalar.activation(out=sq, in_=xh_sb[:, fc, :], func=AF.Square)
                    nc.tensor.matmul(den2_ps[0:1, 0:NOWN], ones_bf, sq,
                                     start=(fc == 0), stop=(fc == CK - 1))
                s2 = wk.tile([1, NOWN], F32, **SM1)
                nc.scalar.activation(out=s2, in_=den2_ps[0:1, 0:NOWN], func=AF.Sqrt,
                                     scale=1.0 / C, bias=bias_eps)
                rstd2 = wk.tile([1, NOWN], F32, tag="rstd2", bufs=1)
                nc.vector.reciprocal(out=rstd2, in_=s2)
                r2bc = bcast(rstd2[:], width=NOWN)
                for fc in range(CK):
                    nc.vector.tensor_tensor(out=xn8_sb[:, fc, :],
                                            in0=xh_sb[:, fc, :],
                                            in1=r2bc, op=OP.mult)

                # ---- top-2 routing on [E, NOWN] ----
                lg = wk.tile([E, NOWN], F32, tag="lg", bufs=1)
                nc.vector.tensor_copy(out=lg, in_=lg_ps[0:E, 0:NOWN])
                m1 = wk.tile([1, NOWN], F32, **SM1)
                nc.gpsimd.tensor_reduce(out=m1, in_=lg, axis=mybir.AxisListType.C,
                                        op=OP.max)
                m1bc = bcast8(m1[:])
                eq1 = wk.tile([E, NOWN], F32, tag="eq1", bufs=1)
                nc.vector.tensor_tensor(out=eq1, in0=lg, in1=m1bc, op=OP.is_equal)
                lg2 = wk.tile([E, NOWN], F32, **SM8)
                nc.vector.scalar_tensor_tensor(out=lg2, in0=eq1, scalar=-BIG,
                                               in1=lg, op0=OP.mult, op1=OP.add)
                m2 = wk.tile([1, NOWN], F32, **SM1)
                nc.gpsimd.tensor_reduce(out=m2, in_=lg2, axis=mybir.AxisListType.C,
                                        op=OP.max)
                m2bc = bcast8(m2[:])
                eq2 = wk.tile([E, NOWN], F32, **SM8)
                nc.vector.tensor_tensor(out=eq2, in0=lg, in1=m2bc, op=OP.is_equal)
                # dlt = (m1-m2)*rstd2 ; w1 = sigmoid(dlt); w2 = 1-w1
                dlt = wk.tile([1, NOWN], F32, **SM1)
                nc.vector.tensor_tensor(out=dlt, in0=m1, in1=m2, op=OP.subtract)
                dlts = wk.tile([1, NOWN], F32, **SM1)
                nc.vector.tensor_tensor(out=dlts, in0=dlt, in1=rstd2, op=OP.mult)
                w1 = wk.tile([1, NOWN], F32, **SM1)
                nc.scalar.activation(out=w1, in_=dlts, func=AF.Sigmoid)
                w1bc = bcast8(w1[:])
                # comb = eq1*w1 + eq2*(1-w1) = (eq1-eq2)*w1 + eq2, in place
                nc.vector.tensor_tensor(out=eq1, in0=eq1, in1=eq2, op=OP.subtract)
                nc.vector.tensor_tensor(out=eq1, in0=eq1, in1=w1bc, op=OP.mult)
                nc.vector.tensor_tensor(out=eq1, in0=eq1, in1=eq2, op=OP.add)
                # scale by PSC/WS (prod fp8 scale / up-weight descale)
                nc.vector.tensor_scalar(out=comb_row, in0=eq1, scalar1=PSC / WS,
                                        scalar2=None, op0=OP.mult)

                # ---- pass 1: gate/up + silu -> prod8 per expert ----
                def load_guw_moe(e, half, eng):
                    t = moe.tile([128, FK * 8, 2, 128], F8, tag="wgu2", bufs=4)
                    eng.dma_start(
                        out=t,
                        in_=guw[:, e, half * HGU:(half + 1) * HGU].rearrange(
                            "p (a b c) -> p a b c", b=2, c=128))
                    return t

                def load_dww_moe(m, eng):
                    t = moe.tile([128, E * 3, 2, 128], F8, tag="wdw2", bufs=3)
                    eng.dma_start(out=t, in_=dww[:, m, :].rearrange(
                        "p (a b c) -> p a b c", b=2, c=128))
                    return t

                comb_bf = wk.tile([E, NOWN], BF16, tag="combbf", bufs=1)
                nc.vector.tensor_copy(out=comb_bf, in_=comb_row)
                for e in range(E):
                    wgg = pre_gu.pop((e, 0), None) or load_guw_moe(e, 0, nc.sync)
                    wgu = pre_gu.pop((e, 1), None) or load_guw_moe(e, 1, nc.scalar)
                    cb_ps = ps.tile([128, TB], F32, **MM)
                    nc.tensor.matmul(cb_ps[:, 0:NOWN],
                                     sel8_sb[:, e * 128:(e + 1) * 128],
                                     comb_bf, start=True, stop=True)
                    cbc = wk.tile([128, NOWN], F32, tag="cbc", bufs=1)
                    nc.vector.tensor_copy(out=cbc, in_=cb_ps[:, 0:NOWN])
                    for f in range(FK):
                        g_ps = ps.tile([128, TB], F32, **MM)
                        for kp in range(8):
                            nc.tensor.matmul(
                                g_ps[:, 0:NOWN], wgg[:, f * 8 + kp, :, :],
                                xn8_sb[:, 2 * kp:2 * kp + 2, :],
                                start=(kp == 0), stop=(kp == 7),
                                perf_mode=PM.DoubleRow,
                            )
                        u_ps = ps.tile([128, TB], F32, **MM)
                        for kp in range(8):
                            nc.tensor.matmul(
                                u_ps[:, 0:NOWN], wgu[:, f * 8 + kp, :, :],
                                xn8_sb[:, 2 * kp:2 * kp + 2, :],
                                start=(kp == 0), stop=(kp == 7),
                                perf_mode=PM.DoubleRow,
                            )
                        sil = wk.tile([128, NOWN], BF16, **MC)
                        nc.scalar.activation(out=sil, in_=g_ps[:, 0:NOWN],
                                             func=AF.Silu, scale=1.0 / WS)
                        ucm = wk.tile([128, NOWN], BF16, **MC)
                        nc.vector.tensor_tensor(out=ucm, in0=u_ps[:, 0:NOWN],
                                                in1=cbc, op=OP.mult)
                        nc.vector.tensor_tensor(out=prod8_sb[:, e, f, :],
                                                in0=sil, in1=ucm, op=OP.mult)

                # ---- pass 2: down proj, accumulate experts in psum ----
                for m in range(CK):
                    wd = pre_dw.pop(m, None) or load_dww_moe(m, nc.sync)
                    eo_ps = ps.tile([128, TB], F32, **MM)
                    for e in range(E):
                        for kp in range(3):
                            nc.tensor.matmul(
                                eo_ps[:, 0:NOWN], wd[:, e * 3 + kp, :, :],
                                prod8_sb[:, e, 2 * kp:2 * kp + 2, :],
                                start=(e == 0 and kp == 0),
                                stop=(e == E - 1 and kp == 2),
                                perf_mode=PM.DoubleRow,
                            )
                    ym = wk.tile([128, NOWN], F16, tag="ymc", bufs=2)
                    nc.vector.scalar_tensor_tensor(
                        out=ym, in0=eo_ps[:, 0:NOWN],
                        scalar=1.0 / (WS * PSC), in1=xh_sb[:, m, :],
                        op0=OP.mult, op1=OP.add,
                    )
                    nc.sync.dma_start(out=y[:, m, :], in_=ym)

            pre_gu, pre_dw = {}, {}
            for g in range(NBLK):
                if 'A' in phases:
                    phaseA(g)
                if g == 0:
                    deferred_const_loads()
                    if 'C' in phases:
                        # act-queue prefetches: the Act sequencer reaches these
                        # only after A0's first Square, keeping the DMA engines
                        # free for the critical startup loads
                        for e in range(2):
                            for half in range(2):
                                pre_gu[(e, half)] = load_guw_half(
                                    e, half, nc.scalar, after=lastconst_dma)
                        pre_dw[0] = load_dww(0, nc.scalar, after=lastconst_dma)
                        pre_dw[1] = load_dww(1, nc.scalar, after=lastconst_dma)
                if 'B' in phases:
                    phaseB(g)
            ab.release()
            if 'C' in phases:
                with tc.tile_pool(name="moe", bufs=1) as moe:
                    phaseC(pre_gu, pre_dw, moe)

    _split_multi_waits(nc)
    return nc


# ---------------------------------------------------------------------------

_NC_CACHE = {}


def _get_nc():
    if "nc" not in _NC_CACHE:
        _NC_CACHE["nc"] = build_nc()
    return _NC_CACHE["nc"]


def _chunk_pm(a, nchunk):
    """[nchunk*128, free...] -> [128, nchunk, free...]"""
    return np.ascontiguousarray(
        a.reshape(nchunk, 128, *a.shape[1:]).transpose(1, 0, *range(2, a.ndim + 1))
    )


def prepare_in_maps(x, cos, sin, ln1_w, q_w, k_w, v_w, o_w, qn_w, kn_w, ln2_w,
                    gate_w, gate_up_w, down_w):
    bf = ml_dtypes.bfloat16
    f8 = ml_dtypes.float8_e4m3
    x = np.asarray(x, dtype=np.float32)
    x_flat = x.reshape(N, C)

    xT = _chunk_pm(np.ascontiguousarray(x_flat.T).astype(bf), CK)
    rstd1 = (1.0 / np.sqrt((x_flat.astype(np.float64) ** 2).mean(axis=1) + EPS)
             ).astype(np.float32)[None, :]

    ln1 = np.asarray(ln1_w, dtype=np.float32)[:, None]
    ln2 = np.asarray(ln2_w, dtype=np.float32)[:, None]
    qwf = np.asarray(q_w, dtype=np.float32) * ln1
    kwf = np.asarray(k_w, dtype=np.float32) * ln1
    vwf = np.asarray(v_w, dtype=np.float32) * ln1
    gatewf = np.asarray(gate_w, dtype=np.float32) * ln2
    guwf = np.asarray(gate_up_w, dtype=np.float32) * ln2[None]    # [E, C, 2F]
    dwf = np.asarray(down_w, dtype=np.float32)                    # [E, F, C]
    owf = np.asarray(o_w, dtype=np.float32)

    cos0 = np.asarray(cos, dtype=np.float32)[0]
    sin0 = np.asarray(sin, dtype=np.float32)[0]
    cosT = np.ascontiguousarray(cos0.T).astype(bf)
    sinT = np.ascontiguousarray(sin0.T).astype(bf)
    protm = np.zeros((128, 128), dtype=np.float32)
    for m in range(64):
        protm[m + 64, m] = -1.0
    for m in range(64, 128):
        protm[m - 64, m] = 1.0

    r = np.arange(128)[:, None]
    col = np.arange(TB)[None, :]
    masks = np.stack(
        [(col >= r + 128 * s).astype(bf) for s in range(4)], axis=1
    )

    # fp8 MoE weights, shared across cores
    # guw host layout: [128, E, FGU*8*2*128]; lhsT slice [128, 2, 128] is
    # (grp, kp) with pair index i selecting k-chunk 2kp+i.
    gu6 = (guwf * WS).astype(f8)                       # [E, C, 2F]
    gu_r = gu6.reshape(E, 8, 2, 128, FGU, 128)         # e, kp, i, p, grp, d
    guw_h = np.ascontiguousarray(
        gu_r.transpose(3, 0, 4, 1, 2, 5).reshape(128, E, FGU * 8 * 2 * 128))
    # dww host layout: [128, CK(m), E*3*2*128]; lhsT slice (e, kp) pair i
    # selects f-chunk 2kp+i; partition p = f % 128; d = c within group m.
    dw6 = (dwf * WS).astype(f8)                        # [E, F, C]
    dw_r = dw6.reshape(E, 3, 2, 128, CK, 128)          # e, kp, i, p, m, d
    dww_h = np.ascontiguousarray(
        dw_r.transpose(3, 4, 0, 1, 2, 5).reshape(128, CK, E * 3 * 2 * 128))

    gatew_h = _chunk_pm(gatewf.astype(np.float16), CK)

    in_maps = []
    for c in range(N_CORES):
        oslice = owf[512 * c:512 * (c + 1), :].astype(bf)  # [512, C]
        o4 = oslice.reshape(HQ, 128, CK, 128)              # hk, p, m, d
        ow_h = np.ascontiguousarray(o4.transpose(1, 2, 0, 3))
        # owned tokens: batch b local [128c, 128c+128)
        own_idx = np.concatenate([
            np.arange(b * T + 128 * c, b * T + 128 * (c + 1)) for b in range(B)
        ])
        xo = x_flat[own_idx, :].T                          # [C, 256]
        xown_h = _chunk_pm(np.ascontiguousarray(xo).astype(np.float16), CK)
        in_maps.append({
            "xT": xT,
            "xown": xown_h,
            "qw": np.ascontiguousarray(
                qwf[:, 512 * c:512 * (c + 1)].astype(bf)
                .reshape(CK, 128, HQ, 128).transpose(1, 2, 0, 3)),
            "kw": _chunk_pm(kwf[:, 128 * c:128 * (c + 1)].astype(bf), CK),
            "vw": _chunk_pm(vwf[:, 128 * c:128 * (c + 1)].astype(bf), CK),
            "ow": ow_h,
            "gatew": gatew_h,
            "guw": guw_h,
            "dww": dww_h,
            "cosb": cosT,
            "sinb": sinT,
            "masks": masks,
            "rstd1": rstd1,
            "qnw": np.asarray(qn_w, dtype=np.float32)[:, None],
            "knw": np.asarray(kn_w, dtype=np.float32)[:, None],
            "protb": protm.astype(bf),
            "sel8": np.kron(np.eye(E, dtype=np.float32),
                            np.ones((1, 128), dtype=np.float32)).astype(bf),
        })

    return in_maps


def combine(ys):
    out = np.zeros((N, C), dtype=np.float32)
    for c in range(N_CORES):
        yc = np.asarray(ys[c], dtype=np.float32)     # [128, CK, 256]
        # yc[p, fc, 128*b + i] -> token b*T + 128*c + i, feature fc*128+p
        feat_major = yc.transpose(1, 0, 2).reshape(C, NOWN)
        for b in range(B):
            toks = slice(b * T + 128 * c, b * T + 128 * (c + 1))
            out[toks, :] = feat_major[:, 128 * b:128 * (b + 1)].T
    return out.reshape(B, T, C)


def kernel(**inputs):
    in_maps = prepare_in_maps(**inputs)
    nc = _get_nc()
    res = run_bass_kernel_spmd(nc, in_maps, core_ids=list(range(N_CORES)))
    return combine([res.results[c]["y"] for c in range(N_CORES)])


# revision 11
# speedup vs baseline: 1.1586x; 1.0038x over previous
"""Fused attention+MoE block on 8 trn2 NeuronCores, v2.

Sharding: tensor-parallel attention (4 q-heads + 1 KV-head per core) as
before, but the attention output partials are ReduceScattered (2 per-batch
collectives) so each core ends up owning 256 tokens of the full hidden
state in feature-major layout. The MoE is then data-parallel: every core
runs all 8 experts densely on its own 256 tokens with fp8e4 DoubleRow
matmuls (2 k-chunks per instruction) and fp8 weights streamed from HBM.
Routing (top-2 over E=8) is computed on an [E, 256] tile with
partition-dim max reductions. Output y = hidden + moe for the owned
tokens; the host just concatenates core slices.
"""

import numpy as np
import ml_dtypes

import concourse.bass as bass
from concourse.bass import _add_dep_helper
import concourse.mybir as mybir
import concourse.tile as tile
from concourse.bass_utils import run_bass_kernel_spmd
from concourse.masks import make_identity
from concourse.vector_clock import ScopedClock

F32 = mybir.dt.float32
F32R = mybir.dt.float32r
F16 = mybir.dt.float16
BF16 = mybir.dt.bfloat16
F8 = mybir.dt.float8e4
U32 = mybir.dt.uint32
AF = mybir.ActivationFunctionType
OP = mybir.AluOpType
PM = mybir.MatmulPerfMode

B, T, C = 2, 1024, 2048
H, KV, D = 32, 8, 128
E, F, TOPK = 8, 768, 2
N = B * T
CK = C // 128          # 16
NBLK = 4               # token blocks of 512
TB = 512
HQ = H // 8            # 4 q heads per core
FK = F // 128          # 6
FGU = 2 * F // 128     # 12
NOWN = 256             # tokens owned per core (128 per batch)
WS = 64.0              # fp8 weight scale
PSC = 16.0             # fp8 prod scale
EPS = 1e-6
N_CORES = 8
BIG = 1e9

# ---------------------------------------------------------------------------
# walrus here rejects >1 sync-wait per instruction; split extras onto NoOps.


class _SplitDrainTileContext(tile.TileContext):
    def _drain_and_barrier(self, tick_clock, wait_clock):
        drain_inst = self.nc.sync.drain()
        wait_clock.add_sem_waits(
            drain_inst.ins, ScopedClock({None: tick_clock.global_clock})
        )
        si = drain_inst.ins.sync_info
        if si is not None and len(si.on_wait) > 1:
            ow = list(si.on_wait)
            drain_inst.ins.sync_info = mybir.SyncInfo(
                on_wait=ow[:1], on_update=list(si.on_update)
            )
            rest = ow[1:]
            while rest:
                extra = self.nc.sync.drain()
                extra.ins.sync_info = mybir.SyncInfo(on_wait=rest[:1], on_update=[])
                rest = rest[1:]
        self.nc.all_engine_barrier()
        assert self.sems is not None
        popped = self.nc._tile_sem_poison_stack.pop()
        assert popped is self._sem_poison
        self.nc.clear_and_free_semaphores(list(self.sems.allocated().values()))
        self.nc.all_engine_barrier()


def _split_multi_waits(nc):
    for bb in nc.main_func.blocks:
        insts = list(bb.instructions)
        out = []
        changed = False
        for ins in insts:
            si = ins.sync_info
            if si is not None and len(si.on_wait) > 1:
                ow = list(si.on_wait)
                for w in ow[:-1]:
                    nop = mybir.InstNoOp(name=f"waitnop-{nc.next_id()}", ins=[], outs=[])
                    nop.engine = ins.engine
                    nop.sync_info = mybir.SyncInfo(on_wait=[w], on_update=[])
                    out.append(nop)
                ins.sync_info = mybir.SyncInfo(
                    on_wait=[ow[-1]], on_update=list(si.on_update)
                )
                changed = True
            out.append(ins)
        if changed:
            bb.instructions = out


# ---------------------------------------------------------------------------


def build_nc(phases='ABC'):
    nc = bass.Bass("TRN2", target_bir_lowering=False, debug=False, num_devices=N_CORES)

    xT = nc.dram_tensor("xT", [128, CK, N], BF16, kind="ExternalInput")
    xown = nc.dram_tensor("xown", [128, CK, NOWN], F16, kind="ExternalInput")
    qw = nc.dram_tensor("qw", [128, HQ, CK, 128], BF16, kind="ExternalInput")
    kw = nc.dram_tensor("kw", [128, CK, 128], BF16, kind="ExternalInput")
    vw = nc.dram_tensor("vw", [128, CK, 128], BF16, kind="ExternalInput")
    ow = nc.dram_tensor("ow", [128, CK, HQ, 128], BF16, kind="ExternalInput")
    gatew = nc.dram_tensor("gatew", [128, CK, E], F16, kind="ExternalInput")
    guw = nc.dram_tensor("guw", [128, E, FGU * 8 * 2 * 128], F8, kind="ExternalInput")
    dww = nc.dram_tensor("dww", [128, CK, E * 3 * 2 * 128], F8, kind="ExternalInput")
    cosb = nc.dram_tensor("cosb", [128, T], BF16, kind="ExternalInput")
    sinb = nc.dram_tensor("sinb", [128, T], BF16, kind="ExternalInput")
    masks = nc.dram_tensor("masks", [128, 4, TB], BF16, kind="ExternalInput")
    rstd1 = nc.dram_tensor("rstd1", [1, N], F32, kind="ExternalInput")
    qnw = nc.dram_tensor("qnw", [128, 1], F32, kind="ExternalInput")
    knw = nc.dram_tensor("knw", [128, 1], F32, kind="ExternalInput")
    protb = nc.dram_tensor("protb", [128, 128], BF16, kind="ExternalInput")
    sel8 = nc.dram_tensor("sel8", [E, E * 128], BF16, kind="ExternalInput")

    y = nc.dram_tensor("y", [128, CK, NOWN], F16, kind="ExternalOutput")

    with _SplitDrainTileContext(nc) as tc:
        with (
            tc.tile_pool(name="const", bufs=1) as cpool,
            tc.tile_pool(name="dram", bufs=1, space="DRAM") as dram,
            tc.tile_pool(name="ps", bufs=1, space="PSUM") as ps,
            tc.tile_pool(name="big", bufs=1) as big,
            tc.tile_pool(name="work", bufs=3) as wk,
        ):
            ab = tc.alloc_tile_pool(name="ab", bufs=1)
            MM = dict(tag="mm", bufs=5)       # f32 [128,TB] psum
            ROW = dict(tag="row", bufs=2)     # f32 [<=8,TB] psum
            TRP = dict(tag="trp", bufs=1)     # bf16 [128,128] psum
            TBF = dict(tag="t512b", bufs=5)   # bf16 [128,TB] transients
            RWF = dict(tag="rowf", bufs=3)    # f32 [1,TB]
            BCS = dict(tag="bcs", bufs=3)     # f32 [128,TB] bcast results
            SM8 = dict(tag="sm8", bufs=2)     # f32 [8,NOWN] small routing tiles
            SM1 = dict(tag="sm1", bufs=3)     # f32 [1,NOWN]
            MC = dict(tag="mc", bufs=4)       # bf16 [128,NOWN] moe transients

            # ---- constants ----
            ident = cpool.tile([128, 128], BF16)
            make_identity(nc, ident)
            ones_bf = cpool.tile([128, 1], BF16)
            nc.vector.memset(ones_bf, 1.0)
            onesrow_f = cpool.tile([1, 128], F32)
            nc.vector.memset(onesrow_f, 1.0)
            onesrow_r = cpool.tile([1, 128], F32R)
            nc.vector.tensor_copy(out=onesrow_r, in_=onesrow_f)
            ones8_f = cpool.tile([1, E], F32)
            nc.vector.memset(ones8_f, 1.0)
            ones8_r = cpool.tile([1, E], F32R)
            nc.vector.tensor_copy(out=ones8_r, in_=ones8_f)
            bias_q = cpool.tile([1, 1], F32)
            nc.vector.memset(bias_q, float(D) * EPS)
            bias_eps = cpool.tile([1, 1], F32)
            nc.vector.memset(bias_eps, EPS)
            idf1 = cpool.tile([1, 1], F32)
            nc.vector.memset(idf1, 1.0)

            xtb0 = ab.tile([128, CK, TB], BF16, tag="xtb", bufs=1, name="xtb0")
            nc.sync.dma_start(out=xtb0[:, 0:CK // 2, :], in_=xT[:, 0:CK // 2, 0:TB])
            x0dma = nc.sync.dma_start(out=xtb0[:, CK // 2:CK, :],
                                      in_=xT[:, CK // 2:CK, 0:TB])
            qw_sb = ab.tile([128, HQ, CK, 128], BF16)
            for _hd in range(HQ):
                nc.sync.dma_start(out=qw_sb[:, _hd, :, :], in_=qw[:, _hd, :, :])
            kw_sb = ab.tile([128, CK, 128], BF16)
            nc.sync.dma_start(out=kw_sb, in_=kw[:])
            vw_sb = ab.tile([128, CK, 128], BF16)
            nc.sync.dma_start(out=vw_sb, in_=vw[:])
            prot_sb = cpool.tile([128, 128], BF16)
            nc.sync.dma_start(out=prot_sb, in_=protb[:])
            cos_sb = cpool.tile([128, T], BF16)
            nc.sync.dma_start(out=cos_sb, in_=cosb[:])
            sin_sb = cpool.tile([128, T], BF16)
            nc.sync.dma_start(out=sin_sb, in_=sinb[:])
            masks_sb = cpool.tile([128, 4, TB], BF16)
            qnw_sb = cpool.tile([128, 1], F32)
            nc.sync.dma_start(out=qnw_sb, in_=qnw[:])
            knw_sb = cpool.tile([128, 1], F32)
            lastconst_dma = nc.sync.dma_start(out=knw_sb, in_=knw[:])
            gatew_sb = cpool.tile([128, CK, E], F16)
            sel8_sb = cpool.tile([E, E * 128], BF16)

            ow_sb = ab.tile([128, CK, HQ, 128], BF16)

            qT_sb = ab.tile([128, HQ, T], BF16)            # per-batch
            kT_sb = ab.tile([128, T], BF16)
            vnat_sb = ab.tile([128, T // 128, 128], BF16)
            rk_sb = ab.tile([128, T // 128], F32)     # per-key-tile rstd_k
            rv_sb = ab.tile([128, T // 128], F32)     # per-key-tile rstd1
            xh_sb = big.tile([128, CK, NOWN], F16)          # own hidden (x+attn)
            xn8_sb = big.tile([128, CK, NOWN], F8)
            comb_row = big.tile([E, NOWN], F32)             # routing weights

            def deferred_const_loads():
                nc.sync.dma_start(out=ow_sb, in_=ow[:])
                nc.sync.dma_start(out=masks_sb, in_=masks[:])
                nc.sync.dma_start(out=gatew_sb, in_=gatew[:])
                nc.sync.dma_start(out=sel8_sb, in_=sel8[:])

            rsin = [dram.tile([8, 128, CK, 128], F16, name=f"rsin{b}") for b in range(B)]
            rsout = [dram.tile([128, CK, 128], F16, name=f"rsout{b}") for b in range(B)]

            def bcast(row_f32_ap, width=TB, act_copy=False, **pool_kw):
                """[1,width] f32 -> SBUF [128,width] f32 via K=1 f32r matmul."""
                rr = wk.tile([1, TB], F32R, tag="rwr", bufs=2)
                nc.vector.tensor_copy(out=rr[:, 0:width], in_=row_f32_ap)
                bc_ps = ps.tile([128, TB], F32, **MM)
                nc.tensor.matmul(bc_ps[:, 0:width], onesrow_r,
                                 rr[:, 0:width], start=True, stop=True)
                kw_ = pool_kw or BCS
                bc = wk.tile([128, TB], F32, **kw_)
                if act_copy:
                    nc.scalar.copy(out=bc[:, 0:width], in_=bc_ps[:, 0:width])
                else:
                    nc.vector.tensor_copy(out=bc[:, 0:width], in_=bc_ps[:, 0:width])
                return bc[:, 0:width]

            def bcast8(row_f32_ap):
                """[1,NOWN] f32 -> SBUF [E,NOWN] f32, exact (fp32 matmul so
                is_equal against the source values still holds bitwise)."""
                p8 = ps.tile([8, TB], F32, **ROW)
                nc.tensor.matmul(p8[0:E, 0:NOWN], ones8_f, row_f32_ap,
                                 start=True, stop=True)
                t8 = wk.tile([E, NOWN], F32, **SM8)
                nc.vector.tensor_copy(out=t8, in_=p8[0:E, 0:NOWN])
                return t8

            def colsum_rstd(feat_ps, scale, bias_ap):
                """rsqrt(scale*colsum(feat^2)+bias) -> [1,TB] f32 row."""
                sq = wk.tile([128, TB], BF16, **TBF)
                nc.scalar.activation(out=sq, in_=feat_ps, func=AF.Square)
                ssum = ps.tile([8, TB], F32, **ROW)
                nc.tensor.matmul(ssum[0:1, :], ones_bf, sq, start=True, stop=True)
                srow = wk.tile([1, TB], F32, **RWF)
                nc.scalar.activation(out=srow, in_=ssum[0:1, :], func=AF.Sqrt,
                                     scale=scale, bias=bias_ap)
                rrow = wk.tile([1, TB], F32, **RWF)
                nc.vector.reciprocal(out=rrow, in_=srow)
                return rrow

            def rope_plain(feat_sb, j, w_sb, out_ap):
                """out = rope(w * feat); rstd applied separately (rope is
                linear per token, so the per-token scale commutes)."""
                tcol = j * TB
                qhat = wk.tile([128, TB], BF16, **TBF)
                nc.vector.tensor_scalar(out=qhat, in0=feat_sb, scalar1=w_sb,
                                        scalar2=None, op0=OP.mult)
                rot_ps = ps.tile([128, TB], F32, **MM)
                nc.tensor.matmul(rot_ps, prot_sb, qhat, start=True, stop=True)
                qc = wk.tile([128, TB], BF16, **TBF)
                nc.vector.tensor_tensor(
                    out=qc, in0=qhat, in1=cos_sb[:, tcol:tcol + TB], op=OP.mult,
                )
                rsm = wk.tile([128, TB], BF16, **TBF)
                nc.vector.tensor_tensor(
                    out=rsm, in0=rot_ps, in1=sin_sb[:, tcol:tcol + TB], op=OP.mult,
                )
                nc.vector.tensor_tensor(out=out_ap, in0=qc, in1=rsm, op=OP.add)

            # =========================== Phase A: QKV ======================
            def phaseA(g):
                b, j = divmod(g, 2)
                n0 = g * TB
                if g == 0:
                    xtb = xtb0
                else:
                    xtb = ab.tile([128, CK, TB], BF16, tag="xtb", bufs=1)
                    nc.sync.dma_start(out=xtb[:, 0:CK // 2, :],
                                      in_=xT[:, 0:CK // 2, n0:n0 + TB])
                    nc.sync.dma_start(out=xtb[:, CK // 2:CK, :],
                                      in_=xT[:, CK // 2:CK, n0:n0 + TB])
                r1row = wk.tile([1, TB], F32, **RWF)
                nc.sync.dma_start(out=r1row, in_=rstd1[:, n0:n0 + TB])
                for cc in range(TB // 128):
                    r1t = ps.tile([128, TB], F32, **MM)
                    nc.tensor.transpose(r1t[:, 0:1],
                                        r1row[:, cc * 128:(cc + 1) * 128], idf1)
                    nc.vector.tensor_copy(
                        out=rv_sb[:, 4 * j + cc:4 * j + cc + 1],
                        in_=r1t[:, 0:1])

                def accum(kind, hd):
                    p = ps.tile([128, TB], F32, **MM)
                    for kc in range(CK):
                        if kind == 'q':
                            lhs = qw_sb[:, hd, kc, :]
                        elif kind == 'k':
                            lhs = kw_sb[:, kc, :]
                        else:
                            lhs = vw_sb[:, kc, :]
                        nc.tensor.matmul(p, lhs, xtb[:, kc, :],
                                         start=(kc == 0), stop=(kc == CK - 1))
                    return p

                def stage2(kind, p):
                    # free the psum early: rope reads the bf16 SBUF copy
                    if kind == 'q':
                        row = colsum_rstd(p, 1.0, bias_q)
                    elif kind == 'k':
                        row = colsum_rstd(p, 1.0 / D, bias_eps)
                    else:
                        row = None
                    fsb = wk.tile([128, TB], BF16, tag="fsb", bufs=3)
                    nc.scalar.copy(out=fsb, in_=p)
                    return (row, fsb)

                def stage3(kind, hd, st2):
                    row, fsb = st2
                    if kind == 'q':
                        qbc = bcast(row[:], act_copy=True)
                        qr = wk.tile([128, TB], BF16, **TBF)
                        rope_plain(fsb, j, qnw_sb, qr)
                        nc.vector.tensor_tensor(
                            out=qT_sb[:, hd, j * TB:j * TB + TB],
                            in0=qr, in1=qbc, op=OP.mult)
                    elif kind == 'k':
                        rope_plain(fsb, j, knw_sb,
                                   kT_sb[:, j * TB:j * TB + TB])
                        for cc in range(TB // 128):
                            rkt = ps.tile([128, TB], F32, **MM)
                            nc.tensor.transpose(
                                rkt[:, 0:1], row[:, cc * 128:(cc + 1) * 128],
                                idf1)
                            nc.vector.tensor_copy(
                                out=rk_sb[:, 4 * j + cc:4 * j + cc + 1],
                                in_=rkt[:, 0:1])
                    else:
                        # x's rstd applied post-transpose, where token is the
                        # partition dim (per-partition scalar, no broadcast)
                        for cc in range(TB // 128):
                            vtr = ps.tile([128, 128], BF16, **TRP)
                            nc.tensor.transpose(
                                vtr, fsb[:, cc * 128:(cc + 1) * 128], ident)
                            nc.vector.tensor_scalar(
                                out=vnat_sb[:, j * 4 + cc, :], in0=vtr,
                                scalar1=rv_sb[:, 4 * j + cc:4 * j + cc + 1],
                                scalar2=None, op0=OP.mult)

                seq = [('q', 0), ('q', 1), ('q', 2), ('q', 3),
                       ('k', None), ('v', None)]
                st = []
                for idx, (kind, hd) in enumerate(seq):
                    p = accum(kind, hd)
                    st.append([kind, hd, p, None])
                    if idx >= 1:
                        st[idx - 1][3] = stage2(st[idx - 1][0], st[idx - 1][2])
                    if idx >= 2:
                        stage3(st[idx - 2][0], st[idx - 2][1], st[idx - 2][3])
                st[-1][3] = stage2(st[-1][0], st[-1][2])
                stage3(st[-2][0], st[-2][1], st[-2][3])
                stage3(st[-1][0], st[-1][1], st[-1][3])

            # ====================== Phase B: attention =====================
            def phaseB(g):
                b, j = divmod(g, 2)
                q0 = j * TB
                ntk = 4 * j + 4
                avT = ab.tile([128, HQ, TB], BF16, tag="avT", bufs=1)

                def fin(hd, av_sb, den_ps):
                    dsb = wk.tile([1, TB], F32, **RWF)
                    nc.vector.tensor_copy(out=dsb, in_=den_ps[0:1, :])
                    rec = wk.tile([1, TB], F32, **RWF)
                    nc.vector.reciprocal(out=rec, in_=dsb)
                    rbc = bcast(rec[:])
                    nc.vector.tensor_tensor(out=avT[:, hd, :], in0=av_sb,
                                            in1=rbc, op=OP.mult)

                pend = None
                for hd in range(HQ):
                    av_ps = ps.tile([128, TB], F32, **MM)
                    den_ps = ps.tile([8, TB], F32, **ROW)
                    for i in range(ntk):
                        tk0 = i * 128
                        s = i - 4 * j
                        # diagonal tiles: columns < 128*s are fully masked, so
                        # restrict score/exp/den/av to the live column range
                        c0 = max(s, 0) * 128
                        w = TB - c0
                        sc_ps = ps.tile([128, TB], F32, **MM)
                        nc.tensor.matmul(sc_ps[:, c0:TB], kT_sb[:, tk0:tk0 + 128],
                                         qT_sb[:, hd, q0 + c0:q0 + TB],
                                         start=True, stop=True,
                                         skip_group_check=True)
                        ex = wk.tile([128, TB], BF16, **TBF)
                        if s < 0:
                            nc.scalar.activation(out=ex, in_=sc_ps, func=AF.Exp,
                                                 scale=rk_sb[:, i:i + 1])
                        else:
                            ext = wk.tile([128, TB], BF16, **TBF)
                            nc.scalar.activation(out=ext[:, c0:TB],
                                                 in_=sc_ps[:, c0:TB], func=AF.Exp,
                                                 scale=rk_sb[:, i:i + 1])
                            nc.vector.tensor_tensor(out=ex[:, c0:TB],
                                                    in0=ext[:, c0:TB],
                                                    in1=masks_sb[:, s, c0:TB],
                                                    op=OP.mult)
                        nc.tensor.matmul(den_ps[0:1, c0:TB], ones_bf,
                                         ex[:, c0:TB],
                                         start=(i == 0), stop=(i == ntk - 1),
                                         skip_group_check=True)
                        nc.tensor.matmul(av_ps[:, c0:TB], vnat_sb[:, i, :],
                                         ex[:, c0:TB],
                                         start=(i == 0), stop=(i == ntk - 1),
                                         skip_group_check=True)
                    # free the av psum early via an Act copy; fin reads SBUF
                    av_sb = wk.tile([128, TB], F32, **BCS)
                    nc.scalar.copy(out=av_sb, in_=av_ps)
                    if pend is not None:
                        fin(*pend)
                    pend = (hd, av_sb, den_ps)
                fin(*pend)
                for mq in range(4):
                    attq = ab.tile([128, 4, TB], F16, tag="attb", bufs=2)
                    for mi in range(4):
                        m = 4 * mq + mi
                        att_ps = ps.tile([128, TB], F32, **MM)
                        for hk in range(HQ):
                            nc.tensor.matmul(att_ps, ow_sb[:, m, hk, :],
                                             avT[:, hk, :], start=(hk == 0),
                                             stop=(hk == HQ - 1))
                        nc.scalar.copy(out=attq[:, mi, :], in_=att_ps)
                    for tg in range(4):
                        nc.sync.dma_start(
                            out=rsin[b][4 * j + tg, :, 4 * mq:4 * mq + 4, :],
                            in_=attq[:, :, tg * 128:(tg + 1) * 128])
                if j == 1:
                    nc.gpsimd.collective_compute(
                        "ReduceScatter", OP.add,
                        replica_groups=[list(range(N_CORES))],
                        ins=[rsin[b].opt()], outs=[rsout[b].opt()],
                    )

            # ========================= Phase C: MoE ========================
            HGU = FK * 8 * 2 * 128    # flat size of one gate (or up) half

            def load_guw_half(e, half, eng=None, after=None):
                t = wk.tile([128, FK * 8, 2, 128], F8, tag="wgu", bufs=4)
                d = (eng or nc.gpsimd).dma_start(
                    out=t,
                    in_=guw[:, e, half * HGU:(half + 1) * HGU].rearrange(
                        "p (a b c) -> p a b c", b=2, c=128))
                if after is not None:
                    _add_dep_helper(d.ins, after.ins, sync=True,
                                    reason="prefetch after startup loads")
                return t

            def load_dww(m, eng=None, after=None):
                t = wk.tile([128, E * 3, 2, 128], F8, tag="wdw", bufs=2)
                d = (eng or nc.sync).dma_start(out=t, in_=dww[:, m, :].rearrange(
                    "p (a b c) -> p a b c", b=2, c=128))
                if after is not None:
                    _add_dep_helper(d.ins, after.ins, sync=True,
                                    reason="prefetch after startup loads")
                return t

            def phaseC(pre_gu, pre_dw, moe):
                # assemble own hidden = attn partial sums (+x) for owned tokens
                for b in range(B):
                    nc.sync.dma_start(out=xh_sb[:, :, 128 * b:128 * b + 128],
                                      in_=rsout[b][:])
                prod8_sb = moe.tile([128, E, FK, NOWN], F8, name="prod8_sb")
                xow = moe.tile([128, CK, NOWN], F16, name="xow_all")
                nc.sync.dma_start(out=xow, in_=xown[:])
                # per-batch halves: batch 0's adds run during the RS1 wait
                for b in range(B):
                    cs = slice(128 * b, 128 * b + 128)
                    for fc in range(CK):
                        nc.vector.tensor_tensor(out=xh_sb[:, fc, cs],
                                                in0=xh_sb[:, fc, cs],
                                                in1=xow[:, fc, cs], op=OP.add)
                # gate/stats/xn8 per batch half: batch 0's half runs
                # inside the batch-1 ReduceScatter window
                lg = wk.tile([E, NOWN], F32, tag="lg", bufs=1)
                rstd2 = wk.tile([1, NOWN], F32, tag="rstd2", bufs=1)
                r2bc_t = wk.tile([128, TB], F32, **BCS)
                for hb in range(B):
                    cs = slice(128 * hb, 128 * hb + 128)
                    lg_ps = ps.tile([8, TB], F32, **ROW)
                    den2_ps = ps.tile([8, TB], F32, **ROW)
                    for fc in range(CK):
                        nc.tensor.matmul(lg_ps[0:E, 0:128], gatew_sb[:, fc, :],
                                         xh_sb[:, fc, cs],
                                         start=(fc == 0), stop=(fc == CK - 1))
                        sq = wk.tile([128, NOWN], BF16, **MC)
                        nc.scalar.activation(out=sq[:, 0:128],
                                             in_=xh_sb[:, fc, cs], func=AF.Square)
                        nc.tensor.matmul(den2_ps[0:1, 0:128], ones_bf,
                                         sq[:, 0:128],
                                         start=(fc == 0), stop=(fc == CK - 1))
                    nc.vector.tensor_copy(out=lg[:, cs], in_=lg_ps[0:E, 0:128])
                    s2 = wk.tile([1, NOWN], F32, **SM1)
                    nc.scalar.activation(out=s2[:, 0:128],
                                         in_=den2_ps[0:1, 0:128], func=AF.Sqrt,
                                         scale=1.0 / C, bias=bias_eps)
                    nc.vector.reciprocal(out=rstd2[:, cs], in_=s2[:, 0:128])
                    rr = wk.tile([1, TB], F32R, tag="rwr", bufs=2)
                    nc.vector.tensor_copy(out=rr[:, 0:128], in_=rstd2[:, cs])
                    bc_ps = ps.tile([128, TB], F32, **MM)
                    nc.tensor.matmul(bc_ps[:, 0:128], onesrow_r, rr[:, 0:128],
                                     start=True, stop=True)
                    nc.scalar.copy(out=r2bc_t[:, cs], in_=bc_ps[:, 0:128])
                    for fc in range(CK):
                        nc.vector.tensor_tensor(out=xn8_sb[:, fc, cs],
                                                in0=xh_sb[:, fc, cs],
                                                in1=r2bc_t[:, cs], op=OP.mult)

                # ---- top-2 routing on [E, NOWN] ----
                m1 = wk.tile([1, NOWN], F32, **SM1)
                nc.gpsimd.tensor_reduce(out=m1, in_=lg, axis=mybir.AxisListType.C,
                                        op=OP.max)
                m1bc = bcast8(m1[:])
                eq1 = wk.tile([E, NOWN], F32, tag="eq1", bufs=1)
                nc.vector.tensor_tensor(out=eq1, in0=lg, in1=m1bc, op=OP.is_equal)
                lg2 = wk.tile([E, NOWN], F32, **SM8)
                nc.vector.scalar_tensor_tensor(out=lg2, in0=eq1, scalar=-BIG,
                                               in1=lg, op0=OP.mult, op1=OP.add)
                m2 = wk.tile([1, NOWN], F32, **SM1)
                nc.gpsimd.tensor_reduce(out=m2, in_=lg2, axis=mybir.AxisListType.C,
                                        op=OP.max)
                m2bc = bcast8(m2[:])
                eq2 = wk.tile([E, NOWN], F32, **SM8)
                nc.vector.tensor_tensor(out=eq2, in0=lg, in1=m2bc, op=OP.is_equal)
                # dlt = (m1-m2)*rstd2 ; w1 = sigmoid(dlt); w2 = 1-w1
                dlt = wk.tile([1, NOWN], F32, **SM1)
                nc.vector.tensor_tensor(out=dlt, in0=m1, in1=m2, op=OP.subtract)
                dlts = wk.tile([1, NOWN], F32, **SM1)
                nc.vector.tensor_tensor(out=dlts, in0=dlt, in1=rstd2, op=OP.mult)
                w1 = wk.tile([1, NOWN], F32, **SM1)
                nc.scalar.activation(out=w1, in_=dlts, func=AF.Sigmoid)
                w1bc = bcast8(w1[:])
                # comb = eq1*w1 + eq2*(1-w1) = (eq1-eq2)*w1 + eq2, in place
                nc.vector.tensor_tensor(out=eq1, in0=eq1, in1=eq2, op=OP.subtract)
                nc.vector.tensor_tensor(out=eq1, in0=eq1, in1=w1bc, op=OP.mult)
                nc.vector.tensor_tensor(out=eq1, in0=eq1, in1=eq2, op=OP.add)
                # scale by PSC/WS (prod fp8 scale / up-weight descale)
                nc.vector.tensor_scalar(out=comb_row, in0=eq1, scalar1=PSC / WS,
                                        scalar2=None, op0=OP.mult)

                # ---- pass 1: gate/up + silu -> prod8 per expert ----
                def load_guw_moe(e, half, eng):
                    t = moe.tile([128, FK * 8, 2, 128], F8, tag="wgu2", bufs=4)
                    eng.dma_start(
                        out=t,
                        in_=guw[:, e, half * HGU:(half + 1) * HGU].rearrange(
                            "p (a b c) -> p a b c", b=2, c=128))
                    return t

                def load_dww_moe(m, eng):
                    t = moe.tile([128, E * 3, 2, 128], F8, tag="wdw2", bufs=3)
                    eng.dma_start(out=t, in_=dww[:, m, :].rearrange(
                        "p (a b c) -> p a b c", b=2, c=128))
                    return t

                comb_bf = wk.tile([E, NOWN], BF16, tag="combbf", bufs=1)
                nc.vector.tensor_copy(out=comb_bf, in_=comb_row)
                for e in range(E):
                    wgg = pre_gu.pop((e, 0), None) or load_guw_moe(e, 0, nc.sync)
                    wgu = pre_gu.pop((e, 1), None) or load_guw_moe(e, 1, nc.scalar)
                    cb_ps = ps.tile([128, TB], F32, **MM)
                    nc.tensor.matmul(cb_ps[:, 0:NOWN],
                                     sel8_sb[:, e * 128:(e + 1) * 128],
                                     comb_bf, start=True, stop=True)
                    cbc = wk.tile([128, NOWN], F32, tag="cbc", bufs=1)
                    nc.vector.tensor_copy(out=cbc, in_=cb_ps[:, 0:NOWN])
                    for f in range(FK):
                        g_ps = ps.tile([128, TB], F32, **MM)
                        for kp in range(8):
                            nc.tensor.matmul(
                                g_ps[:, 0:NOWN], wgg[:, f * 8 + kp, :, :],
                                xn8_sb[:, 2 * kp:2 * kp + 2, :],
                                start=(kp == 0), stop=(kp == 7),
                                perf_mode=PM.DoubleRow,
                            )
                        u_ps = ps.tile([128, TB], F32, **MM)
                        for kp in range(8):
                            nc.tensor.matmul(
                                u_ps[:, 0:NOWN], wgu[:, f * 8 + kp, :, :],
                                xn8_sb[:, 2 * kp:2 * kp + 2, :],
                                start=(kp == 0), stop=(kp == 7),
                                perf_mode=PM.DoubleRow,
                            )
                        sil = wk.tile([128, NOWN], BF16, **MC)
                        nc.scalar.activation(out=sil, in_=g_ps[:, 0:NOWN],
                                             func=AF.Silu, scale=1.0 / WS)
                        ucm = wk.tile([128, NOWN], BF16, **MC)
                        nc.vector.tensor_tensor(out=ucm, in0=u_ps[:, 0:NOWN],
                                                in1=cbc, op=OP.mult)
                        nc.vector.tensor_tensor(out=prod8_sb[:, e, f, :],
                                                in0=sil, in1=ucm, op=OP.mult)

                # ---- pass 2: down proj, accumulate experts in psum ----
                for m in range(CK):
                    wd = pre_dw.pop(m, None) or load_dww_moe(m, nc.sync)
                    eo_ps = ps.tile([128, TB], F32, **MM)
                    for e in range(E):
                        for kp in range(3):
                            nc.tensor.matmul(
                                eo_ps[:, 0:NOWN], wd[:, e * 3 + kp, :, :],
                                prod8_sb[:, e, 2 * kp:2 * kp + 2, :],
                                start=(e == 0 and kp == 0),
                                stop=(e == E - 1 and kp == 2),
                                perf_mode=PM.DoubleRow,
                            )
                    ym = wk.tile([128, NOWN], F16, tag="ymc", bufs=2)
                    nc.vector.scalar_tensor_tensor(
                        out=ym, in0=eo_ps[:, 0:NOWN],
                        scalar=1.0 / (WS * PSC), in1=xh_sb[:, m, :],
                        op0=OP.mult, op1=OP.add,
                    )
                    nc.sync.dma_start(out=y[:, m, :], in_=ym)

            pre_gu, pre_dw = {}, {}
            for g in range(NBLK):
                if 'A' in phases:
                    phaseA(g)
                if g == 0:
                    deferred_const_loads()
                    if 'C' in phases:
                        # act-queue prefetches: the Act sequencer reaches these
                        # only after A0's first Square, keeping the DMA engines
                        # free for the critical startup loads
                        for e in range(2):
                            for half in range(2):
                                pre_gu[(e, half)] = load_guw_half(
                                    e, half, nc.scalar, after=lastconst_dma)
                        pre_dw[0] = load_dww(0, nc.scalar, after=lastconst_dma)
                        pre_dw[1] = load_dww(1, nc.scalar, after=lastconst_dma)
                if 'B' in phases:
                    phaseB(g)
            ab.release()
            if 'C' in phases:
                with tc.tile_pool(name="moe", bufs=1) as moe:
                    phaseC(pre_gu, pre_dw, moe)

    _split_multi_waits(nc)
    return nc


# ---------------------------------------------------------------------------

_NC_CACHE = {}


def _get_nc():
    if "nc" not in _NC_CACHE:
        _NC_CACHE["nc"] = build_nc()
    return _NC_CACHE["nc"]


def _chunk_pm(a, nchunk):
    """[nchunk*128, free...] -> [128, nchunk, free...]"""
    return np.ascontiguousarray(
        a.reshape(nchunk, 128, *a.shape[1:]).transpose(1, 0, *range(2, a.ndim + 1))
    )


def prepare_in_maps(x, cos, sin, ln1_w, q_w, k_w, v_w, o_w, qn_w, kn_w, ln2_w,
                    gate_w, gate_up_w, down_w):
    bf = ml_dtypes.bfloat16
    f8 = ml_dtypes.float8_e4m3
    x = np.asarray(x, dtype=np.float32)
    x_flat = x.reshape(N, C)

    xT = _chunk_pm(np.ascontiguousarray(x_flat.T).astype(bf), CK)
    rstd1 = (1.0 / np.sqrt((x_flat.astype(np.float64) ** 2).mean(axis=1) + EPS)
             ).astype(np.float32)[None, :]

    ln1 = np.asarray(ln1_w, dtype=np.float32)[:, None]
    ln2 = np.asarray(ln2_w, dtype=np.float32)[:, None]
    qwf = np.asarray(q_w, dtype=np.float32) * ln1
    kwf = np.asarray(k_w, dtype=np.float32) * ln1
    vwf = np.asarray(v_w, dtype=np.float32) * ln1
    gatewf = np.asarray(gate_w, dtype=np.float32) * ln2
    guwf = np.asarray(gate_up_w, dtype=np.float32) * ln2[None]    # [E, C, 2F]
    dwf = np.asarray(down_w, dtype=np.float32)                    # [E, F, C]
    owf = np.asarray(o_w, dtype=np.float32)

    cos0 = np.asarray(cos, dtype=np.float32)[0]
    sin0 = np.asarray(sin, dtype=np.float32)[0]
    cosT = np.ascontiguousarray(cos0.T).astype(bf)
    sinT = np.ascontiguousarray(sin0.T).astype(bf)
    protm = np.zeros((128, 128), dtype=np.float32)
    for m in range(64):
        protm[m + 64, m] = -1.0
    for m in range(64, 128):
        protm[m - 64, m] = 1.0

    r = np.arange(128)[:, None]
    col = np.arange(TB)[None, :]
    masks = np.stack(
        [(col >= r + 128 * s).astype(bf) for s in range(4)], axis=1
    )

    # fp8 MoE weights, shared across cores
    # guw host layout: [128, E, FGU*8*2*128]; lhsT slice [128, 2, 128] is
    # (grp, kp) with pair index i selecting k-chunk 2kp+i.
    gu6 = (guwf * WS).astype(f8)                       # [E, C, 2F]
    gu_r = gu6.reshape(E, 8, 2, 128, FGU, 128)         # e, kp, i, p, grp, d
    guw_h = np.ascontiguousarray(
        gu_r.transpose(3, 0, 4, 1, 2, 5).reshape(128, E, FGU * 8 * 2 * 128))
    # dww host layout: [128, CK(m), E*3*2*128]; lhsT slice (e, kp) pair i
    # selects f-chunk 2kp+i; partition p = f % 128; d = c within group m.
    dw6 = (dwf * WS).astype(f8)                        # [E, F, C]
    dw_r = dw6.reshape(E, 3, 2, 128, CK, 128)          # e, kp, i, p, m, d
    dww_h = np.ascontiguousarray(
        dw_r.transpose(3, 4, 0, 1, 2, 5).reshape(128, CK, E * 3 * 2 * 128))

    gatew_h = _chunk_pm(gatewf.astype(np.float16), CK)

    in_maps = []
    for c in range(N_CORES):
        oslice = owf[512 * c:512 * (c + 1), :].astype(bf)  # [512, C]
        o4 = oslice.reshape(HQ, 128, CK, 128)              # hk, p, m, d
        ow_h = np.ascontiguousarray(o4.transpose(1, 2, 0, 3))
        # owned tokens: batch b local [128c, 128c+128)
        own_idx = np.concatenate([
            np.arange(b * T + 128 * c, b * T + 128 * (c + 1)) for b in range(B)
        ])
        xo = x_flat[own_idx, :].T                          # [C, 256]
        xown_h = _chunk_pm(np.ascontiguousarray(xo).astype(np.float16), CK)
        in_maps.append({
            "xT": xT,
            "xown": xown_h,
            "qw": np.ascontiguousarray(
                qwf[:, 512 * c:512 * (c + 1)].astype(bf)
                .reshape(CK, 128, HQ, 128).transpose(1, 2, 0, 3)),
            "kw": _chunk_pm(kwf[:, 128 * c:128 * (c + 1)].astype(bf), CK),
            "vw": _chunk_pm(vwf[:, 128 * c:128 * (c + 1)].astype(bf), CK),
            "ow": ow_h,
            "gatew": gatew_h,
            "guw": guw_h,
            "dww": dww_h,
            "cosb": cosT,
            "sinb": sinT,
            "masks": masks,
            "rstd1": rstd1,
            "qnw": np.asarray(qn_w, dtype=np.float32)[:, None],
            "knw": np.asarray(kn_w, dtype=np.float32)[:, None],
            "protb": protm.astype(bf),
            "sel8": np.kron(np.eye(E, dtype=np.float32),
                            np.ones((1, 128), dtype=np.float32)).astype(bf),
        })

    return in_maps


def combine(ys):
    out = np.zeros((N, C), dtype=np.float32)
    for c in range(N_CORES):
        yc = np.asarray(ys[c], dtype=np.float32)     # [128, CK, 256]
        # yc[p, fc, 128*b + i] -> token b*T + 128*c + i, feature fc*128+p
        feat_major = yc.transpose(1, 0, 2).reshape(C, NOWN)
        for b in range(B):
            toks = slice(b * T + 128 * c, b * T + 128 * (c + 1))
            out[toks, :] = feat_major[:, 128 * b:128 * (b + 1)].T
    return out.reshape(B, T, C)


def kernel(**inputs):
    in_maps = prepare_in_maps(**inputs)
    nc = _get_nc()
    res = run_bass_kernel_spmd(nc, in_maps, core_ids=list(range(N_CORES)))
    return combine([res.results[c]["y"] for c in range(N_CORES)])
